# revision 28
# baseline (speedup 1.0000x reference)
"""Trainium2 Bass kernel for nn_AutoEncoder_31533649887292.

8-core SPMD plan (uniform program, per-core data):
  - encoder replicated on all cores (serial conv chain, tap-accumulated matmuls)
  - lt1 (43008->512, but cols 17408..43008 multiply zeros -> dropped):
    K-sharded 8-way, partials AllReduce'd (512 floats)
  - lt2/lt3/rev1/rev2 replicated (output-on-partition matvec layout)
  - rev3 (512->43008) output-sharded 8-way + AllGather (fp16)
  - decoders run sequentially, replicated; per-decoder fc1 output-sharded
    8-way with ONE fused AllGather for both decoders
  - conv_out (1->1 conv) folded into fc1 weights host-side
  - numerics: fp16 matmul operands, fp32 PSUM/stats; compensated scales
    S_REV3=64 (undone inside dec rb2 weights) and S_FC=256 (undone at output)
"""
import numpy as np
import ml_dtypes

import concourse.bacc as bacc
import concourse.mybir as mybir
import concourse.tile as tile
from concourse.bass_utils import run_bass_kernel_spmd

F16 = mybir.dt.float16
F32 = mybir.dt.float32
NP16 = np.float16

N_CORES = 8
EPS = 1e-5
ALPHA = 0.01
S_REV3 = 64.0
S_FC = 256.0

H = 32
W0, W1 = 65, 98           # output widths
WD0, WD1 = 68, 100        # decoder entry widths (H=2)
FLAT0 = 17408             # e0 flatten / s0 size
NK_LT1 = 17               # 2176/128 k-chunks per core


# ----------------------------------------------------------------------------
# host-side weight packing helpers
# ----------------------------------------------------------------------------

def pack_conv(w):
    """w (Cout, Cin, 3, 3) -> lhsT pack (Cin, 9*Cout), tap t=3dy+dx."""
    Cout, Cin = w.shape[0], w.shape[1]
    out = np.zeros((Cin, 9 * Cout), NP16)
    for dy in range(3):
        for dx in range(3):
            t = 3 * dy + dx
            out[:, t * Cout:(t + 1) * Cout] = w[:, :, dy, dx].T
    return out


def pack_convt(w):
    """w (Cin, Cout, 3, 3) -> (Cin, 9*Cout), tap t=3ky+kx, already lhsT."""
    Cin, Cout = w.shape[0], w.shape[1]
    out = np.zeros((Cin, 9 * Cout), NP16)
    for ky in range(3):
        for kx in range(3):
            t = 3 * ky + kx
            out[:, t * Cout:(t + 1) * Cout] = w[:, :, ky, kx]
    return out


def pack_matvec(wT, nk, nm):
    """wT (K, N) (K=128*nk, N=128*nm) -> (128, nk*nm*128) block pack:
    block (k, m) at cols (k*nm+m)*128."""
    K, N = wT.shape
    out = np.zeros((128, nk * nm * 128), NP16)
    for k in range(nk):
        for m in range(nm):
            blk = wT[k * 128:(k + 1) * 128, m * 128:(m + 1) * 128]
            out[:blk.shape[0], (k * nm + m) * 128:(k * nm + m) * 128 + blk.shape[1]] = blk
    return out


def col1(v, dtype=np.float32):
    return np.ascontiguousarray(np.asarray(v, dtype).reshape(-1, 1))


def build_convout_fold(fc1_w, fc1_b, w_out, b_out, Hh, Wh):
    n = Hh * Wh
    C = np.zeros((n, n), np.float32)
    w = np.asarray(w_out)[0, 0]
    idx = np.arange(n).reshape(Hh, Wh)
    ys, xs = np.meshgrid(np.arange(Hh), np.arange(Wh), indexing='ij')
    for dy in range(3):
        for dx in range(3):
            yi, xi = ys + dy - 1, xs + dx - 1
            valid = (yi >= 0) & (yi < Hh) & (xi >= 0) & (xi < Wh)
            C[idx[ys[valid], xs[valid]], idx[yi[valid], xi[valid]]] += w[dy, dx]
    fc1_w = np.asarray(fc1_w, np.float32)
    new_w = fc1_w @ C
    new_b = np.asarray(fc1_b, np.float32) + fc1_w @ (np.float32(b_out[0]) * np.ones(n, np.float32))
    return new_w, new_b


# ----------------------------------------------------------------------------
# device program
# ----------------------------------------------------------------------------

class Ctx:
    pass


def emit_conv(g, name, src, dst, Cin, Cout, Hin, Win, stride, w_ap, b_ap,
              act, rows_per_tile=None, extra_ident_rhs=None):
    """Tap-accumulated 3x3 conv.
    src: padded fp16 tile (Cin, Hin+2, Win+2); dst padded fp16 tile or None.
    b_ap: f16 ROW bias (1, Cout), folded into psum via ones-matmul.
    act: 'lrelu' | 'none'. extra_ident_rhs: AP (Cout, Hout, Wout) added via
    identity matmul (residual). Returns list of (psum_ap, y0, nrows) if dst
    is None (caller evicts)."""
    nc = g.nc
    Hout = (Hin + stride - 1) // stride
    Wout = (Win + stride - 1) // stride
    if rows_per_tile is None:
        rows_per_tile = max(1, 512 // Wout)
    tiles = []
    y0 = 0
    while y0 < Hout:
        nr = min(rows_per_tile, Hout - y0)
        ps = g.psum.tile([Cout, nr, Wout], F32, tag="mm")
        mi = 0
        for dy in range(3):
            for dx in range(3):
                t = 3 * dy + dx
                rhs = src[0:Cin,
                          dy + stride * y0: dy + stride * (y0 + nr - 1) + 1: stride,
                          dx: dx + stride * (Wout - 1) + 1: stride]
                nc.tensor.matmul(ps[:], w_ap[:, t * Cout:(t + 1) * Cout], rhs,
                                 start=(mi == 0), stop=False)
                mi += 1
        if extra_ident_rhs is not None:
            nc.tensor.matmul(ps[:], g.ident[0:Cout, 0:Cout],
                             extra_ident_rhs[0:Cout, y0:y0 + nr, 0:Wout],
                             start=False, stop=False)
        # bias broadcast into psum: lhsT = bias row (1, Cout), rhs = ones (1, N)
        nc.tensor.matmul(ps[:], b_ap, g.ones[0:1, 0:nr * Wout],
                         start=False, stop=True)
        if dst is not None:
            emit_act(g, dst[0:Cout, 1 + y0:1 + y0 + nr, 1:1 + Wout], ps,
                     Cout, nr * Wout, act)
        tiles.append((ps, y0, nr))
        y0 += nr
    return tiles


def emit_act(g, dst_ap, ps, C, n, act):
    """dst = lrelu(ps) (or copy). lrelu = max(0.01*ps, ps): ACT mul + DVE max."""
    nc = g.nc
    if act == 'lrelu':
        tmp = g.sbuf.tile([128, 512], F32, tag="evtmp")
        nc.scalar.mul(tmp[0:C, 0:n], ps[:], ALPHA)
        nc.vector.tensor_max(dst_ap, tmp[0:C, 0:n], ps[:])
    else:
        nc.scalar.copy(dst_ap, ps[:])


def zero_border(g, buf, C, Hp, Wp):
    """zero only the 1-px border of a padded (C, Hp, Wp) buffer."""
    nc = g.nc
    nc.gpsimd.memset(buf[0:C, 0:1, :], 0.0)
    nc.gpsimd.memset(buf[0:C, Hp - 1:Hp, :], 0.0)
    nc.gpsimd.memset(buf[0:C, 1:Hp - 1, 0:1], 0.0)
    nc.gpsimd.memset(buf[0:C, 1:Hp - 1, Wp - 1:Wp], 0.0)


def emit_bn(g, ds_tiles, C, npx, b_ap, g_ap, bb_ap, dsf32, ds16_dst):
    """BN with batch stats. ds_tiles: psum tiles from ds conv (list of
    (ps, y0, nr) covering (C, H, W)); evict to dsf32 (C, npx-ish 3D or 2D)
    with accum sums; then stats + apply -> ds16_dst (fp16)."""
    nc = g.nc
    nt = len(ds_tiles)
    acc = g.sbuf.tile([C, nt], F32, tag="bn_acc")
    for i, (ps, y0, nr) in enumerate(ds_tiles):
        nc.scalar.activation(dsf32[0:C, y0:y0 + nr, :], ps[:],
                             mybir.ActivationFunctionType.Identity,
                             bias=b_ap, scale=1.0,
                             accum_out=acc[:, i:i + 1])
    ssum = g.sbuf.tile([C, 1], F32, tag="bn_s")
    if nt > 1:
        nc.vector.tensor_reduce(ssum[:], acc[:], mybir.AxisListType.X,
                                mybir.AluOpType.add)
    else:
        nc.vector.tensor_copy(ssum[:], acc[:])
    sq = g.sbuf.tile([C, 1], F32, tag="bn_sq")
    scr = g.scratch  # (128, 2080) f32 scratch
    nc.scalar.activation(scr[0:C, 0:npx], dsf32[0:C].opt(),
                         mybir.ActivationFunctionType.Square,
                         accum_out=sq[:])
    inv_n = 1.0 / npx
    mean = g.sbuf.tile([C, 1], F32, tag="bn_m")
    nc.scalar.mul(mean[:], ssum[:], inv_n)
    ex2 = g.sbuf.tile([C, 1], F32, tag="bn_e")
    nc.scalar.mul(ex2[:], sq[:], inv_n)
    m2 = g.sbuf.tile([C, 1], F32, tag="bn_m2")
    nc.vector.tensor_mul(m2[:], mean[:], mean[:])
    var = g.sbuf.tile([C, 1], F32, tag="bn_v")
    nc.vector.tensor_sub(var[:], ex2[:], m2[:])
    nc.vector.tensor_scalar_add(var[:], var[:], EPS)
    std = g.sbuf.tile([C, 1], F32, tag="bn_std")
    nc.scalar.activation(std[:], var[:], mybir.ActivationFunctionType.Sqrt,
                         bias=0.0, scale=1.0)
    istd = g.sbuf.tile([C, 1], F32, tag="bn_istd")
    nc.vector.reciprocal(istd[:], std[:])
    s = g.sbuf.tile([C, 1], F32, tag="bn_sc")
    nc.vector.tensor_mul(s[:], g_ap, istd[:])
    ms = g.sbuf.tile([C, 1], F32, tag="bn_ms")
    nc.vector.tensor_mul(ms[:], mean[:], s[:])
    t = g.sbuf.tile([C, 1], F32, tag="bn_t")
    nc.vector.tensor_sub(t[:], bb_ap, ms[:])
    nc.vector.tensor_scalar(ds16_dst[:], dsf32[0:C].opt(), s[:], t[:],
                            mybir.AluOpType.mult, mybir.AluOpType.add)


def emit_matvec_op(g, w_ap, nk, nm, rhs_cols, biasrow_ap, act, out16, psum_tag):
    """out-on-partitions matvec: w_ap (128, nk*nm*128) blocks; rhs_cols
    (128, nk) fp16; psum (128, nm); biasrow (1, 128*nm) f16 folded via
    ones-matmul; act lrelu or none; out16 (128, nm) fp16 (or f32)."""
    nc = g.nc
    ps = g.psum.tile([128, nm], F32, tag="mm")
    for m in range(nm):
        for k in range(nk):
            nc.tensor.matmul(ps[:, m:m + 1],
                             w_ap[:, (k * nm + m) * 128:(k * nm + m) * 128 + 128],
                             rhs_cols[:, k:k + 1],
                             start=(k == 0), stop=False)
        nc.tensor.matmul(ps[:, m:m + 1], biasrow_ap[0:1, m * 128:(m + 1) * 128],
                         g.ones[0:1, 0:1], start=False, stop=True)
    emit_act(g, out16[:], ps, 128, nm, act)


def build_program():
    nc = bacc.Bacc("TRN2", target_bir_lowering=False, debug=False,
                   num_devices=N_CORES)
    g = Ctx()
    g.nc = nc

    def inp(name, shape, dt):
        return nc.dram_tensor(name, list(shape), dt, kind="ExternalInput").ap()

    # --- declare I/O ---
    I = {}
    I['xpatch'] = inp('xpatch', (9, 2080), F16)
    enc_specs = [('ew0', (9, 32)), ('ew11', (32, 576)), ('ew12', (64, 576)),
                 ('ewd1', (32, 64)), ('ew21', (64, 1152)), ('ew22', (128, 1152)),
                 ('ewd2', (64, 128)), ('ew31', (128, 1152)), ('ew32', (128, 1152))]
    for n, s in enc_specs:
        I[n] = inp(n, s, F16)
    for n, c in [('eb0', 32), ('eb11', 64), ('eb12', 64), ('ebd1', 64),
                 ('eg1', 64), ('ebn1', 64), ('eb21', 128), ('eb22', 128),
                 ('ebd2', 128), ('eg2', 128), ('ebn2', 128), ('eb31', 128),
                 ('eb32', 128)]:
        I[n] = inp(n, (c, 1), F32)
    I['ident'] = inp('ident', (128, 128), F16)
    I['ones'] = inp('ones', (1, 512), F16)
    I['ones32f'] = inp('ones32f', (1, 32), F32)
    for n, c in [('eb0r', 32), ('eb11r', 64), ('eb12r', 64), ('eb21r', 128),
                 ('eb22r', 128), ('eb31r', 128), ('eb32r', 128), ('ltb1cr', 64),
                 ('mb2r', 256), ('mb3r', 128), ('mb4r', 256), ('mb5r', 512)]:
        I[n] = inp(n, (1, c), F16)
    I['lt1w'] = inp('lt1w', (128, 136 * 64), F16)
    I['ltb1c'] = inp('ltb1c', (64, 1), F32)
    for n, nk, nm in [('mw2', 4, 2), ('mw3', 2, 1), ('mw4', 1, 2), ('mw5', 2, 4)]:
        I[n] = inp(n, (128, nk * nm * 128), F16)
        I[n.replace('w', 'b')] = inp(n.replace('w', 'b'), (128, nm), F32)
    I['rev3w'] = inp('rev3w', (128, 4 * 42 * 128), F16)
    I['rev3br'] = inp('rev3br', (1, 5376), F32)
    for i, (wd, nk, nt) in enumerate([(WD0, NK_LT1, 17), (WD1, 25, 25)]):
        p = f'd{i}_'
        I[p + 'w_in'] = inp(p + 'w_in', (128, 576), F16)
        I[p + 'rb1w1'] = inp(p + 'rb1w1', (64, 576), F16)
        I[p + 'rb1w2'] = inp(p + 'rb1w2', (64, 576), F16)
        I[p + 'ct1w'] = inp(p + 'ct1w', (64, 576), F16)
        I[p + 'rb2w1'] = inp(p + 'rb2w1', (64, 288), F16)
        I[p + 'rb2w2'] = inp(p + 'rb2w2', (32, 288), F16)
        I[p + 'rb2ds'] = inp(p + 'rb2ds', (64, 32), F16)
        I[p + 'ct2w'] = inp(p + 'ct2w', (32, 288), F16)
        I[p + 'rb3w1'] = inp(p + 'rb3w1', (128, 3), F16)
        I[p + 'rb3w2'] = inp(p + 'rb3w2', (8, 9), F32)
        I[p + 'rb3ds'] = inp(p + 'rb3ds', (32, 1), F16)
        for n, c in [('b_in', 64), ('rb1b1', 64), ('rb1b2', 64), ('ct1b', 64),
                     ('rb2b1', 32), ('rb2b2', 32), ('rb2dsb', 32), ('rb2g', 32),
                     ('rb2bb', 32), ('ct2b', 32), ('rb3b1', 1), ('rb3b2', 1),
                     ('rb3dsb', 1), ('rb3g', 1), ('rb3bb', 1), ('fb1', 64)]:
            I[p + n] = inp(p + n, (c, 1), F32)
        I[p + 'rb3b2p8'] = inp(p + 'rb3b2p8', (8, 1), F32)
        for n, c in [('b_inr', 64), ('rb1b1r', 64), ('rb1b2r', 64),
                     ('ct1br', 64), ('rb2b1r', 32), ('rb2b2r', 32),
                     ('ct2br', 32), ('rb3b1r', 1), ('fb1r', 64), ('fb2r', 256)]:
            I[p + n] = inp(p + n, (1, c), F16)
        I[p + 'fw1'] = inp(p + 'fw1', (128, nk * 64), F16)
        I[p + 'fw2'] = inp(p + 'fw2', (128, 4 * 2 * 128), F16)
        I[p + 'fb2'] = inp(p + 'fb2', (128, 2), F32)
        I[p + 'fw3'] = inp(p + 'fw3', (128, 2 * nt * 128), F16)
        I[p + 'fb3r'] = inp(p + 'fb3r', (1, nt * 128), F32)
    I['border'] = inp('border', (1, 32), F32)

    O = {}
    O['d0'] = nc.dram_tensor('d0', [H, 64], F32, kind="ExternalOutput").ap()
    O['d1'] = nc.dram_tensor('d1', [H, 96], F32, kind="ExternalOutput").ap()
    O['m0'] = nc.dram_tensor('m0', [H, 1], F32, kind="ExternalOutput").ap()
    O['m1'] = nc.dram_tensor('m1', [H, 2], F32, kind="ExternalOutput").ap()

    # internal DRAM
    e0_dram = nc.dram_tensor('e0_dram', [FLAT0], F16)
    z1p_dram = nc.dram_tensor('z1p_dram', [64], F32)
    z1r_dram = nc.dram_tensor('z1r_dram', [512], F32, addr_space="Shared")
    rloc_dram = nc.dram_tensor('rloc_dram', [5376], F16)
    rall_dram = nc.dram_tensor('rall_dram', [43008], F16, addr_space="Shared")
    hh_dram = [nc.dram_tensor(f'hh{i}_dram', [128 * (NK_LT1, 25)[i]], F16)
               for i in range(2)]
    zf1_dram = nc.dram_tensor('zf1_dram', [128], F32)
    zfall_dram = nc.dram_tensor('zfall_dram', [1024], F32, addr_space="Shared")
    y_dram = [nc.dram_tensor(f'y{i}_dram', [128 * (17, 25)[i]], F32)
              for i in range(2)]

    rg = [list(range(N_CORES))]

    with tile.TileContext(nc) as tc:
        with (
            tc.tile_pool(name="sbuf", bufs=1) as sbuf,
            tc.tile_pool(name="wstream", bufs=2) as wstream,
            tc.tile_pool(name="psum", bufs=3, space="PSUM") as psum,
        ):
            g.sbuf, g.psum = sbuf, psum
            D = I
            I = {}
            for _n, _ap in D.items():
                if _n in ('rev3w', 'lt1w', 'xpatch'):
                    continue
                _t = sbuf.tile(list(_ap.shape), _ap.dtype, tag="in_" + _n)
                nc.sync.dma_start(_t[:], _ap)
                I[_n] = _t
            g.ident = I['ident']
            g.ones = I['ones']
            g.scratch = sbuf.tile([128, 800], F32, tag="scratch")

            # ================= ENCODER =================
            B0 = sbuf.tile([32, 34, 67], F16, tag="big1")
            nc.gpsimd.memset(B0[:], 0.0)
            # L0: K=9 im2col; row tiles of 7; patches streamed per tile
            y0 = 0
            while y0 < 32:
                nr = min(7, 32 - y0)
                xp = wstream.tile([9, 512], F16, tag="xp")
                nc.sync.dma_start(xp[0:9, 0:nr * 65],
                                  D['xpatch'][:, y0 * 65:(y0 + nr) * 65])
                ps = psum.tile([32, nr, 65], F32, tag="mm")
                nc.tensor.matmul(ps[:], I['ew0'], xp[0:9, 0:nr * 65],
                                 start=True, stop=False)
                nc.tensor.matmul(ps[:], I['eb0r'], g.ones[0:1, 0:nr * 65],
                                 start=False, stop=True)
                emit_act(g, B0[0:32, 1 + y0:1 + y0 + nr, 1:66], ps, 32, nr * 65,
                         'lrelu')
                y0 += nr
            # rb1 (32->64, s2): c1
            B1 = sbuf.tile([64, 18, 35], F16, tag="B1")
            nc.gpsimd.memset(B1[:], 0.0)
            emit_conv(g, 'e_rb1c1', B0, B1, 32, 64, 32, 65, 2, I['ew11'],
                      I['eb11r'], 'lrelu', rows_per_tile=8)
            # rb1 ds (1x1 s2) + bn
            ds_tiles = []
            for (ty, nr) in [(0, 8), (8, 8)]:
                ps = psum.tile([64, nr, 33], F32, tag="mm")
                rhs = B0[0:32, 1 + 2 * ty: 1 + 2 * ty + 2 * nr: 2, 1:67:2]
                nc.tensor.matmul(ps[:], I['ewd1'], rhs, start=True, stop=True)
                ds_tiles.append((ps, ty, nr))
            dsA_f32 = sbuf.tile([64, 16, 33], F32, tag="bigf32")
            dsA16 = sbuf.tile([64, 16, 33], F16, tag="dsA16")
            emit_bn(g, ds_tiles, 64, 528, I['ebd1'], I['eg1'], I['ebn1'],
                    dsA_f32, dsA16)
            # rb1 c2 + identity add
            B2 = sbuf.tile([64, 18, 35], F16, tag="B2")
            nc.gpsimd.memset(B2[:], 0.0)
            emit_conv(g, 'e_rb1c2', B1, B2, 64, 64, 16, 33, 1, I['ew12'],
                      I['eb12r'], 'lrelu', rows_per_tile=8,
                      extra_ident_rhs=dsA16)
            # rb2 (64->128, s2)
            B3 = sbuf.tile([128, 10, 19], F16, tag="B3")
            nc.gpsimd.memset(B3[:], 0.0)
            emit_conv(g, 'e_rb2c1', B2, B3, 64, 128, 16, 33, 2, I['ew21'],
                      I['eb21r'], 'lrelu')
            ps = psum.tile([128, 8, 17], F32, tag="mm")
            nc.tensor.matmul(ps[:], I['ewd2'], B2[0:64, 1:17:2, 1:35:2],
                             start=True, stop=True)
            dsB_f32 = sbuf.tile([128, 8, 17], F32, tag="dsB_f32")
            dsB16 = sbuf.tile([128, 8, 17], F16, tag="dsB16")
            emit_bn(g, [(ps, 0, 8)], 128, 136, I['ebd2'], I['eg2'], I['ebn2'],
                    dsB_f32, dsB16)
            B4 = sbuf.tile([128, 10, 19], F16, tag="B4")
            nc.gpsimd.memset(B4[:], 0.0)
            emit_conv(g, 'e_rb2c2', B3, B4, 128, 128, 8, 17, 1, I['ew22'],
                      I['eb22r'], 'lrelu', extra_ident_rhs=dsB16)
            # rb3 (128->128, s1, no ds)
            B5 = sbuf.tile([128, 10, 19], F16, tag="B5")
            nc.gpsimd.memset(B5[:], 0.0)
            emit_conv(g, 'e_rb3c1', B4, B5, 128, 128, 8, 17, 1, I['ew31'],
                      I['eb31r'], 'lrelu')
            B6 = sbuf.tile([128, 10, 19], F16, tag="B6")
            nc.gpsimd.memset(B6[:], 0.0)
            emit_conv(g, 'e_rb3c2', B5, B6, 128, 128, 8, 17, 1, I['ew32'],
                      I['eb32r'], 'lrelu', extra_ident_rhs=B4[0:128, 1:9, 1:18])

            # e0 export + reload as k-chunk columns (full 136 chunks)
            nc.sync.dma_start(e0_dram.ap(), B6[0:128, 1:9, 1:18])
            e0c = sbuf.tile([128, 136], F16, tag="e0c")
            e0r = e0_dram.ap().rearrange("(a b) -> b a", b=128)
            nc.sync.dma_start(e0c[:], e0r)

            # ====== LT1 output-sharded (64 outputs per core) + AllGather ======
            psz = psum.tile([64, 1], F32, tag="mm")
            for kb in range(4):
                lt1b = wstream.tile([128, 34 * 64], F16, tag="lt1b")
                nc.sync.dma_start(lt1b[:], D['lt1w'][:, kb * 2176:(kb + 1) * 2176])
                for kk in range(34):
                    k = 34 * kb + kk
                    nc.tensor.matmul(psz[:], lt1b[:, kk * 64:(kk + 1) * 64],
                                     e0c[:, k:k + 1],
                                     start=(k == 0), stop=False)
            nc.tensor.matmul(psz[:], I['ltb1cr'], g.ones[0:1, 0:1],
                             start=False, stop=True)
            z1p = sbuf.tile([64, 1], F32, tag="z1p")
            emit_act(g, z1p[:], psz, 64, 1, 'lrelu')
            nc.sync.dma_start(z1p_dram.ap(), z1p[:])
            nc.gpsimd.collective_compute(
                "AllGather", mybir.AluOpType.bypass, replica_groups=rg,
                ins=[z1p_dram.ap()], outs=[z1r_dram.ap()])
            z1g = sbuf.tile([128, 4], F32, tag="z1g")
            nc.sync.dma_start(z1g[:], z1r_dram.ap().rearrange("(a b) -> b a", b=128))
            z16 = sbuf.tile([128, 4], F16, tag="z16")
            nc.vector.tensor_copy(z16[:], z1g[:])

            # ================= mids =================
            mids = [('mw2', 'mb2', 4, 2), ('mw3', 'mb3', 2, 1),
                    ('mw4', 'mb4', 1, 2), ('mw5', 'mb5', 2, 4)]
            zcur = z16
            for wn, bn, nk, nm in mids:
                wt = I[wn]
                znext = sbuf.tile([128, nm], F16, tag=wn + "_z")
                emit_matvec_op(g, wt, nk, nm, zcur, I[bn + 'r'], 'lrelu', znext, "mid")
                zcur = znext

            # ================= rev3 + AllGather =================
            # rhs-streaming, nt-major blocks; per-tile DMA out to dram
            NT_R3 = [512] * 10 + [256]
            off = 0
            for wnt in NT_R3:
                wck = wstream.tile([128, 4 * 512], F16, tag="rev3wc")
                nc.sync.dma_start(wck[0:128, 0:4 * wnt],
                                  D['rev3w'][:, 4 * off:4 * off + 4 * wnt])
                ps = psum.tile([1, wnt], F32, tag="mm")
                for k in range(4):
                    nc.tensor.matmul(ps[:], zcur[:, k:k + 1],
                                     wck[0:128, k * wnt:(k + 1) * wnt],
                                     start=(k == 0), stop=(k == 3))
                rsb = wstream.tile([1, 512], F16, tag="rsb")
                nc.vector.scalar_tensor_tensor(
                    rsb[0:1, 0:wnt], ps[:], 1.0,
                    I['rev3br'][0:1, off:off + wnt],
                    mybir.AluOpType.mult, mybir.AluOpType.add)
                nc.sync.dma_start(rloc_dram.ap()[off:off + wnt], rsb[0:1, 0:wnt])
                off += wnt
            nc.gpsimd.collective_compute(
                "AllGather", mybir.AluOpType.bypass, replica_groups=rg,
                ins=[rloc_dram.ap()], outs=[rall_dram.ap()])

            # ================= decoders: conv chains + fc1 =================
            zf1both = sbuf.tile([128, 1], F32, tag="zf1both")
            WDM = WD1
            sIn = sbuf.tile([128, 4, WDM + 2], F16, tag="d_sIn")
            A1 = sbuf.tile([64, 4, WDM + 2], F16, tag="d_A1")
            A2 = sbuf.tile([64, 4, WDM + 2], F16, tag="d_A2")
            A3 = sbuf.tile([64, 4, WDM + 2], F16, tag="d_A3")
            B1d = sbuf.tile([64, 6, 2 * WDM + 2], F16, tag="d_B1d")
            C1 = sbuf.tile([32, 6, 2 * WDM + 2], F16, tag="d_C1")
            C2 = sbuf.tile([32, 6, 2 * WDM + 2], F16, tag="d_C2")
            D1 = sbuf.tile([32, 10, 4 * WDM + 2], F16, tag="big1")
            z1sh = sbuf.tile([8, 3, 4 * WDM + 2], F16, tag="d_z1sh")
            for _b in (sIn, A1, A2, A3, B1d, C1, C2, D1, z1sh):
                nc.gpsimd.memset(_b[:], 0.0)
            for di, wd in enumerate([WD0, WD1]):
                p = f'd{di}_'
                w4 = 4 * wd
                npx3 = 8 * w4 // 4  # = 2*w4? no: level3 pixels = 8 * (4*wd) / 4
                # level sizes: L1 (H=2, wd), L2 (H=4, 2wd), L3 (H=8, 4wd)
                w2 = 2 * wd
                # -- weights
                wts = {wn: I[p + wn] for wn in
                       ['w_in', 'rb1w1', 'rb1w2', 'ct1w', 'rb2w1', 'rb2w2',
                        'rb2ds', 'ct2w', 'rb3w1', 'rb3ds']}
                rb3w2 = I[p + 'rb3w2']

                off = 0 if di == 0 else FLAT0
                rsl = rall_dram.ap()[off:off + 128 * 2 * wd].rearrange(
                    "(c h w) -> c h w", c=128, h=2)
                nc.sync.dma_start(sIn[0:128, 1:3, 1:1 + wd], rsl)
                emit_conv(g, p + 'cin', sIn, A1, 128, 64, 2, wd, 1,
                          wts['w_in'], I[p + 'b_inr'], 'lrelu')
                emit_conv(g, p + 'rb1c1', A1, A2, 64, 64, 2, wd, 1,
                          wts['rb1w1'], I[p + 'rb1b1r'], 'lrelu')
                emit_conv(g, p + 'rb1c2', A2, A3, 64, 64, 2, wd, 1,
                          wts['rb1w2'], I[p + 'rb1b2r'], 'lrelu',
                          extra_ident_rhs=A1[0:64, 1:3, 1:1 + wd])
                # ct1: 64->64, L1 (2, wd) -> L2 (4, 2wd)
                TAPS = {0: [(1, 0)], 1: [(2, 0), (0, 1)]}
                for q in (0, 1):
                    for d in (0, 1):
                        taps = [(ky, kx, dy, dx) for (ky, dy) in TAPS[q]
                                for (kx, dx) in TAPS[d]]
                        ps = psum.tile([64, 2, wd], F32, tag="mm")
                        for mi, (ky, kx, dy, dx) in enumerate(taps):
                            t = 3 * ky + kx
                            rhs = A3[0:64, 1 + dy:3 + dy, 1 + dx:1 + dx + wd]
                            nc.tensor.matmul(ps[:], wts['ct1w'][:, t * 64:(t + 1) * 64],
                                             rhs, start=(mi == 0), stop=False)
                        nc.tensor.matmul(ps[:], I[p + 'ct1br'],
                                         g.ones[0:1, 0:2 * wd],
                                         start=False, stop=True)
                        emit_act(g, B1d[0:64, 1 + q:1 + q + 4:2, 1 + d:1 + d + w2:2],
                                 ps, 64, 2 * wd, 'lrelu')
                # rb2: 64->32 with ds+bn, at L2 (4, w2)
                rpt = 512 // w2
                emit_conv(g, p + 'rb2c1', B1d, C1, 64, 32, 4, w2, 1,
                          wts['rb2w1'], I[p + 'rb2b1r'], 'lrelu', rows_per_tile=rpt)
                ds_tiles = []
                y0 = 0
                while y0 < 4:
                    nr = min(rpt, 4 - y0)
                    ps = psum.tile([32, nr, w2], F32, tag="mm")
                    nc.tensor.matmul(ps[:], wts['rb2ds'],
                                     B1d[0:64, 1 + y0:1 + y0 + nr, 1:1 + w2],
                                     start=True, stop=True)
                    ds_tiles.append((ps, y0, nr))
                    y0 += nr
                dsC_f32 = sbuf.tile([32, 4, w2], F32, tag="bigf32")
                dsC16 = sbuf.tile([32, 4, w2], F16, tag="d_dsC16")
                emit_bn(g, ds_tiles, 32, 4 * w2, I[p + 'rb2dsb'], I[p + 'rb2g'],
                        I[p + 'rb2bb'], dsC_f32, dsC16)
                emit_conv(g, p + 'rb2c2', C1, C2, 32, 32, 4, w2, 1,
                          wts['rb2w2'], I[p + 'rb2b2r'], 'lrelu',
                          rows_per_tile=rpt, extra_ident_rhs=dsC16)
                # ct2: 32->32, L2 (4, w2) -> L3 (8, w4)
                for q in (0, 1):
                    for d in (0, 1):
                        taps = [(ky, kx, dy, dx) for (ky, dy) in TAPS[q]
                                for (kx, dx) in TAPS[d]]
                        y0 = 0
                        while y0 < 4:
                            nr = min(rpt, 4 - y0)
                            ps = psum.tile([32, nr, w2], F32, tag="mm")
                            for mi, (ky, kx, dy, dx) in enumerate(taps):
                                t = 3 * ky + kx
                                rhs = C2[0:32, 1 + y0 + dy:1 + y0 + dy + nr,
                                         1 + dx:1 + dx + w2]
                                nc.tensor.matmul(ps[:], wts['ct2w'][:, t * 32:(t + 1) * 32],
                                                 rhs, start=(mi == 0), stop=False)
                            nc.tensor.matmul(ps[:], I[p + 'ct2br'],
                                             g.ones[0:1, 0:nr * w2],
                                             start=False, stop=True)
                            emit_act(g, D1[0:32, 1 + 2 * y0 + q:1 + 2 * y0 + q + 2 * nr:2,
                                           1 + d:1 + d + w4:2],
                                     ps, 32, nr * w2, 'lrelu')
                            y0 += nr
                # ---- rb3 tail (32 -> 1) at L3 (8, w4) ----
                npx = 8 * w4
                npx2 = npx // 2
                z1f = sbuf.tile([1, npx], F16, tag="d_flat1")
                for hf in range(2):
                    P = sbuf.tile([128, 3, npx2], F16, tag="d_patches")
                    for dy in range(3):
                        for dx in range(3):
                            t = 3 * dy + dx
                            srcw = D1[0:32, dy + 4 * hf:dy + 4 * hf + 4,
                                      dx:dx + w4]
                            nc.sync.dma_start(
                                P[(32 * t) % 128:(32 * t) % 128 + 32,
                                  t // 4, 0:npx2], srcw)
                    n0 = 0
                    while n0 < npx2:
                        nn = min(512, npx2 - n0)
                        ps = psum.tile([1, nn], F32, tag="mm")
                        for j, kr in ((0, 128), (1, 128), (2, 32)):
                            nc.tensor.matmul(ps[:], wts['rb3w1'][0:kr, j:j + 1],
                                             P[0:kr, j, n0:n0 + nn],
                                             start=(j == 0), stop=False)
                        nc.tensor.matmul(ps[:], I[p + 'rb3b1r'],
                                         g.ones[0:1, 0:nn],
                                         start=False, stop=True)
                        emit_act(g, z1f[:, hf * npx2 + n0:hf * npx2 + n0 + nn],
                                 ps, 1, nn, 'lrelu')
                        n0 += nn
                for dy in range(3):
                    p0 = max(0, 1 - dy)
                    p1 = min(8, 9 - dy)
                    r0 = p0 + dy - 1
                    r1 = p1 + dy - 1
                    nc.sync.dma_start(
                        z1sh[p0:p1, dy, 1:1 + w4],
                        z1f[0:1, r0 * w4:r1 * w4].rearrange(
                            "a (h w) -> a h w", w=w4))
                # conv2 1->1 on H-partition layout (DVE); rows pre-shifted
                acc = sbuf.tile([8, w4], F32, tag="d_acc")
                nc.gpsimd.memset(acc[:], 0.0)
                for dy in range(3):
                    for dx in range(3):
                        t = 3 * dy + dx
                        nc.vector.scalar_tensor_tensor(
                            acc[:], z1sh[0:8, dy, dx:dx + w4],
                            rb3w2[:, t:t + 1], acc[:],
                            mybir.AluOpType.mult, mybir.AluOpType.add)
                # ds 32->1 + bn
                dsD = sbuf.tile([1, npx], F16, tag="d_flat2")
                dacc = sbuf.tile([1, 8], F32, tag=p + "dacc")
                for r in range(8):
                    ps = psum.tile([1, w4], F32, tag="mm")
                    nc.tensor.matmul(ps[:], wts['rb3ds'],
                                     D1[0:32, 1 + r, 1:1 + w4],
                                     start=True, stop=True)
                    nc.scalar.activation(dsD[:, r * w4:(r + 1) * w4], ps[:],
                                         mybir.ActivationFunctionType.Identity,
                                         bias=I[p + 'rb3dsb'], scale=1.0,
                                         accum_out=dacc[:, r:r + 1])
                dsum = sbuf.tile([1, 1], F32, tag=p + "dsum")
                nc.vector.tensor_reduce(dsum[:], dacc[:], mybir.AxisListType.X,
                                        mybir.AluOpType.add)
                dacc2 = sbuf.tile([1, 4], F32, tag=p + "dacc2")
                qn = npx // 4
                for qq in range(4):
                    nc.scalar.activation(g.scratch[0:1, 0:qn],
                                         dsD[0:1, qq * qn:(qq + 1) * qn],
                                         mybir.ActivationFunctionType.Square,
                                         accum_out=dacc2[:, qq:qq + 1])
                dsq = sbuf.tile([1, 1], F32, tag=p + "dsq")
                nc.vector.tensor_reduce(dsq[:], dacc2[:], mybir.AxisListType.X,
                                        mybir.AluOpType.add)
                inv_n = 1.0 / npx
                dmean = sbuf.tile([1, 1], F32, tag=p + "dmean")
                nc.scalar.mul(dmean[:], dsum[:], inv_n)
                dex2 = sbuf.tile([1, 1], F32, tag=p + "dex2")
                nc.scalar.mul(dex2[:], dsq[:], inv_n)
                dm2 = sbuf.tile([1, 1], F32, tag=p + "dm2")
                nc.vector.tensor_mul(dm2[:], dmean[:], dmean[:])
                dvar = sbuf.tile([1, 1], F32, tag=p + "dvar")
                nc.vector.tensor_sub(dvar[:], dex2[:], dm2[:])
                nc.vector.tensor_scalar_add(dvar[:], dvar[:], EPS)
                dstd = sbuf.tile([1, 1], F32, tag=p + "dstd")
                nc.scalar.activation(dstd[:], dvar[:],
                                     mybir.ActivationFunctionType.Sqrt,
                                     bias=0.0, scale=1.0)
                distd = sbuf.tile([1, 1], F32, tag=p + "distd")
                nc.vector.reciprocal(distd[:], dstd[:])
                dsc = sbuf.tile([1, 1], F32, tag=p + "dsc")
                nc.vector.tensor_mul(dsc[:], I[p + 'rb3g'], distd[:])
                dms = sbuf.tile([1, 1], F32, tag=p + "dms")
                nc.vector.tensor_mul(dms[:], dmean[:], dsc[:])
                dt_ = sbuf.tile([1, 1], F32, tag=p + "dt")
                nc.vector.tensor_sub(dt_[:], I[p + 'rb3bb'], dms[:])
                nc.vector.tensor_scalar(dsD[:], dsD[:], dsc[:], dt_[:],
                                        mybir.AluOpType.mult, mybir.AluOpType.add)
                dsimg = sbuf.tile([8, w4], F16, tag="d_dsimg")
                nc.gpsimd.dma_start(dsimg[:],
                                    dsD[:].rearrange("a (h w) -> a h w", h=8))
                hsum = sbuf.tile([8, w4], F32, tag="d_hsum")
                nc.vector.scalar_tensor_tensor(hsum[:], acc[:],
                                               I[p + 'rb3b2p8'][:],
                                               dsimg[:],
                                               mybir.AluOpType.add,
                                               mybir.AluOpType.add)
                hh16 = sbuf.tile([8, w4], F16, tag="d_hh16")
                htmp = sbuf.tile([8, w4], F32, tag="d_htmp")
                nc.scalar.mul(htmp[:], hsum[:], ALPHA)
                nc.vector.tensor_max(hh16[:], htmp[:], hsum[:])
                nc.sync.dma_start(
                    hh_dram[di].ap()[0:npx].rearrange("(h w) -> h w", h=8), hh16[:])
                nk = (NK_LT1, 25)[di]
                hT = sbuf.tile([128, nk], F16, tag=p + "hT")
                nc.sync.dma_start(hT[:],
                                  hh_dram[di].ap().rearrange("(a b) -> b a", b=128))
                # fc1 shard: 64 outputs
                fw1 = I[p + 'fw1']
                psf = psum.tile([64, 1], F32, tag="mm")
                for k in range(nk):
                    nc.tensor.matmul(psf[:], fw1[:, k * 64:(k + 1) * 64],
                                     hT[:, k:k + 1], start=(k == 0), stop=False)
                nc.tensor.matmul(psf[:], I[p + 'fb1r'], g.ones[0:1, 0:1],
                                 start=False, stop=True)
                emit_act(g, zf1both[64 * di:64 * di + 64, 0:1], psf, 64, 1,
                         'lrelu')

            # fused fc1 AllGather
            nc.sync.dma_start(zf1_dram.ap(), zf1both[:])
            nc.gpsimd.collective_compute(
                "AllGather", mybir.AluOpType.bypass, replica_groups=rg,
                ins=[zf1_dram.ap()], outs=[zfall_dram.ap()])

            # ================= decoders: fc2/fc3 + masking =================
            for di, (wimg, m) in enumerate([(W0, 1), (W1, 2)]):
                p = f'd{di}_'
                nt = (17, 25)[di]
                zfg = sbuf.tile([128, 4], F32, tag=p + "zfg")
                # zfall[128*c + 64*dec + j]; dec di's vector z[i], i = 64*c + j.
                # dst (p, k) holds z[128k + p]: c = 2k + p//64, j = p%64
                #   -> dram idx = 256k + 128*(p//64) + 64*di + p%64
                zview = zfall_dram.ap().rearrange("(k h j) -> h j k", h=4, j=64)
                # zview[h, j, k] = dram[256k + 64h + j]; need h = 2*(p//64) + di
                for half in range(2):
                    nc.sync.dma_start(
                        zfg[64 * half:64 * half + 64, 0:4],
                        zview[2 * half + di, :, :])
                zfg16 = sbuf.tile([128, 4], F16, tag=p + "zfg16")
                nc.vector.tensor_copy(zfg16[:], zfg[:])
                fw2 = I[p + 'fw2']
                zf2 = sbuf.tile([128, 2], F16, tag=p + "zf2")
                emit_matvec_op(g, fw2, 4, 2, zfg16, I[p + 'fb2r'], 'lrelu',
                               zf2, "mid")
                fw3 = I[p + 'fw3']
                npx3 = nt * 128
                NT3 = [512] * (npx3 // 512) + ([npx3 % 512] if npx3 % 512 else [])
                off = 0
                pos = 0
                for wnt in NT3:
                    ps = psum.tile([1, wnt], F32, tag="mm")
                    for k in range(2):
                        nc.tensor.matmul(ps[:], zf2[:, k:k + 1],
                                         fw3[0:128, pos + k * wnt:pos + (k + 1) * wnt],
                                         start=(k == 0), stop=(k == 1))
                    yfl = wstream.tile([1, 512], F32, tag="yfl")
                    nc.vector.scalar_tensor_tensor(
                        yfl[0:1, 0:wnt], ps[:], 1.0,
                        I[p + 'fb3r'][0:1, off:off + wnt],
                        mybir.AluOpType.mult, mybir.AluOpType.add)
                    nc.sync.dma_start(y_dram[di].ap()[off:off + wnt],
                                      yfl[0:1, 0:wnt])
                    pos += 2 * wnt
                    off += wnt
                ysb = sbuf.tile([H, wimg], F32, tag=p + "ysb")
                nc.sync.dma_start(ysb[:], y_dram[di].ap()[0:H * wimg]
                                  .rearrange("(h w) -> h w", h=H))
                # masking
                nz = sbuf.tile([H, m], F32, tag=p + "nz")
                nc.vector.tensor_scalar(nz[:], ysb[0:H, wimg - m:wimg], 0.0, None,
                                        mybir.AluOpType.is_gt)
                nc.sync.dma_start(O[f'm{di}'], nz[:])
                nzsq = sbuf.tile([H, 32], F32, tag=p + "nzsq")
                nc.gpsimd.memset(nzsq[:], 0.0)
                nc.vector.tensor_copy(nzsq[0:H, 0:m], nz[:])
                nzT = sbuf.tile([H, 32], F32, tag=p + "nzT")
                nc.vector.transpose(nzT[:], nzsq[:])
                AT = sbuf.tile([m + 1, 32], F32, tag=p + "AT")
                nc.sync.dma_start(AT[0:1, :], I['border'][0:1, :])
                nc.sync.dma_start(AT[1:1 + m, :], nzT[0:m, :])
                E = sbuf.tile([m + 1, 32 * (m + 1)], F32, tag=p + "E")
                nc.gpsimd.memset(E[:], 0.0)
                for j in range(m):
                    nc.sync.dma_start(E[j:j + 1, 32 * j:32 * (j + 1)],
                                      nzT[j:j + 1, 0:32])
                nc.sync.dma_start(E[m:m + 1, 32 * m:32 * (m + 1)],
                                  I['ones32f'][0:1, :])
                psm = psum.tile([H, 32 * (m + 1)], F32, tag="mm")
                nc.tensor.matmul(psm[:], AT[:], E[:], start=True, stop=True)
                dout = sbuf.tile([H, 32 * (m + 1)], F32, tag=p + "dout")
                nc.vector.scalar_tensor_tensor(dout[:], ysb[0:H, 0:32 * (m + 1)],
                                               1.0 / S_FC, psm[:],
                                               mybir.AluOpType.mult,
                                               mybir.AluOpType.mult)
                nc.sync.dma_start(O[f'd{di}'], dout[:])

    nc.compile()
    return nc


# ----------------------------------------------------------------------------
# host-side input prep
# ----------------------------------------------------------------------------

def prep_inputs(x, enc0_params, lt_params, rev_params, dec_params):
    """Returns list of 8 per-core input dicts."""
    f32 = lambda a: np.asarray(a, np.float32)
    f16 = lambda a: np.asarray(a, np.float32).astype(NP16)

    base = {}
    # L0 im2col patches from x (pure gather + zero pad)
    xi = f32(x)[0, 0]  # (32, 65)
    xpad = np.zeros((34, 67), np.float32)
    xpad[1:33, 1:66] = xi
    patches = np.zeros((9, 2080), np.float32)
    for dy in range(3):
        for dx in range(3):
            patches[3 * dy + dx] = xpad[dy:dy + 32, dx:dx + 65].reshape(-1)
    base['xpatch'] = f16(patches)

    e = enc0_params
    base['ew0'] = f16(f32(e['w0'])[:, 0].reshape(32, 9).T)
    base['eb0'] = col1(e['b0'])
    base['ew11'] = pack_conv(f32(e['rb1']['w1']))
    base['eb11'] = col1(e['rb1']['b1'])
    base['ew12'] = pack_conv(f32(e['rb1']['w2']))
    base['eb12'] = col1(e['rb1']['b2'])
    base['ewd1'] = f16(f32(e['rb1']['ds_w'])[:, :, 0, 0].T)
    base['ebd1'] = col1(e['rb1']['ds_b'])
    base['eg1'] = col1(e['rb1']['bn_g'])
    base['ebn1'] = col1(e['rb1']['bn_b'])
    base['ew21'] = pack_conv(f32(e['rb2']['w1']))
    base['eb21'] = col1(e['rb2']['b1'])
    base['ew22'] = pack_conv(f32(e['rb2']['w2']))
    base['eb22'] = col1(e['rb2']['b2'])
    base['ewd2'] = f16(f32(e['rb2']['ds_w'])[:, :, 0, 0].T)
    base['ebd2'] = col1(e['rb2']['ds_b'])
    base['eg2'] = col1(e['rb2']['bn_g'])
    base['ebn2'] = col1(e['rb2']['bn_b'])
    base['ew31'] = pack_conv(f32(e['rb3']['w1']))
    base['eb31'] = col1(e['rb3']['b1'])
    base['ew32'] = pack_conv(f32(e['rb3']['w2']))
    base['eb32'] = col1(e['rb3']['b2'])
    base['ident'] = np.eye(128, dtype=NP16)
    base['ones'] = np.ones((1, 512), NP16)
    base['ones32f'] = np.ones((1, 32), np.float32)
    row16 = lambda a, s=1.0: (np.asarray(a, np.float32) * np.float32(s)).reshape(1, -1).astype(NP16)
    base['eb0r'] = row16(e['b0'])
    base['eb11r'] = row16(e['rb1']['b1'])
    base['eb12r'] = row16(e['rb1']['b2'])
    base['eb21r'] = row16(e['rb2']['b1'])
    base['eb22r'] = row16(e['rb2']['b2'])
    base['eb31r'] = row16(e['rb3']['b1'])
    base['eb32r'] = row16(e['rb3']['b2'])
    base['mb2r'] = row16(lt_params['b2'])
    base['mb3r'] = row16(lt_params['b3'])
    base['mb4r'] = row16(rev_params['b1'])
    base['mb5r'] = row16(rev_params['b2'])

    for i, (wn, bn, nk, nm) in enumerate([('mw2', 'mb2', 4, 2), ('mw3', 'mb3', 2, 1),
                                          ('mw4', 'mb4', 1, 2), ('mw5', 'mb5', 2, 4)]):
        src = [lt_params, lt_params, rev_params, rev_params][i]
        key = ['w2', 'w3', 'w1', 'w2'][i]
        w = f32(src[key])          # (out, in)
        b = f32(src[key.replace('w', 'b')])
        base[wn] = pack_matvec(w.T, nk, nm)
        base[bn] = b.reshape(nm, 128).T.copy()

    border = np.ones((1, 32), np.float32)
    border[0, [0, 1, 30, 31]] = 0.0
    base['border'] = border

    # decoder shared (replicated) weights
    for di in range(2):
        d = dec_params[di]
        p = f'd{di}_'
        S = np.float32(S_REV3)
        base[p + 'w_in'] = pack_conv(f32(d['w_in']))
        base[p + 'b_in'] = col1(f32(d['b_in']) * S)
        base[p + 'rb1w1'] = pack_conv(f32(d['rb1']['w1']))
        base[p + 'rb1b1'] = col1(f32(d['rb1']['b1']) * S)
        base[p + 'rb1w2'] = pack_conv(f32(d['rb1']['w2']))
        base[p + 'rb1b2'] = col1(f32(d['rb1']['b2']) * S)
        base[p + 'ct1w'] = pack_convt(f32(d['ct1_w']))
        base[p + 'ct1b'] = col1(f32(d['ct1_b']) * S)
        base[p + 'rb2w1'] = pack_conv(f32(d['rb2']['w1']) / S)
        base[p + 'rb2b1'] = col1(d['rb2']['b1'])
        base[p + 'rb2w2'] = pack_conv(f32(d['rb2']['w2']))
        base[p + 'rb2b2'] = col1(d['rb2']['b2'])
        base[p + 'rb2ds'] = f16(f32(d['rb2']['ds_w'])[:, :, 0, 0].T / S)
        base[p + 'rb2dsb'] = col1(d['rb2']['ds_b'])
        base[p + 'rb2g'] = col1(d['rb2']['bn_g'])
        base[p + 'rb2bb'] = col1(d['rb2']['bn_b'])
        base[p + 'ct2w'] = pack_convt(f32(d['ct2_w']))
        base[p + 'ct2b'] = col1(d['ct2_b'])
        base[p + 'b_inr'] = row16(d['b_in'], S)
        base[p + 'rb1b1r'] = row16(d['rb1']['b1'], S)
        base[p + 'rb1b2r'] = row16(d['rb1']['b2'], S)
        base[p + 'ct1br'] = row16(d['ct1_b'], S)
        base[p + 'rb2b1r'] = row16(d['rb2']['b1'])
        base[p + 'rb2b2r'] = row16(d['rb2']['b2'])
        base[p + 'ct2br'] = row16(d['ct2_b'])
        base[p + 'rb3b1r'] = row16(d['rb3']['b1'])
        base[p + 'fb2r'] = row16(d['fc2_b'], S_FC)
        # rb3: conv1 32->1: flat k = cin + 32*t -> chunks (128, 3)
        w1 = f32(d['rb3']['w1'])  # (1, 32, 3, 3)
        flat = np.zeros(384, np.float32)
        for dy in range(3):
            for dx in range(3):
                t = 3 * dy + dx
                flat[32 * t:32 * t + 32] = w1[0, :, dy, dx]
        base[p + 'rb3w1'] = f16(flat.reshape(3, 128).T)
        base[p + 'rb3b1'] = col1(d['rb3']['b1'])
        w2 = f32(d['rb3']['w2'])[0, 0]  # (3,3)
        base[p + 'rb3w2'] = np.tile(w2.reshape(1, 9), (8, 1)).astype(np.float32)
        base[p + 'rb3b2'] = col1(d['rb3']['b2'])
        base[p + 'rb3b2p8'] = np.full((8, 1), np.float32(np.asarray(d['rb3']['b2']).ravel()[0]), np.float32)
        base[p + 'rb3ds'] = f16(f32(d['rb3']['ds_w'])[:, :, 0, 0].T)
        base[p + 'rb3dsb'] = col1(d['rb3']['ds_b'])
        base[p + 'rb3g'] = col1(d['rb3']['bn_g'])
        base[p + 'rb3bb'] = col1(d['rb3']['bn_b'])
        # fc2 / fc3 (replicated)
        w2f = f32(d['fc2_w'])
        base[p + 'fw2'] = pack_matvec(w2f.T, 4, 2)
        base[p + 'fb2'] = (f32(d['fc2_b']) * S_FC).reshape(2, 128).T.copy()
        nt = (17, 25)[di]
        w3 = f32(d['fc3_w'])      # (2080/3136, 256)
        w3p = np.zeros((nt * 128, 256), np.float32)
        w3p[:w3.shape[0]] = w3
        w3pT = np.ascontiguousarray(w3p.T)    # (256, nt*128)
        npx3 = nt * 128
        fw3 = np.zeros((128, 2 * npx3), NP16)
        pos = 0
        off = 0
        for wnt in [512] * (npx3 // 512) + ([npx3 % 512] if npx3 % 512 else []):
            for k in range(2):
                fw3[:, pos:pos + wnt] = w3pT[128 * k:128 * (k + 1), off:off + wnt]
                pos += wnt
            off += wnt
        base[p + 'fw3'] = fw3
        b3p = np.zeros(nt * 128, np.float32)
        b3p[:w3.shape[0]] = f32(d['fc3_b']) * S_FC
        base[p + 'fb3r'] = b3p.reshape(1, -1)

    # per-core shards
    W1eff = f32(lt_params['w1'])[:, :FLAT0]    # (512, 17408)
    W1T = W1eff.T                              # (17408, 512)
    W3r = f32(rev_params['w3']) * np.float32(S_REV3)   # (43008, 512)
    b3r = f32(rev_params['b3']) * np.float32(S_REV3)
    in_maps = []
    for c in range(N_CORES):
        m = dict(base)
        # lt1 output-shard: 64 outputs per core; block k = W1T[128k:+128, 64c:+64]
        lt1w = np.zeros((128, 136 * 64), NP16)
        for k in range(136):
            lt1w[:, k * 64:(k + 1) * 64] = W1T[128 * k:128 * (k + 1),
                                               64 * c:64 * (c + 1)]
        m['lt1w'] = lt1w
        m['ltb1c'] = col1(f32(lt_params['b1'])[64 * c:64 * (c + 1)])
        m['ltb1cr'] = f32(lt_params['b1'])[64 * c:64 * (c + 1)].reshape(1, -1).astype(NP16)
        W3c = W3r[5376 * c:5376 * (c + 1)]     # (5376, 512)
        W3cT = np.ascontiguousarray(W3c.T)     # (512, 5376)
        r3 = np.zeros((128, 4 * 42 * 128), NP16)
        off = 0
        pos = 0
        for wnt in [512] * 10 + [256]:
            for k in range(4):
                r3[:, pos:pos + wnt] = W3cT[128 * k:128 * (k + 1), off:off + wnt]
                pos += wnt
            off += wnt
        m['rev3w'] = r3
        m['rev3br'] = b3r[5376 * c:5376 * (c + 1)].reshape(1, -1)
        for di in range(2):
            d = dec_params[di]
            p = f'd{di}_'
            Hh, Wh = 8, (4 * WD0, 4 * WD1)[di]
            fw, fb = build_convout_fold(d['fc1_w'], d['fc1_b'], f32(d['w_out']),
                                        f32(d['b_out']), Hh, Wh)
            fw = fw * np.float32(S_FC)
            fb = fb * np.float32(S_FC)
            rows = fw[64 * c:64 * (c + 1)]     # (64, npx)
            nk = (NK_LT1, 25)[di]
            fwp = np.zeros((128, nk * 64), NP16)
            rT = rows.T                        # (npx, 64)
            for k in range(nk):
                fwp[:, k * 64:(k + 1) * 64] = rT[k * 128:(k + 1) * 128]
            m[p + 'fw1'] = fwp
            m[p + 'fb1'] = col1(fb[64 * c:64 * (c + 1)])
            m[p + 'fb1r'] = fb[64 * c:64 * (c + 1)].reshape(1, -1).astype(NP16)
        in_maps.append(m)
    return in_maps


_CACHE = {}


def kernel(x, enc0_params, lt_params, rev_params, dec_params):
    if 'nc' not in _CACHE:
        _CACHE['nc'] = build_program()
    nc = _CACHE['nc']
    in_maps = prep_inputs(x, enc0_params, lt_params, rev_params, dec_params)
    res = run_bass_kernel_spmd(nc, in_maps, list(range(N_CORES)))
    r0 = res.results[0]
    d0 = np.asarray(r0['d0'], np.float32)
    d1 = np.asarray(r0['d1'], np.float32)
    m0 = np.asarray(r0['m0'], np.float32)
    m1 = np.asarray(r0['m1'], np.float32)
    return d0, d1, m0, m1


# revision 35
# speedup vs baseline: 1.1517x; 1.1517x over previous
"""Trainium2 Bass kernel for nn_AutoEncoder_31533649887292.

8-core SPMD plan (uniform program, per-core data):
  - encoder replicated on all cores (serial conv chain, tap-accumulated matmuls)
  - lt1 (43008->512, but cols 17408..43008 multiply zeros -> dropped):
    K-sharded 8-way, partials AllReduce'd (512 floats)
  - lt2/lt3/rev1/rev2 replicated (output-on-partition matvec layout)
  - rev3 (512->43008) output-sharded 8-way + AllGather (fp16)
  - decoders run sequentially, replicated; per-decoder fc1 output-sharded
    8-way with ONE fused AllGather for both decoders
  - conv_out (1->1 conv) folded into fc1 weights host-side
  - numerics: fp16 matmul operands, fp32 PSUM/stats; compensated scales
    S_REV3=64 (undone inside dec rb2 weights) and S_FC=256 (undone at output)
"""
import numpy as np
import ml_dtypes

import concourse.bacc as bacc
import concourse.mybir as mybir
import concourse.tile as tile
from concourse.bass_utils import run_bass_kernel_spmd

F16 = mybir.dt.float16
F32 = mybir.dt.float32
NP16 = np.float16

N_CORES = 8
EPS = 1e-5
ALPHA = 0.01
S_REV3 = 64.0
S_FC = 256.0

H = 32
W0, W1 = 65, 98           # output widths
WD0, WD1 = 68, 100        # decoder entry widths (H=2)
FLAT0 = 17408             # e0 flatten / s0 size
NK_LT1 = 17               # 2176/128 k-chunks per core


# ----------------------------------------------------------------------------
# host-side weight packing helpers
# ----------------------------------------------------------------------------

def pack_conv(w):
    """w (Cout, Cin, 3, 3) -> lhsT pack (Cin, 9*Cout), tap t=3dy+dx."""
    Cout, Cin = w.shape[0], w.shape[1]
    out = np.zeros((Cin, 9 * Cout), NP16)
    for dy in range(3):
        for dx in range(3):
            t = 3 * dy + dx
            out[:, t * Cout:(t + 1) * Cout] = w[:, :, dy, dx].T
    return out


def pack_convt(w):
    """w (Cin, Cout, 3, 3) -> (Cin, 9*Cout), tap t=3ky+kx, already lhsT."""
    Cin, Cout = w.shape[0], w.shape[1]
    out = np.zeros((Cin, 9 * Cout), NP16)
    for ky in range(3):
        for kx in range(3):
            t = 3 * ky + kx
            out[:, t * Cout:(t + 1) * Cout] = w[:, :, ky, kx]
    return out


def pack_matvec(wT, nk, nm):
    """wT (K, N) (K=128*nk, N=128*nm) -> (128, nk*nm*128) block pack:
    block (k, m) at cols (k*nm+m)*128."""
    K, N = wT.shape
    out = np.zeros((128, nk * nm * 128), NP16)
    for k in range(nk):
        for m in range(nm):
            blk = wT[k * 128:(k + 1) * 128, m * 128:(m + 1) * 128]
            out[:blk.shape[0], (k * nm + m) * 128:(k * nm + m) * 128 + blk.shape[1]] = blk
    return out


def col1(v, dtype=np.float32):
    return np.ascontiguousarray(np.asarray(v, dtype).reshape(-1, 1))


def build_convout_fold(fc1_w, fc1_b, w_out, b_out, Hh, Wh):
    n = Hh * Wh
    C = np.zeros((n, n), np.float32)
    w = np.asarray(w_out)[0, 0]
    idx = np.arange(n).reshape(Hh, Wh)
    ys, xs = np.meshgrid(np.arange(Hh), np.arange(Wh), indexing='ij')
    for dy in range(3):
        for dx in range(3):
            yi, xi = ys + dy - 1, xs + dx - 1
            valid = (yi >= 0) & (yi < Hh) & (xi >= 0) & (xi < Wh)
            C[idx[ys[valid], xs[valid]], idx[yi[valid], xi[valid]]] += w[dy, dx]
    fc1_w = np.asarray(fc1_w, np.float32)
    new_w = fc1_w @ C
    new_b = np.asarray(fc1_b, np.float32) + fc1_w @ (np.float32(b_out[0]) * np.ones(n, np.float32))
    return new_w, new_b


# Small per-core-identical inputs consolidated into two packed tensors
# (one DMA each). Layout shared by builder and host via these specs.
PACK16 = [
    ('ew0', 9, 32), ('ew11', 32, 576), ('ew12', 64, 576), ('ewd1', 32, 64),
    ('ew21', 64, 1152), ('ew22', 128, 1152), ('ewd2', 64, 128),
    ('ew31', 128, 1152), ('ew32', 128, 1152),
    ('ident', 128, 128), ('ones', 1, 512),
    ('eb0r', 1, 32), ('eb11r', 1, 64), ('eb12r', 1, 64), ('eb21r', 1, 128),
    ('eb22r', 1, 128), ('eb31r', 1, 128), ('eb32r', 1, 128),
    ('mb2r', 1, 256), ('mb3r', 1, 128), ('mb4r', 1, 256), ('mb5r', 1, 512),
    ('mw2', 128, 1024), ('mw3', 128, 256), ('mw4', 128, 256), ('mw5', 128, 1024),
] + [(f'd{i}_' + n, p, w) for i in range(2) for n, p, w in [
    ('w_in', 128, 576), ('rb1w1', 64, 576), ('rb1w2', 64, 576),
    ('ct1w', 64, 576), ('rb2w1', 64, 288), ('rb2w2', 32, 288),
    ('rb2ds', 64, 32), ('ct2w', 32, 288), ('rb3w1', 128, 3), ('rb3ds', 32, 1),
    ('fw2', 128, 1024),
    ('b_inr', 1, 64), ('rb1b1r', 1, 64), ('rb1b2r', 1, 64), ('ct1br', 1, 64),
    ('rb2b1r', 1, 32), ('rb2b2r', 1, 32), ('ct2br', 1, 32), ('rb3b1r', 1, 1),
    ('fb2r', 1, 256)]]
PACK32 = [
    ('ones32f', 1, 32), ('border', 1, 32),
    ('ebd1', 64, 1), ('eg1', 64, 1), ('ebn1', 64, 1),
    ('ebd2', 128, 1), ('eg2', 128, 1), ('ebn2', 128, 1),
] + [(f'd{i}_' + n, p, w) for i in range(2) for n, p, w in [
    ('rb2dsb', 32, 1), ('rb2g', 32, 1), ('rb2bb', 32, 1),
    ('rb3dsb', 1, 1), ('rb3g', 1, 1), ('rb3bb', 1, 1),
    ('rb3w2', 8, 9), ('rb3b2p8', 8, 1),
    ('fb3r', 1, (2176, 3200)[i])]]


def _pack_layout(spec):
    offs = {}
    off = 0
    for name, pp, ww in spec:
        offs[name] = (off, pp, ww)
        off += ww
    return offs, off


OFF16, TOT16 = _pack_layout(PACK16)
OFF32, TOT32 = _pack_layout(PACK32)


# ----------------------------------------------------------------------------
# device program
# ----------------------------------------------------------------------------

class Ctx:
    pass


def emit_conv(g, name, src, dst, Cin, Cout, Hin, Win, stride, w_ap, b_ap,
              act, rows_per_tile=None, extra_ident_rhs=None):
    """Tap-accumulated 3x3 conv.
    src: padded fp16 tile (Cin, Hin+2, Win+2); dst padded fp16 tile or None.
    b_ap: f16 ROW bias (1, Cout), folded into psum via ones-matmul.
    act: 'lrelu' | 'none'. extra_ident_rhs: AP (Cout, Hout, Wout) added via
    identity matmul (residual). Returns list of (psum_ap, y0, nrows) if dst
    is None (caller evicts)."""
    nc = g.nc
    Hout = (Hin + stride - 1) // stride
    Wout = (Win + stride - 1) // stride
    if rows_per_tile is None:
        rows_per_tile = max(1, 512 // Wout)
    tiles = []
    y0 = 0
    while y0 < Hout:
        nr = min(rows_per_tile, Hout - y0)
        ps = g.psum.tile([Cout, nr, Wout], F32, tag="mm")
        mi = 0
        for dy in range(3):
            for dx in range(3):
                t = 3 * dy + dx
                rhs = src[0:Cin,
                          dy + stride * y0: dy + stride * (y0 + nr - 1) + 1: stride,
                          dx: dx + stride * (Wout - 1) + 1: stride]
                nc.tensor.matmul(ps[:], w_ap[:, t * Cout:(t + 1) * Cout], rhs,
                                 start=(mi == 0), stop=False)
                mi += 1
        if extra_ident_rhs is not None:
            nc.tensor.matmul(ps[:], g.ident[0:Cout, 0:Cout],
                             extra_ident_rhs[0:Cout, y0:y0 + nr, 0:Wout],
                             start=False, stop=False)
        # bias broadcast into psum: lhsT = bias row (1, Cout), rhs = ones (1, N)
        nc.tensor.matmul(ps[:], b_ap, g.ones[0:1, 0:nr * Wout],
                         start=False, stop=True)
        if dst is not None:
            emit_act(g, dst[0:Cout, 1 + y0:1 + y0 + nr, 1:1 + Wout], ps,
                     Cout, nr * Wout, act)
        tiles.append((ps, y0, nr))
        y0 += nr
    return tiles


def emit_act(g, dst_ap, ps, C, n, act):
    """dst = lrelu(ps) (or copy). lrelu = max(0.01*ps, ps): ACT mul + DVE max."""
    nc = g.nc
    if act == 'lrelu':
        tmp = g.sbuf.tile([128, 512], F32, tag="evtmp")
        nc.scalar.mul(tmp[0:C, 0:n], ps[:], ALPHA)
        nc.vector.tensor_max(dst_ap, tmp[0:C, 0:n], ps[:])
    else:
        nc.scalar.copy(dst_ap, ps[:])


def zero_border(g, buf, C, Hp, Wp):
    """zero only the 1-px border of a padded (C, Hp, Wp) buffer."""
    nc = g.nc
    nc.gpsimd.memset(buf[0:C, 0:1, :], 0.0)
    nc.gpsimd.memset(buf[0:C, Hp - 1:Hp, :], 0.0)
    nc.gpsimd.memset(buf[0:C, 1:Hp - 1, 0:1], 0.0)
    nc.gpsimd.memset(buf[0:C, 1:Hp - 1, Wp - 1:Wp], 0.0)


def emit_bn(g, ds_tiles, C, npx, b_ap, g_ap, bb_ap, dsf32, ds16_dst):
    """BN with batch stats. ds_tiles: psum tiles from ds conv (list of
    (ps, y0, nr) covering (C, H, W)); evict to dsf32 (C, npx-ish 3D or 2D)
    with accum sums; then stats + apply -> ds16_dst (fp16)."""
    nc = g.nc
    nt = len(ds_tiles)
    acc = g.sbuf.tile([C, nt], F32, tag="bn_acc")
    for i, (ps, y0, nr) in enumerate(ds_tiles):
        nc.scalar.activation(dsf32[0:C, y0:y0 + nr, :], ps[:],
                             mybir.ActivationFunctionType.Identity,
                             bias=b_ap, scale=1.0,
                             accum_out=acc[:, i:i + 1])
    ssum = g.sbuf.tile([C, 1], F32, tag="bn_s")
    if nt > 1:
        nc.vector.tensor_reduce(ssum[:], acc[:], mybir.AxisListType.X,
                                mybir.AluOpType.add)
    else:
        nc.vector.tensor_copy(ssum[:], acc[:])
    sq = g.sbuf.tile([C, 1], F32, tag="bn_sq")
    scr = g.scratch  # (128, 2080) f32 scratch
    nc.scalar.activation(scr[0:C, 0:npx], dsf32[0:C].opt(),
                         mybir.ActivationFunctionType.Square,
                         accum_out=sq[:])
    inv_n = 1.0 / npx
    mean = g.sbuf.tile([C, 1], F32, tag="bn_m")
    nc.scalar.mul(mean[:], ssum[:], inv_n)
    ex2 = g.sbuf.tile([C, 1], F32, tag="bn_e")
    nc.scalar.mul(ex2[:], sq[:], inv_n)
    m2 = g.sbuf.tile([C, 1], F32, tag="bn_m2")
    nc.vector.tensor_mul(m2[:], mean[:], mean[:])
    var = g.sbuf.tile([C, 1], F32, tag="bn_v")
    nc.vector.tensor_sub(var[:], ex2[:], m2[:])
    nc.vector.tensor_scalar_add(var[:], var[:], EPS)
    std = g.sbuf.tile([C, 1], F32, tag="bn_std")
    nc.scalar.activation(std[:], var[:], mybir.ActivationFunctionType.Sqrt,
                         bias=0.0, scale=1.0)
    istd = g.sbuf.tile([C, 1], F32, tag="bn_istd")
    nc.vector.reciprocal(istd[:], std[:])
    s = g.sbuf.tile([C, 1], F32, tag="bn_sc")
    nc.vector.tensor_mul(s[:], g_ap, istd[:])
    ms = g.sbuf.tile([C, 1], F32, tag="bn_ms")
    nc.vector.tensor_mul(ms[:], mean[:], s[:])
    t = g.sbuf.tile([C, 1], F32, tag="bn_t")
    nc.vector.tensor_sub(t[:], bb_ap, ms[:])
    nc.vector.tensor_scalar(ds16_dst[:], dsf32[0:C].opt(), s[:], t[:],
                            mybir.AluOpType.mult, mybir.AluOpType.add)


def emit_matvec_op(g, w_ap, nk, nm, rhs_cols, biasrow_ap, act, out16, psum_tag):
    """out-on-partitions matvec: w_ap (128, nk*nm*128) blocks; rhs_cols
    (128, nk) fp16; psum (128, nm); biasrow (1, 128*nm) f16 folded via
    ones-matmul; act lrelu or none; out16 (128, nm) fp16 (or f32)."""
    nc = g.nc
    ps = g.psum.tile([128, nm], F32, tag="mm")
    for m in range(nm):
        for k in range(nk):
            nc.tensor.matmul(ps[:, m:m + 1],
                             w_ap[:, (k * nm + m) * 128:(k * nm + m) * 128 + 128],
                             rhs_cols[:, k:k + 1],
                             start=(k == 0), stop=False)
        nc.tensor.matmul(ps[:, m:m + 1], biasrow_ap[0:1, m * 128:(m + 1) * 128],
                         g.ones[0:1, 0:1], start=False, stop=True)
    emit_act(g, out16[:], ps, 128, nm, act)


def build_program():
    nc = bacc.Bacc("TRN2", target_bir_lowering=False, debug=False,
                   num_devices=N_CORES)
    g = Ctx()
    g.nc = nc

    def inp(name, shape, dt):
        return nc.dram_tensor(name, list(shape), dt, kind="ExternalInput").ap()

    # --- declare I/O ---
    I = {}
    I['xpatch'] = inp('xpatch', (9, 2080), F16)
    I['pack16'] = inp('pack16', (128, TOT16), F16)
    I['pack32'] = inp('pack32', (128, TOT32), F32)
    I['lt1w'] = inp('lt1w', (128, 136 * 64), F16)
    I['ltb1cr'] = inp('ltb1cr', (1, 64), F16)
    I['rev3w'] = inp('rev3w', (128, 4 * 42 * 128), F16)
    I['rev3br'] = inp('rev3br', (1, 5376), F32)
    for i, (wd, nk, nt) in enumerate([(WD0, NK_LT1, 17), (WD1, 25, 25)]):
        p = f'd{i}_'
        I[p + 'fb1r'] = inp(p + 'fb1r', (1, 64), F16)
        I[p + 'fw1'] = inp(p + 'fw1', (128, nk * 64), F16)
        I[p + 'fw3'] = inp(p + 'fw3', (128, 2 * nt * 128), F16)

    O = {}
    O['d0'] = nc.dram_tensor('d0', [H, 64], F32, kind="ExternalOutput").ap()
    O['d1'] = nc.dram_tensor('d1', [H, 96], F32, kind="ExternalOutput").ap()
    O['m0'] = nc.dram_tensor('m0', [H, 1], F32, kind="ExternalOutput").ap()
    O['m1'] = nc.dram_tensor('m1', [H, 2], F32, kind="ExternalOutput").ap()

    # internal DRAM
    e0_dram = nc.dram_tensor('e0_dram', [FLAT0], F16)
    z1p_dram = nc.dram_tensor('z1p_dram', [64], F32)
    z1r_dram = nc.dram_tensor('z1r_dram', [512], F32, addr_space="Shared")
    rloc_dram = nc.dram_tensor('rloc_dram', [5376], F16)
    rall_dram = nc.dram_tensor('rall_dram', [43008], F16, addr_space="Shared")
    hh_dram = [nc.dram_tensor(f'hh{i}_dram', [128 * (NK_LT1, 25)[i]], F16)
               for i in range(2)]
    zf1_dram = nc.dram_tensor('zf1_dram', [128], F32)
    zfall_dram = nc.dram_tensor('zfall_dram', [1024], F32, addr_space="Shared")
    y_dram = [nc.dram_tensor(f'y{i}_dram', [128 * (17, 25)[i]], F32)
              for i in range(2)]

    rg = [list(range(N_CORES))]

    with tile.TileContext(nc) as tc:
        with (
            tc.tile_pool(name="sbuf", bufs=1) as sbuf,
            tc.tile_pool(name="wstream", bufs=2) as wstream,
            tc.tile_pool(name="psum", bufs=3, space="PSUM") as psum,
        ):
            g.sbuf, g.psum = sbuf, psum
            D = I
            I = {}
            for _n, _ap in D.items():
                if _n in ('rev3w', 'lt1w', 'xpatch', 'pack16', 'pack32'):
                    continue
                _t = sbuf.tile(list(_ap.shape), _ap.dtype, tag="in_" + _n)
                nc.sync.dma_start(_t[:], _ap)
                I[_n] = _t
            pk16 = sbuf.tile([128, TOT16], F16, tag="pack16")
            nc.sync.dma_start(pk16[:], D['pack16'])
            pk32 = sbuf.tile([128, TOT32], F32, tag="pack32")
            nc.sync.dma_start(pk32[:], D['pack32'])
            for _n, (_o, _p, _w) in OFF16.items():
                I[_n] = pk16[0:_p, _o:_o + _w]
            for _n, (_o, _p, _w) in OFF32.items():
                I[_n] = pk32[0:_p, _o:_o + _w]
            g.ident = I['ident']
            g.ones = I['ones']
            g.scratch = sbuf.tile([128, 800], F32, tag="scratch")

            # ================= ENCODER =================
            B0 = sbuf.tile([32, 34, 67], F16, tag="big1")
            nc.gpsimd.memset(B0[:], 0.0)
            # L0: K=9 im2col; row tiles of 7; patches streamed per tile
            y0 = 0
            while y0 < 32:
                nr = min(7, 32 - y0)
                xp = wstream.tile([9, 512], F16, tag="xp")
                nc.sync.dma_start(xp[0:9, 0:nr * 65],
                                  D['xpatch'][:, y0 * 65:(y0 + nr) * 65])
                ps = psum.tile([32, nr, 65], F32, tag="mm")
                nc.tensor.matmul(ps[:], I['ew0'], xp[0:9, 0:nr * 65],
                                 start=True, stop=False)
                nc.tensor.matmul(ps[:], I['eb0r'], g.ones[0:1, 0:nr * 65],
                                 start=False, stop=True)
                emit_act(g, B0[0:32, 1 + y0:1 + y0 + nr, 1:66], ps, 32, nr * 65,
                         'lrelu')
                y0 += nr
            # rb1 (32->64, s2): c1
            B1 = sbuf.tile([64, 18, 35], F16, tag="B1")
            nc.gpsimd.memset(B1[:], 0.0)
            emit_conv(g, 'e_rb1c1', B0, B1, 32, 64, 32, 65, 2, I['ew11'],
                      I['eb11r'], 'lrelu', rows_per_tile=8)
            # rb1 ds (1x1 s2) + bn
            ds_tiles = []
            for (ty, nr) in [(0, 8), (8, 8)]:
                ps = psum.tile([64, nr, 33], F32, tag="mm")
                rhs = B0[0:32, 1 + 2 * ty: 1 + 2 * ty + 2 * nr: 2, 1:67:2]
                nc.tensor.matmul(ps[:], I['ewd1'], rhs, start=True, stop=True)
                ds_tiles.append((ps, ty, nr))
            dsA_f32 = sbuf.tile([64, 16, 33], F32, tag="bigf32")
            dsA16 = sbuf.tile([64, 16, 33], F16, tag="dsA16")
            emit_bn(g, ds_tiles, 64, 528, I['ebd1'], I['eg1'], I['ebn1'],
                    dsA_f32, dsA16)
            # rb1 c2 + identity add
            B2 = sbuf.tile([64, 18, 35], F16, tag="B2")
            nc.gpsimd.memset(B2[:], 0.0)
            emit_conv(g, 'e_rb1c2', B1, B2, 64, 64, 16, 33, 1, I['ew12'],
                      I['eb12r'], 'lrelu', rows_per_tile=8,
                      extra_ident_rhs=dsA16)
            # rb2 (64->128, s2)
            B3 = sbuf.tile([128, 10, 19], F16, tag="B3")
            nc.gpsimd.memset(B3[:], 0.0)
            emit_conv(g, 'e_rb2c1', B2, B3, 64, 128, 16, 33, 2, I['ew21'],
                      I['eb21r'], 'lrelu')
            ps = psum.tile([128, 8, 17], F32, tag="mm")
            nc.tensor.matmul(ps[:], I['ewd2'], B2[0:64, 1:17:2, 1:35:2],
                             start=True, stop=True)
            dsB_f32 = sbuf.tile([128, 8, 17], F32, tag="dsB_f32")
            dsB16 = sbuf.tile([128, 8, 17], F16, tag="dsB16")
            emit_bn(g, [(ps, 0, 8)], 128, 136, I['ebd2'], I['eg2'], I['ebn2'],
                    dsB_f32, dsB16)
            B4 = sbuf.tile([128, 10, 19], F16, tag="B4")
            nc.gpsimd.memset(B4[:], 0.0)
            emit_conv(g, 'e_rb2c2', B3, B4, 128, 128, 8, 17, 1, I['ew22'],
                      I['eb22r'], 'lrelu', extra_ident_rhs=dsB16)
            # rb3 (128->128, s1, no ds)
            B5 = sbuf.tile([128, 10, 19], F16, tag="B5")
            nc.gpsimd.memset(B5[:], 0.0)
            emit_conv(g, 'e_rb3c1', B4, B5, 128, 128, 8, 17, 1, I['ew31'],
                      I['eb31r'], 'lrelu')
            B6 = sbuf.tile([128, 10, 19], F16, tag="B6")
            nc.gpsimd.memset(B6[:], 0.0)
            emit_conv(g, 'e_rb3c2', B5, B6, 128, 128, 8, 17, 1, I['ew32'],
                      I['eb32r'], 'lrelu', extra_ident_rhs=B4[0:128, 1:9, 1:18])

            # e0 export + reload as k-chunk columns (full 136 chunks)
            nc.sync.dma_start(e0_dram.ap(), B6[0:128, 1:9, 1:18])
            e0c = sbuf.tile([128, 136], F16, tag="e0c")
            e0r = e0_dram.ap().rearrange("(a b) -> b a", b=128)
            nc.sync.dma_start(e0c[:], e0r)

            # ====== LT1 output-sharded (64 outputs per core) + AllGather ======
            psz = psum.tile([64, 1], F32, tag="mm")
            for kb in range(4):
                lt1b = wstream.tile([128, 34 * 64], F16, tag="lt1b")
                nc.sync.dma_start(lt1b[:], D['lt1w'][:, kb * 2176:(kb + 1) * 2176])
                for kk in range(34):
                    k = 34 * kb + kk
                    nc.tensor.matmul(psz[:], lt1b[:, kk * 64:(kk + 1) * 64],
                                     e0c[:, k:k + 1],
                                     start=(k == 0), stop=False)
            nc.tensor.matmul(psz[:], I['ltb1cr'], g.ones[0:1, 0:1],
                             start=False, stop=True)
            z1p = sbuf.tile([64, 1], F32, tag="z1p")
            emit_act(g, z1p[:], psz, 64, 1, 'lrelu')
            nc.sync.dma_start(z1p_dram.ap(), z1p[:])
            nc.gpsimd.collective_compute(
                "AllGather", mybir.AluOpType.bypass, replica_groups=rg,
                ins=[z1p_dram.ap()], outs=[z1r_dram.ap()])
            z1g = sbuf.tile([128, 4], F32, tag="z1g")
            nc.sync.dma_start(z1g[:], z1r_dram.ap().rearrange("(a b) -> b a", b=128))
            z16 = sbuf.tile([128, 4], F16, tag="z16")
            nc.vector.tensor_copy(z16[:], z1g[:])

            # ================= mids =================
            mids = [('mw2', 'mb2', 4, 2), ('mw3', 'mb3', 2, 1),
                    ('mw4', 'mb4', 1, 2), ('mw5', 'mb5', 2, 4)]
            zcur = z16
            for wn, bn, nk, nm in mids:
                wt = I[wn]
                znext = sbuf.tile([128, nm], F16, tag=wn + "_z")
                emit_matvec_op(g, wt, nk, nm, zcur, I[bn + 'r'], 'lrelu', znext, "mid")
                zcur = znext

            # ================= rev3 + AllGather =================
            # rhs-streaming, nt-major blocks; per-tile DMA out to dram
            NT_R3 = [512] * 10 + [256]
            off = 0
            for wnt in NT_R3:
                wck = wstream.tile([128, 4 * 512], F16, tag="rev3wc")
                nc.sync.dma_start(wck[0:128, 0:4 * wnt],
                                  D['rev3w'][:, 4 * off:4 * off + 4 * wnt])
                ps = psum.tile([1, wnt], F32, tag="mm")
                for k in range(4):
                    nc.tensor.matmul(ps[:], zcur[:, k:k + 1],
                                     wck[0:128, k * wnt:(k + 1) * wnt],
                                     start=(k == 0), stop=(k == 3))
                rsb = wstream.tile([1, 512], F16, tag="rsb")
                nc.vector.scalar_tensor_tensor(
                    rsb[0:1, 0:wnt], ps[:], 1.0,
                    I['rev3br'][0:1, off:off + wnt],
                    mybir.AluOpType.mult, mybir.AluOpType.add)
                nc.sync.dma_start(rloc_dram.ap()[off:off + wnt], rsb[0:1, 0:wnt])
                off += wnt
            nc.gpsimd.collective_compute(
                "AllGather", mybir.AluOpType.bypass, replica_groups=rg,
                ins=[rloc_dram.ap()], outs=[rall_dram.ap()])

            # ================= decoders: conv chains + fc1 =================
            zf1both = sbuf.tile([128, 1], F32, tag="zf1both")
            WDM = WD1
            sIn = sbuf.tile([128, 4, WDM + 2], F16, tag="d_sIn")
            A1 = sbuf.tile([64, 4, WDM + 2], F16, tag="d_A1")
            A2 = sbuf.tile([64, 4, WDM + 2], F16, tag="d_A2")
            A3 = sbuf.tile([64, 4, WDM + 2], F16, tag="d_A3")
            B1d = sbuf.tile([64, 6, 2 * WDM + 2], F16, tag="d_B1d")
            C1 = sbuf.tile([32, 6, 2 * WDM + 2], F16, tag="d_C1")
            C2 = sbuf.tile([32, 6, 2 * WDM + 2], F16, tag="d_C2")
            D1 = sbuf.tile([32, 10, 4 * WDM + 2], F16, tag="big1")
            z1sh = sbuf.tile([8, 3, 4 * WDM + 2], F16, tag="d_z1sh")
            for _b in (sIn, A1, A2, A3, B1d, C1, C2, D1, z1sh):
                nc.gpsimd.memset(_b[:], 0.0)
            for di, wd in enumerate([WD0, WD1]):
                p = f'd{di}_'
                w4 = 4 * wd
                npx3 = 8 * w4 // 4  # = 2*w4? no: level3 pixels = 8 * (4*wd) / 4
                # level sizes: L1 (H=2, wd), L2 (H=4, 2wd), L3 (H=8, 4wd)
                w2 = 2 * wd
                # -- weights
                wts = {wn: I[p + wn] for wn in
                       ['w_in', 'rb1w1', 'rb1w2', 'ct1w', 'rb2w1', 'rb2w2',
                        'rb2ds', 'ct2w', 'rb3w1', 'rb3ds']}
                rb3w2 = I[p + 'rb3w2']

                off = 0 if di == 0 else FLAT0
                rsl = rall_dram.ap()[off:off + 128 * 2 * wd].rearrange(
                    "(c h w) -> c h w", c=128, h=2)
                nc.sync.dma_start(sIn[0:128, 1:3, 1:1 + wd], rsl)
                emit_conv(g, p + 'cin', sIn, A1, 128, 64, 2, wd, 1,
                          wts['w_in'], I[p + 'b_inr'], 'lrelu')
                emit_conv(g, p + 'rb1c1', A1, A2, 64, 64, 2, wd, 1,
                          wts['rb1w1'], I[p + 'rb1b1r'], 'lrelu')
                emit_conv(g, p + 'rb1c2', A2, A3, 64, 64, 2, wd, 1,
                          wts['rb1w2'], I[p + 'rb1b2r'], 'lrelu',
                          extra_ident_rhs=A1[0:64, 1:3, 1:1 + wd])
                # ct1: 64->64, L1 (2, wd) -> L2 (4, 2wd)
                TAPS = {0: [(1, 0)], 1: [(2, 0), (0, 1)]}
                for q in (0, 1):
                    for d in (0, 1):
                        taps = [(ky, kx, dy, dx) for (ky, dy) in TAPS[q]
                                for (kx, dx) in TAPS[d]]
                        ps = psum.tile([64, 2, wd], F32, tag="mm")
                        for mi, (ky, kx, dy, dx) in enumerate(taps):
                            t = 3 * ky + kx
                            rhs = A3[0:64, 1 + dy:3 + dy, 1 + dx:1 + dx + wd]
                            nc.tensor.matmul(ps[:], wts['ct1w'][:, t * 64:(t + 1) * 64],
                                             rhs, start=(mi == 0), stop=False)
                        nc.tensor.matmul(ps[:], I[p + 'ct1br'],
                                         g.ones[0:1, 0:2 * wd],
                                         start=False, stop=True)
                        emit_act(g, B1d[0:64, 1 + q:1 + q + 4:2, 1 + d:1 + d + w2:2],
                                 ps, 64, 2 * wd, 'lrelu')
                # rb2: 64->32 with ds+bn, at L2 (4, w2)
                rpt = 512 // w2
                emit_conv(g, p + 'rb2c1', B1d, C1, 64, 32, 4, w2, 1,
                          wts['rb2w1'], I[p + 'rb2b1r'], 'lrelu', rows_per_tile=rpt)
                ds_tiles = []
                y0 = 0
                while y0 < 4:
                    nr = min(rpt, 4 - y0)
                    ps = psum.tile([32, nr, w2], F32, tag="mm")
                    nc.tensor.matmul(ps[:], wts['rb2ds'],
                                     B1d[0:64, 1 + y0:1 + y0 + nr, 1:1 + w2],
                                     start=True, stop=True)
                    ds_tiles.append((ps, y0, nr))
                    y0 += nr
                dsC_f32 = sbuf.tile([32, 4, w2], F32, tag="bigf32")
                dsC16 = sbuf.tile([32, 4, w2], F16, tag="d_dsC16")
                emit_bn(g, ds_tiles, 32, 4 * w2, I[p + 'rb2dsb'], I[p + 'rb2g'],
                        I[p + 'rb2bb'], dsC_f32, dsC16)
                emit_conv(g, p + 'rb2c2', C1, C2, 32, 32, 4, w2, 1,
                          wts['rb2w2'], I[p + 'rb2b2r'], 'lrelu',
                          rows_per_tile=rpt, extra_ident_rhs=dsC16)
                # ct2: 32->32, L2 (4, w2) -> L3 (8, w4)
                for q in (0, 1):
                    for d in (0, 1):
                        taps = [(ky, kx, dy, dx) for (ky, dy) in TAPS[q]
                                for (kx, dx) in TAPS[d]]
                        y0 = 0
                        while y0 < 4:
                            nr = min(rpt, 4 - y0)
                            ps = psum.tile([32, nr, w2], F32, tag="mm")
                            for mi, (ky, kx, dy, dx) in enumerate(taps):
                                t = 3 * ky + kx
                                rhs = C2[0:32, 1 + y0 + dy:1 + y0 + dy + nr,
                                         1 + dx:1 + dx + w2]
                                nc.tensor.matmul(ps[:], wts['ct2w'][:, t * 32:(t + 1) * 32],
                                                 rhs, start=(mi == 0), stop=False)
                            nc.tensor.matmul(ps[:], I[p + 'ct2br'],
                                             g.ones[0:1, 0:nr * w2],
                                             start=False, stop=True)
                            emit_act(g, D1[0:32, 1 + 2 * y0 + q:1 + 2 * y0 + q + 2 * nr:2,
                                           1 + d:1 + d + w4:2],
                                     ps, 32, nr * w2, 'lrelu')
                            y0 += nr
                # ---- rb3 tail (32 -> 1) at L3 (8, w4) ----
                npx = 8 * w4
                npx2 = npx // 2
                z1f = sbuf.tile([1, npx], F16, tag="d_flat1")
                for hf in range(2):
                    P = sbuf.tile([128, 3, npx2], F16, tag="d_patches")
                    for dy in range(3):
                        for dx in range(3):
                            t = 3 * dy + dx
                            srcw = D1[0:32, dy + 4 * hf:dy + 4 * hf + 4,
                                      dx:dx + w4]
                            nc.sync.dma_start(
                                P[(32 * t) % 128:(32 * t) % 128 + 32,
                                  t // 4, 0:npx2], srcw)
                    n0 = 0
                    while n0 < npx2:
                        nn = min(512, npx2 - n0)
                        ps = psum.tile([1, nn], F32, tag="mm")
                        for j, kr in ((0, 128), (1, 128), (2, 32)):
                            nc.tensor.matmul(ps[:], wts['rb3w1'][0:kr, j:j + 1],
                                             P[0:kr, j, n0:n0 + nn],
                                             start=(j == 0), stop=False)
                        nc.tensor.matmul(ps[:], I[p + 'rb3b1r'],
                                         g.ones[0:1, 0:nn],
                                         start=False, stop=True)
                        emit_act(g, z1f[:, hf * npx2 + n0:hf * npx2 + n0 + nn],
                                 ps, 1, nn, 'lrelu')
                        n0 += nn
                for dy in range(3):
                    p0 = max(0, 1 - dy)
                    p1 = min(8, 9 - dy)
                    r0 = p0 + dy - 1
                    r1 = p1 + dy - 1
                    nc.sync.dma_start(
                        z1sh[p0:p1, dy, 1:1 + w4],
                        z1f[0:1, r0 * w4:r1 * w4].rearrange(
                            "a (h w) -> a h w", w=w4))
                # conv2 1->1 on H-partition layout (DVE); rows pre-shifted
                acc = sbuf.tile([8, w4], F32, tag="d_acc")
                nc.gpsimd.memset(acc[:], 0.0)
                for dy in range(3):
                    for dx in range(3):
                        t = 3 * dy + dx
                        nc.vector.scalar_tensor_tensor(
                            acc[:], z1sh[0:8, dy, dx:dx + w4],
                            rb3w2[:, t:t + 1], acc[:],
                            mybir.AluOpType.mult, mybir.AluOpType.add)
                # ds 32->1 + bn
                dsD = sbuf.tile([1, npx], F16, tag="d_flat2")
                dacc = sbuf.tile([1, 8], F32, tag=p + "dacc")
                for r in range(8):
                    ps = psum.tile([1, w4], F32, tag="mm")
                    nc.tensor.matmul(ps[:], wts['rb3ds'],
                                     D1[0:32, 1 + r, 1:1 + w4],
                                     start=True, stop=True)
                    nc.scalar.activation(dsD[:, r * w4:(r + 1) * w4], ps[:],
                                         mybir.ActivationFunctionType.Identity,
                                         bias=I[p + 'rb3dsb'], scale=1.0,
                                         accum_out=dacc[:, r:r + 1])
                dsum = sbuf.tile([1, 1], F32, tag=p + "dsum")
                nc.vector.tensor_reduce(dsum[:], dacc[:], mybir.AxisListType.X,
                                        mybir.AluOpType.add)
                dacc2 = sbuf.tile([1, 4], F32, tag=p + "dacc2")
                qn = npx // 4
                for qq in range(4):
                    nc.scalar.activation(g.scratch[0:1, 0:qn],
                                         dsD[0:1, qq * qn:(qq + 1) * qn],
                                         mybir.ActivationFunctionType.Square,
                                         accum_out=dacc2[:, qq:qq + 1])
                dsq = sbuf.tile([1, 1], F32, tag=p + "dsq")
                nc.vector.tensor_reduce(dsq[:], dacc2[:], mybir.AxisListType.X,
                                        mybir.AluOpType.add)
                inv_n = 1.0 / npx
                dmean = sbuf.tile([1, 1], F32, tag=p + "dmean")
                nc.scalar.mul(dmean[:], dsum[:], inv_n)
                dex2 = sbuf.tile([1, 1], F32, tag=p + "dex2")
                nc.scalar.mul(dex2[:], dsq[:], inv_n)
                dm2 = sbuf.tile([1, 1], F32, tag=p + "dm2")
                nc.vector.tensor_mul(dm2[:], dmean[:], dmean[:])
                dvar = sbuf.tile([1, 1], F32, tag=p + "dvar")
                nc.vector.tensor_sub(dvar[:], dex2[:], dm2[:])
                nc.vector.tensor_scalar_add(dvar[:], dvar[:], EPS)
                dstd = sbuf.tile([1, 1], F32, tag=p + "dstd")
                nc.scalar.activation(dstd[:], dvar[:],
                                     mybir.ActivationFunctionType.Sqrt,
                                     bias=0.0, scale=1.0)
                distd = sbuf.tile([1, 1], F32, tag=p + "distd")
                nc.vector.reciprocal(distd[:], dstd[:])
                dsc = sbuf.tile([1, 1], F32, tag=p + "dsc")
                nc.vector.tensor_mul(dsc[:], I[p + 'rb3g'], distd[:])
                dms = sbuf.tile([1, 1], F32, tag=p + "dms")
                nc.vector.tensor_mul(dms[:], dmean[:], dsc[:])
                dt_ = sbuf.tile([1, 1], F32, tag=p + "dt")
                nc.vector.tensor_sub(dt_[:], I[p + 'rb3bb'], dms[:])
                nc.vector.tensor_scalar(dsD[:], dsD[:], dsc[:], dt_[:],
                                        mybir.AluOpType.mult, mybir.AluOpType.add)
                dsimg = sbuf.tile([8, w4], F16, tag="d_dsimg")
                nc.gpsimd.dma_start(dsimg[:],
                                    dsD[:].rearrange("a (h w) -> a h w", h=8))
                hsum = sbuf.tile([8, w4], F32, tag="d_hsum")
                nc.vector.scalar_tensor_tensor(hsum[:], acc[:],
                                               I[p + 'rb3b2p8'][:],
                                               dsimg[:],
                                               mybir.AluOpType.add,
                                               mybir.AluOpType.add)
                hh16 = sbuf.tile([8, w4], F16, tag="d_hh16")
                htmp = sbuf.tile([8, w4], F32, tag="d_htmp")
                nc.scalar.mul(htmp[:], hsum[:], ALPHA)
                nc.vector.tensor_max(hh16[:], htmp[:], hsum[:])
                nc.sync.dma_start(
                    hh_dram[di].ap()[0:npx].rearrange("(h w) -> h w", h=8), hh16[:])
                nk = (NK_LT1, 25)[di]
                hT = sbuf.tile([128, nk], F16, tag=p + "hT")
                nc.sync.dma_start(hT[:],
                                  hh_dram[di].ap().rearrange("(a b) -> b a", b=128))
                # fc1 shard: 64 outputs
                fw1 = I[p + 'fw1']
                psf = psum.tile([64, 1], F32, tag="mm")
                for k in range(nk):
                    nc.tensor.matmul(psf[:], fw1[:, k * 64:(k + 1) * 64],
                                     hT[:, k:k + 1], start=(k == 0), stop=False)
                nc.tensor.matmul(psf[:], I[p + 'fb1r'], g.ones[0:1, 0:1],
                                 start=False, stop=True)
                emit_act(g, zf1both[64 * di:64 * di + 64, 0:1], psf, 64, 1,
                         'lrelu')

            # fused fc1 AllGather
            nc.sync.dma_start(zf1_dram.ap(), zf1both[:])
            nc.gpsimd.collective_compute(
                "AllGather", mybir.AluOpType.bypass, replica_groups=rg,
                ins=[zf1_dram.ap()], outs=[zfall_dram.ap()])

            # ================= decoders: fc2/fc3 + masking =================
            for di, (wimg, m) in enumerate([(W0, 1), (W1, 2)]):
                p = f'd{di}_'
                nt = (17, 25)[di]
                zfg = sbuf.tile([128, 4], F32, tag=p + "zfg")
                # zfall[128*c + 64*dec + j]; dec di's vector z[i], i = 64*c + j.
                # dst (p, k) holds z[128k + p]: c = 2k + p//64, j = p%64
                #   -> dram idx = 256k + 128*(p//64) + 64*di + p%64
                zview = zfall_dram.ap().rearrange("(k h j) -> h j k", h=4, j=64)
                # zview[h, j, k] = dram[256k + 64h + j]; need h = 2*(p//64) + di
                for half in range(2):
                    nc.sync.dma_start(
                        zfg[64 * half:64 * half + 64, 0:4],
                        zview[2 * half + di, :, :])
                zfg16 = sbuf.tile([128, 4], F16, tag=p + "zfg16")
                nc.vector.tensor_copy(zfg16[:], zfg[:])
                fw2 = I[p + 'fw2']
                zf2 = sbuf.tile([128, 2], F16, tag=p + "zf2")
                emit_matvec_op(g, fw2, 4, 2, zfg16, I[p + 'fb2r'], 'lrelu',
                               zf2, "mid")
                fw3 = I[p + 'fw3']
                npx3 = nt * 128
                NT3 = [512] * (npx3 // 512) + ([npx3 % 512] if npx3 % 512 else [])
                off = 0
                pos = 0
                for wnt in NT3:
                    ps = psum.tile([1, wnt], F32, tag="mm")
                    for k in range(2):
                        nc.tensor.matmul(ps[:], zf2[:, k:k + 1],
                                         fw3[0:128, pos + k * wnt:pos + (k + 1) * wnt],
                                         start=(k == 0), stop=(k == 1))
                    yfl = wstream.tile([1, 512], F32, tag="yfl")
                    nc.vector.scalar_tensor_tensor(
                        yfl[0:1, 0:wnt], ps[:], 1.0,
                        I[p + 'fb3r'][0:1, off:off + wnt],
                        mybir.AluOpType.mult, mybir.AluOpType.add)
                    nc.sync.dma_start(y_dram[di].ap()[off:off + wnt],
                                      yfl[0:1, 0:wnt])
                    pos += 2 * wnt
                    off += wnt
                ysb = sbuf.tile([H, wimg], F32, tag=p + "ysb")
                nc.sync.dma_start(ysb[:], y_dram[di].ap()[0:H * wimg]
                                  .rearrange("(h w) -> h w", h=H))
                # masking
                nz = sbuf.tile([H, m], F32, tag=p + "nz")
                nc.vector.tensor_scalar(nz[:], ysb[0:H, wimg - m:wimg], 0.0, None,
                                        mybir.AluOpType.is_gt)
                nc.sync.dma_start(O[f'm{di}'], nz[:])
                nzsq = sbuf.tile([H, 32], F32, tag=p + "nzsq")
                nc.gpsimd.memset(nzsq[:], 0.0)
                nc.vector.tensor_copy(nzsq[0:H, 0:m], nz[:])
                nzT = sbuf.tile([H, 32], F32, tag=p + "nzT")
                nc.vector.transpose(nzT[:], nzsq[:])
                AT = sbuf.tile([m + 1, 32], F32, tag=p + "AT")
                nc.sync.dma_start(AT[0:1, :], I['border'][0:1, :])
                nc.sync.dma_start(AT[1:1 + m, :], nzT[0:m, :])
                E = sbuf.tile([m + 1, 32 * (m + 1)], F32, tag=p + "E")
                nc.gpsimd.memset(E[:], 0.0)
                for j in range(m):
                    nc.sync.dma_start(E[j:j + 1, 32 * j:32 * (j + 1)],
                                      nzT[j:j + 1, 0:32])
                nc.sync.dma_start(E[m:m + 1, 32 * m:32 * (m + 1)],
                                  I['ones32f'][0:1, :])
                psm = psum.tile([H, 32 * (m + 1)], F32, tag="mm")
                nc.tensor.matmul(psm[:], AT[:], E[:], start=True, stop=True)
                dout = sbuf.tile([H, 32 * (m + 1)], F32, tag=p + "dout")
                nc.vector.scalar_tensor_tensor(dout[:], ysb[0:H, 0:32 * (m + 1)],
                                               1.0 / S_FC, psm[:],
                                               mybir.AluOpType.mult,
                                               mybir.AluOpType.mult)
                nc.sync.dma_start(O[f'd{di}'], dout[:])

    nc.compile()
    return nc


# ----------------------------------------------------------------------------
# host-side input prep
# ----------------------------------------------------------------------------

def prep_inputs(x, enc0_params, lt_params, rev_params, dec_params):
    """Returns list of 8 per-core input dicts."""
    f32 = lambda a: np.asarray(a, np.float32)
    f16 = lambda a: np.asarray(a, np.float32).astype(NP16)

    base = {}
    # L0 im2col patches from x (pure gather + zero pad)
    xi = f32(x)[0, 0]  # (32, 65)
    xpad = np.zeros((34, 67), np.float32)
    xpad[1:33, 1:66] = xi
    patches = np.zeros((9, 2080), np.float32)
    for dy in range(3):
        for dx in range(3):
            patches[3 * dy + dx] = xpad[dy:dy + 32, dx:dx + 65].reshape(-1)
    base['xpatch'] = f16(patches)

    e = enc0_params
    base['ew0'] = f16(f32(e['w0'])[:, 0].reshape(32, 9).T)
    base['eb0'] = col1(e['b0'])
    base['ew11'] = pack_conv(f32(e['rb1']['w1']))
    base['eb11'] = col1(e['rb1']['b1'])
    base['ew12'] = pack_conv(f32(e['rb1']['w2']))
    base['eb12'] = col1(e['rb1']['b2'])
    base['ewd1'] = f16(f32(e['rb1']['ds_w'])[:, :, 0, 0].T)
    base['ebd1'] = col1(e['rb1']['ds_b'])
    base['eg1'] = col1(e['rb1']['bn_g'])
    base['ebn1'] = col1(e['rb1']['bn_b'])
    base['ew21'] = pack_conv(f32(e['rb2']['w1']))
    base['eb21'] = col1(e['rb2']['b1'])
    base['ew22'] = pack_conv(f32(e['rb2']['w2']))
    base['eb22'] = col1(e['rb2']['b2'])
    base['ewd2'] = f16(f32(e['rb2']['ds_w'])[:, :, 0, 0].T)
    base['ebd2'] = col1(e['rb2']['ds_b'])
    base['eg2'] = col1(e['rb2']['bn_g'])
    base['ebn2'] = col1(e['rb2']['bn_b'])
    base['ew31'] = pack_conv(f32(e['rb3']['w1']))
    base['eb31'] = col1(e['rb3']['b1'])
    base['ew32'] = pack_conv(f32(e['rb3']['w2']))
    base['eb32'] = col1(e['rb3']['b2'])
    base['ident'] = np.eye(128, dtype=NP16)
    base['ones'] = np.ones((1, 512), NP16)
    base['ones32f'] = np.ones((1, 32), np.float32)
    row16 = lambda a, s=1.0: (np.asarray(a, np.float32) * np.float32(s)).reshape(1, -1).astype(NP16)
    base['eb0r'] = row16(e['b0'])
    base['eb11r'] = row16(e['rb1']['b1'])
    base['eb12r'] = row16(e['rb1']['b2'])
    base['eb21r'] = row16(e['rb2']['b1'])
    base['eb22r'] = row16(e['rb2']['b2'])
    base['eb31r'] = row16(e['rb3']['b1'])
    base['eb32r'] = row16(e['rb3']['b2'])
    base['mb2r'] = row16(lt_params['b2'])
    base['mb3r'] = row16(lt_params['b3'])
    base['mb4r'] = row16(rev_params['b1'])
    base['mb5r'] = row16(rev_params['b2'])

    for i, (wn, bn, nk, nm) in enumerate([('mw2', 'mb2', 4, 2), ('mw3', 'mb3', 2, 1),
                                          ('mw4', 'mb4', 1, 2), ('mw5', 'mb5', 2, 4)]):
        src = [lt_params, lt_params, rev_params, rev_params][i]
        key = ['w2', 'w3', 'w1', 'w2'][i]
        w = f32(src[key])          # (out, in)
        b = f32(src[key.replace('w', 'b')])
        base[wn] = pack_matvec(w.T, nk, nm)
        base[bn] = b.reshape(nm, 128).T.copy()

    border = np.ones((1, 32), np.float32)
    border[0, [0, 1, 30, 31]] = 0.0
    base['border'] = border

    # decoder shared (replicated) weights
    for di in range(2):
        d = dec_params[di]
        p = f'd{di}_'
        S = np.float32(S_REV3)
        base[p + 'w_in'] = pack_conv(f32(d['w_in']))
        base[p + 'b_in'] = col1(f32(d['b_in']) * S)
        base[p + 'rb1w1'] = pack_conv(f32(d['rb1']['w1']))
        base[p + 'rb1b1'] = col1(f32(d['rb1']['b1']) * S)
        base[p + 'rb1w2'] = pack_conv(f32(d['rb1']['w2']))
        base[p + 'rb1b2'] = col1(f32(d['rb1']['b2']) * S)
        base[p + 'ct1w'] = pack_convt(f32(d['ct1_w']))
        base[p + 'ct1b'] = col1(f32(d['ct1_b']) * S)
        base[p + 'rb2w1'] = pack_conv(f32(d['rb2']['w1']) / S)
        base[p + 'rb2b1'] = col1(d['rb2']['b1'])
        base[p + 'rb2w2'] = pack_conv(f32(d['rb2']['w2']))
        base[p + 'rb2b2'] = col1(d['rb2']['b2'])
        base[p + 'rb2ds'] = f16(f32(d['rb2']['ds_w'])[:, :, 0, 0].T / S)
        base[p + 'rb2dsb'] = col1(d['rb2']['ds_b'])
        base[p + 'rb2g'] = col1(d['rb2']['bn_g'])
        base[p + 'rb2bb'] = col1(d['rb2']['bn_b'])
        base[p + 'ct2w'] = pack_convt(f32(d['ct2_w']))
        base[p + 'ct2b'] = col1(d['ct2_b'])
        base[p + 'b_inr'] = row16(d['b_in'], S)
        base[p + 'rb1b1r'] = row16(d['rb1']['b1'], S)
        base[p + 'rb1b2r'] = row16(d['rb1']['b2'], S)
        base[p + 'ct1br'] = row16(d['ct1_b'], S)
        base[p + 'rb2b1r'] = row16(d['rb2']['b1'])
        base[p + 'rb2b2r'] = row16(d['rb2']['b2'])
        base[p + 'ct2br'] = row16(d['ct2_b'])
        base[p + 'rb3b1r'] = row16(d['rb3']['b1'])
        base[p + 'fb2r'] = row16(d['fc2_b'], S_FC)
        # rb3: conv1 32->1: flat k = cin + 32*t -> chunks (128, 3)
        w1 = f32(d['rb3']['w1'])  # (1, 32, 3, 3)
        flat = np.zeros(384, np.float32)
        for dy in range(3):
            for dx in range(3):
                t = 3 * dy + dx
                flat[32 * t:32 * t + 32] = w1[0, :, dy, dx]
        base[p + 'rb3w1'] = f16(flat.reshape(3, 128).T)
        base[p + 'rb3b1'] = col1(d['rb3']['b1'])
        w2 = f32(d['rb3']['w2'])[0, 0]  # (3,3)
        base[p + 'rb3w2'] = np.tile(w2.reshape(1, 9), (8, 1)).astype(np.float32)
        base[p + 'rb3b2'] = col1(d['rb3']['b2'])
        base[p + 'rb3b2p8'] = np.full((8, 1), np.float32(np.asarray(d['rb3']['b2']).ravel()[0]), np.float32)
        base[p + 'rb3ds'] = f16(f32(d['rb3']['ds_w'])[:, :, 0, 0].T)
        base[p + 'rb3dsb'] = col1(d['rb3']['ds_b'])
        base[p + 'rb3g'] = col1(d['rb3']['bn_g'])
        base[p + 'rb3bb'] = col1(d['rb3']['bn_b'])
        # fc2 / fc3 (replicated)
        w2f = f32(d['fc2_w'])
        base[p + 'fw2'] = pack_matvec(w2f.T, 4, 2)
        base[p + 'fb2'] = (f32(d['fc2_b']) * S_FC).reshape(2, 128).T.copy()
        nt = (17, 25)[di]
        w3 = f32(d['fc3_w'])      # (2080/3136, 256)
        w3p = np.zeros((nt * 128, 256), np.float32)
        w3p[:w3.shape[0]] = w3
        w3pT = np.ascontiguousarray(w3p.T)    # (256, nt*128)
        npx3 = nt * 128
        fw3 = np.zeros((128, 2 * npx3), NP16)
        pos = 0
        off = 0
        for wnt in [512] * (npx3 // 512) + ([npx3 % 512] if npx3 % 512 else []):
            for k in range(2):
                fw3[:, pos:pos + wnt] = w3pT[128 * k:128 * (k + 1), off:off + wnt]
                pos += wnt
            off += wnt
        base[p + 'fw3'] = fw3
        b3p = np.zeros(nt * 128, np.float32)
        b3p[:w3.shape[0]] = f32(d['fc3_b']) * S_FC
        base[p + 'fb3r'] = b3p.reshape(1, -1)

    # assemble packed small-input tensors (same for all cores)
    pk16 = np.zeros((128, TOT16), NP16)
    for name, (off, pp, ww) in OFF16.items():
        a = np.asarray(base[name], NP16)
        assert a.shape == (pp, ww), (name, a.shape, (pp, ww))
        pk16[:pp, off:off + ww] = a
    base['pack16'] = pk16
    pk32 = np.zeros((128, TOT32), np.float32)
    for name, (off, pp, ww) in OFF32.items():
        a = np.asarray(base[name], np.float32)
        assert a.shape == (pp, ww), (name, a.shape, (pp, ww))
        pk32[:pp, off:off + ww] = a
    base['pack32'] = pk32

    # per-core shards
    W1eff = f32(lt_params['w1'])[:, :FLAT0]    # (512, 17408)
    W1T = W1eff.T                              # (17408, 512)
    W3r = f32(rev_params['w3']) * np.float32(S_REV3)   # (43008, 512)
    b3r = f32(rev_params['b3']) * np.float32(S_REV3)
    in_maps = []
    for c in range(N_CORES):
        m = dict(base)
        # lt1 output-shard: 64 outputs per core; block k = W1T[128k:+128, 64c:+64]
        lt1w = np.zeros((128, 136 * 64), NP16)
        for k in range(136):
            lt1w[:, k * 64:(k + 1) * 64] = W1T[128 * k:128 * (k + 1),
                                               64 * c:64 * (c + 1)]
        m['lt1w'] = lt1w
        m['ltb1c'] = col1(f32(lt_params['b1'])[64 * c:64 * (c + 1)])
        m['ltb1cr'] = f32(lt_params['b1'])[64 * c:64 * (c + 1)].reshape(1, -1).astype(NP16)
        W3c = W3r[5376 * c:5376 * (c + 1)]     # (5376, 512)
        W3cT = np.ascontiguousarray(W3c.T)     # (512, 5376)
        r3 = np.zeros((128, 4 * 42 * 128), NP16)
        off = 0
        pos = 0
        for wnt in [512] * 10 + [256]:
            for k in range(4):
                r3[:, pos:pos + wnt] = W3cT[128 * k:128 * (k + 1), off:off + wnt]
                pos += wnt
            off += wnt
        m['rev3w'] = r3
        m['rev3br'] = b3r[5376 * c:5376 * (c + 1)].reshape(1, -1)
        for di in range(2):
            d = dec_params[di]
            p = f'd{di}_'
            Hh, Wh = 8, (4 * WD0, 4 * WD1)[di]
            fw, fb = build_convout_fold(d['fc1_w'], d['fc1_b'], f32(d['w_out']),
                                        f32(d['b_out']), Hh, Wh)
            fw = fw * np.float32(S_FC)
            fb = fb * np.float32(S_FC)
            rows = fw[64 * c:64 * (c + 1)]     # (64, npx)
            nk = (NK_LT1, 25)[di]
            fwp = np.zeros((128, nk * 64), NP16)
            rT = rows.T                        # (npx, 64)
            for k in range(nk):
                fwp[:, k * 64:(k + 1) * 64] = rT[k * 128:(k + 1) * 128]
            m[p + 'fw1'] = fwp
            m[p + 'fb1'] = col1(fb[64 * c:64 * (c + 1)])
            m[p + 'fb1r'] = fb[64 * c:64 * (c + 1)].reshape(1, -1).astype(NP16)
        in_maps.append(m)
    return in_maps


_CACHE = {}


def kernel(x, enc0_params, lt_params, rev_params, dec_params):
    if 'nc' not in _CACHE:
        _CACHE['nc'] = build_program()
    nc = _CACHE['nc']
    in_maps = prep_inputs(x, enc0_params, lt_params, rev_params, dec_params)
    res = run_bass_kernel_spmd(nc, in_maps, list(range(N_CORES)))
    r0 = res.results[0]
    d0 = np.asarray(r0['d0'], np.float32)
    d1 = np.asarray(r0['d1'], np.float32)
    m0 = np.asarray(r0['m0'], np.float32)
    m1 = np.asarray(r0['m1'], np.float32)
    return d0, d1, m0, m1


# revision 36
# speedup vs baseline: 1.1706x; 1.0164x over previous
"""Trainium2 Bass kernel for nn_AutoEncoder_31533649887292.

8-core SPMD plan (uniform program, per-core data):
  - encoder replicated on all cores (serial conv chain, tap-accumulated matmuls)
  - lt1 (43008->512, but cols 17408..43008 multiply zeros -> dropped):
    K-sharded 8-way, partials AllReduce'd (512 floats)
  - lt2/lt3/rev1/rev2 replicated (output-on-partition matvec layout)
  - rev3 (512->43008) output-sharded 8-way + AllGather (fp16)
  - decoders run sequentially, replicated; per-decoder fc1 output-sharded
    8-way with ONE fused AllGather for both decoders
  - conv_out (1->1 conv) folded into fc1 weights host-side
  - numerics: fp16 matmul operands, fp32 PSUM/stats; compensated scales
    S_REV3=64 (undone inside dec rb2 weights) and S_FC=256 (undone at output)
"""
import numpy as np
import ml_dtypes

import concourse.bacc as bacc
import concourse.mybir as mybir
import concourse.tile as tile
from concourse.bass_utils import run_bass_kernel_spmd

F16 = mybir.dt.float16
F32 = mybir.dt.float32
NP16 = np.float16

N_CORES = 8
EPS = 1e-5
ALPHA = 0.01
S_REV3 = 64.0
S_FC = 256.0

H = 32
W0, W1 = 65, 98           # output widths
WD0, WD1 = 68, 100        # decoder entry widths (H=2)
FLAT0 = 17408             # e0 flatten / s0 size
NK_LT1 = 17               # 2176/128 k-chunks per core


# ----------------------------------------------------------------------------
# host-side weight packing helpers
# ----------------------------------------------------------------------------

def pack_conv(w):
    """w (Cout, Cin, 3, 3) -> lhsT pack (Cin, 9*Cout), tap t=3dy+dx."""
    Cout, Cin = w.shape[0], w.shape[1]
    out = np.zeros((Cin, 9 * Cout), NP16)
    for dy in range(3):
        for dx in range(3):
            t = 3 * dy + dx
            out[:, t * Cout:(t + 1) * Cout] = w[:, :, dy, dx].T
    return out


def pack_convt(w):
    """w (Cin, Cout, 3, 3) -> (Cin, 9*Cout), tap t=3ky+kx, already lhsT."""
    Cin, Cout = w.shape[0], w.shape[1]
    out = np.zeros((Cin, 9 * Cout), NP16)
    for ky in range(3):
        for kx in range(3):
            t = 3 * ky + kx
            out[:, t * Cout:(t + 1) * Cout] = w[:, :, ky, kx]
    return out


def pack_matvec(wT, nk, nm):
    """wT (K, N) (K=128*nk, N=128*nm) -> (128, nk*nm*128) block pack:
    block (k, m) at cols (k*nm+m)*128."""
    K, N = wT.shape
    out = np.zeros((128, nk * nm * 128), NP16)
    for k in range(nk):
        for m in range(nm):
            blk = wT[k * 128:(k + 1) * 128, m * 128:(m + 1) * 128]
            out[:blk.shape[0], (k * nm + m) * 128:(k * nm + m) * 128 + blk.shape[1]] = blk
    return out


def col1(v, dtype=np.float32):
    return np.ascontiguousarray(np.asarray(v, dtype).reshape(-1, 1))


def build_convout_fold(fc1_w, fc1_b, w_out, b_out, Hh, Wh):
    n = Hh * Wh
    C = np.zeros((n, n), np.float32)
    w = np.asarray(w_out)[0, 0]
    idx = np.arange(n).reshape(Hh, Wh)
    ys, xs = np.meshgrid(np.arange(Hh), np.arange(Wh), indexing='ij')
    for dy in range(3):
        for dx in range(3):
            yi, xi = ys + dy - 1, xs + dx - 1
            valid = (yi >= 0) & (yi < Hh) & (xi >= 0) & (xi < Wh)
            C[idx[ys[valid], xs[valid]], idx[yi[valid], xi[valid]]] += w[dy, dx]
    fc1_w = np.asarray(fc1_w, np.float32)
    new_w = fc1_w @ C
    new_b = np.asarray(fc1_b, np.float32) + fc1_w @ (np.float32(b_out[0]) * np.ones(n, np.float32))
    return new_w, new_b


# Small per-core-identical inputs consolidated into two packed tensors
# (one DMA each). Layout shared by builder and host via these specs.
PACK16 = [
    ('ew0', 9, 32), ('ew11', 32, 576), ('ew12', 64, 576), ('ewd1', 32, 64),
    ('ew21', 64, 1152), ('ew22', 128, 1152), ('ewd2', 64, 128),
    ('ew31', 128, 1152), ('ew32', 128, 1152),
    ('ident', 128, 128), ('ones', 1, 512),
    ('eb0r', 1, 32), ('eb11r', 1, 64), ('eb12r', 1, 64), ('eb21r', 1, 128),
    ('eb22r', 1, 128), ('eb31r', 1, 128), ('eb32r', 1, 128),
    ('mb2r', 1, 256), ('mb3r', 1, 128), ('mb4r', 1, 256), ('mb5r', 1, 512),
    ('mw2', 128, 1024), ('mw3', 128, 256), ('mw4', 128, 256), ('mw5', 128, 1024),
] + [(f'd{i}_' + n, p, w) for i in range(2) for n, p, w in [
    ('w_in', 128, 576), ('rb1w1', 64, 576), ('rb1w2', 64, 576),
    ('ct1w', 64, 576), ('rb2w1', 64, 288), ('rb2w2', 32, 288),
    ('rb2ds', 64, 32), ('ct2w', 32, 288), ('rb3w1', 128, 3), ('rb3ds', 32, 1),
    ('fw2', 128, 1024),
    ('b_inr', 1, 64), ('rb1b1r', 1, 64), ('rb1b2r', 1, 64), ('ct1br', 1, 64),
    ('rb2b1r', 1, 32), ('rb2b2r', 1, 32), ('ct2br', 1, 32), ('rb3b1r', 1, 1),
    ('fb2r', 1, 256)]]
PACK32 = [
    ('ones32f', 1, 32), ('border', 1, 32),
    ('ebd1', 64, 1), ('eg1', 64, 1), ('ebn1', 64, 1),
    ('ebd2', 128, 1), ('eg2', 128, 1), ('ebn2', 128, 1),
] + [(f'd{i}_' + n, p, w) for i in range(2) for n, p, w in [
    ('rb2dsb', 32, 1), ('rb2g', 32, 1), ('rb2bb', 32, 1),
    ('rb3dsb', 1, 1), ('rb3g', 1, 1), ('rb3bb', 1, 1),
    ('rb3w2', 8, 9), ('rb3b2p8', 8, 1),
    ('fb3r', 1, (2176, 3200)[i])]]


def _pack_layout(spec):
    offs = {}
    off = 0
    for name, pp, ww in spec:
        offs[name] = (off, pp, ww)
        off += ww
    return offs, off


OFF16, TOT16 = _pack_layout(PACK16)
OFF32, TOT32 = _pack_layout(PACK32)


# ----------------------------------------------------------------------------
# device program
# ----------------------------------------------------------------------------

class Ctx:
    pass


def emit_conv(g, name, src, dst, Cin, Cout, Hin, Win, stride, w_ap, b_ap,
              act, rows_per_tile=None, extra_ident_rhs=None):
    """Tap-accumulated 3x3 conv.
    src: padded fp16 tile (Cin, Hin+2, Win+2); dst padded fp16 tile or None.
    b_ap: f16 ROW bias (1, Cout), folded into psum via ones-matmul.
    act: 'lrelu' | 'none'. extra_ident_rhs: AP (Cout, Hout, Wout) added via
    identity matmul (residual). Returns list of (psum_ap, y0, nrows) if dst
    is None (caller evicts)."""
    nc = g.nc
    Hout = (Hin + stride - 1) // stride
    Wout = (Win + stride - 1) // stride
    if rows_per_tile is None:
        rows_per_tile = max(1, 512 // Wout)
    tiles = []
    y0 = 0
    while y0 < Hout:
        nr = min(rows_per_tile, Hout - y0)
        ps = g.psum.tile([Cout, nr, Wout], F32, tag="mm")
        mi = 0
        for dy in range(3):
            for dx in range(3):
                t = 3 * dy + dx
                rhs = src[0:Cin,
                          dy + stride * y0: dy + stride * (y0 + nr - 1) + 1: stride,
                          dx: dx + stride * (Wout - 1) + 1: stride]
                nc.tensor.matmul(ps[:], w_ap[:, t * Cout:(t + 1) * Cout], rhs,
                                 start=(mi == 0), stop=False)
                mi += 1
        if extra_ident_rhs is not None:
            nc.tensor.matmul(ps[:], g.ident[0:Cout, 0:Cout],
                             extra_ident_rhs[0:Cout, y0:y0 + nr, 0:Wout],
                             start=False, stop=False)
        # bias broadcast into psum: lhsT = bias row (1, Cout), rhs = ones (1, N)
        nc.tensor.matmul(ps[:], b_ap, g.ones[0:1, 0:nr * Wout],
                         start=False, stop=True)
        if dst is not None:
            emit_act(g, dst[0:Cout, 1 + y0:1 + y0 + nr, 1:1 + Wout], ps,
                     Cout, nr * Wout, act)
        tiles.append((ps, y0, nr))
        y0 += nr
    return tiles


def emit_act(g, dst_ap, ps, C, n, act):
    """dst = lrelu(ps) (or copy). lrelu = max(0.01*ps, ps): ACT mul + DVE max."""
    nc = g.nc
    if act == 'lrelu':
        tmp = g.sbuf.tile([128, 512], F32, tag="evtmp")
        nc.vector.tensor_scalar_mul(tmp[0:C, 0:n], ps[:], ALPHA)
        nc.vector.tensor_max(dst_ap, tmp[0:C, 0:n], ps[:])
    else:
        nc.scalar.copy(dst_ap, ps[:])


def zero_border(g, buf, C, Hp, Wp):
    """zero only the 1-px border of a padded (C, Hp, Wp) buffer."""
    nc = g.nc
    nc.gpsimd.memset(buf[0:C, 0:1, :], 0.0)
    nc.gpsimd.memset(buf[0:C, Hp - 1:Hp, :], 0.0)
    nc.gpsimd.memset(buf[0:C, 1:Hp - 1, 0:1], 0.0)
    nc.gpsimd.memset(buf[0:C, 1:Hp - 1, Wp - 1:Wp], 0.0)


def emit_bn(g, ds_tiles, C, npx, b_ap, g_ap, bb_ap, dsf32, ds16_dst):
    """BN with batch stats. ds_tiles: psum tiles from ds conv (list of
    (ps, y0, nr) covering (C, H, W)); evict to dsf32 (C, npx-ish 3D or 2D)
    with accum sums; then stats + apply -> ds16_dst (fp16)."""
    nc = g.nc
    nt = len(ds_tiles)
    acc = g.sbuf.tile([C, nt], F32, tag="bn_acc")
    for i, (ps, y0, nr) in enumerate(ds_tiles):
        nc.scalar.activation(dsf32[0:C, y0:y0 + nr, :], ps[:],
                             mybir.ActivationFunctionType.Identity,
                             bias=b_ap, scale=1.0,
                             accum_out=acc[:, i:i + 1])
    ssum = g.sbuf.tile([C, 1], F32, tag="bn_s")
    if nt > 1:
        nc.vector.tensor_reduce(ssum[:], acc[:], mybir.AxisListType.X,
                                mybir.AluOpType.add)
    else:
        nc.vector.tensor_copy(ssum[:], acc[:])
    sq = g.sbuf.tile([C, 1], F32, tag="bn_sq")
    scr = g.scratch  # (128, 2080) f32 scratch
    nc.scalar.activation(scr[0:C, 0:npx], dsf32[0:C].opt(),
                         mybir.ActivationFunctionType.Square,
                         accum_out=sq[:])
    inv_n = 1.0 / npx
    mean = g.sbuf.tile([C, 1], F32, tag="bn_m")
    nc.scalar.mul(mean[:], ssum[:], inv_n)
    ex2 = g.sbuf.tile([C, 1], F32, tag="bn_e")
    nc.scalar.mul(ex2[:], sq[:], inv_n)
    m2 = g.sbuf.tile([C, 1], F32, tag="bn_m2")
    nc.vector.tensor_mul(m2[:], mean[:], mean[:])
    var = g.sbuf.tile([C, 1], F32, tag="bn_v")
    nc.vector.tensor_sub(var[:], ex2[:], m2[:])
    nc.vector.tensor_scalar_add(var[:], var[:], EPS)
    std = g.sbuf.tile([C, 1], F32, tag="bn_std")
    nc.scalar.activation(std[:], var[:], mybir.ActivationFunctionType.Sqrt,
                         bias=0.0, scale=1.0)
    istd = g.sbuf.tile([C, 1], F32, tag="bn_istd")
    nc.vector.reciprocal(istd[:], std[:])
    s = g.sbuf.tile([C, 1], F32, tag="bn_sc")
    nc.vector.tensor_mul(s[:], g_ap, istd[:])
    ms = g.sbuf.tile([C, 1], F32, tag="bn_ms")
    nc.vector.tensor_mul(ms[:], mean[:], s[:])
    t = g.sbuf.tile([C, 1], F32, tag="bn_t")
    nc.vector.tensor_sub(t[:], bb_ap, ms[:])
    nc.vector.tensor_scalar(ds16_dst[:], dsf32[0:C].opt(), s[:], t[:],
                            mybir.AluOpType.mult, mybir.AluOpType.add)


def emit_matvec_op(g, w_ap, nk, nm, rhs_cols, biasrow_ap, act, out16, psum_tag):
    """out-on-partitions matvec: w_ap (128, nk*nm*128) blocks; rhs_cols
    (128, nk) fp16; psum (128, nm); biasrow (1, 128*nm) f16 folded via
    ones-matmul; act lrelu or none; out16 (128, nm) fp16 (or f32)."""
    nc = g.nc
    ps = g.psum.tile([128, nm], F32, tag="mm")
    for m in range(nm):
        for k in range(nk):
            nc.tensor.matmul(ps[:, m:m + 1],
                             w_ap[:, (k * nm + m) * 128:(k * nm + m) * 128 + 128],
                             rhs_cols[:, k:k + 1],
                             start=(k == 0), stop=False)
        nc.tensor.matmul(ps[:, m:m + 1], biasrow_ap[0:1, m * 128:(m + 1) * 128],
                         g.ones[0:1, 0:1], start=False, stop=True)
    emit_act(g, out16[:], ps, 128, nm, act)


def build_program():
    nc = bacc.Bacc("TRN2", target_bir_lowering=False, debug=False,
                   num_devices=N_CORES)
    g = Ctx()
    g.nc = nc

    def inp(name, shape, dt):
        return nc.dram_tensor(name, list(shape), dt, kind="ExternalInput").ap()

    # --- declare I/O ---
    I = {}
    I['xpatch'] = inp('xpatch', (9, 2080), F16)
    I['pack16'] = inp('pack16', (128, TOT16), F16)
    I['pack32'] = inp('pack32', (128, TOT32), F32)
    I['lt1w'] = inp('lt1w', (128, 136 * 64), F16)
    I['ltb1cr'] = inp('ltb1cr', (1, 64), F16)
    I['rev3w'] = inp('rev3w', (128, 4 * 42 * 128), F16)
    I['rev3br'] = inp('rev3br', (1, 5376), F32)
    for i, (wd, nk, nt) in enumerate([(WD0, NK_LT1, 17), (WD1, 25, 25)]):
        p = f'd{i}_'
        I[p + 'fb1r'] = inp(p + 'fb1r', (1, 64), F16)
        I[p + 'fw1'] = inp(p + 'fw1', (128, nk * 64), F16)
        I[p + 'fw3'] = inp(p + 'fw3', (128, 2 * nt * 128), F16)

    O = {}
    O['d0'] = nc.dram_tensor('d0', [H, 64], F32, kind="ExternalOutput").ap()
    O['d1'] = nc.dram_tensor('d1', [H, 96], F32, kind="ExternalOutput").ap()
    O['m0'] = nc.dram_tensor('m0', [H, 1], F32, kind="ExternalOutput").ap()
    O['m1'] = nc.dram_tensor('m1', [H, 2], F32, kind="ExternalOutput").ap()

    # internal DRAM
    e0_dram = nc.dram_tensor('e0_dram', [FLAT0], F16)
    z1p_dram = nc.dram_tensor('z1p_dram', [64], F32)
    z1r_dram = nc.dram_tensor('z1r_dram', [512], F32, addr_space="Shared")
    rloc_dram = nc.dram_tensor('rloc_dram', [5376], F16)
    rall_dram = nc.dram_tensor('rall_dram', [43008], F16, addr_space="Shared")
    hh_dram = [nc.dram_tensor(f'hh{i}_dram', [128 * (NK_LT1, 25)[i]], F16)
               for i in range(2)]
    zf1_dram = nc.dram_tensor('zf1_dram', [128], F32)
    zfall_dram = nc.dram_tensor('zfall_dram', [1024], F32, addr_space="Shared")
    y_dram = [nc.dram_tensor(f'y{i}_dram', [128 * (17, 25)[i]], F32)
              for i in range(2)]

    rg = [list(range(N_CORES))]

    with tile.TileContext(nc) as tc:
        with (
            tc.tile_pool(name="sbuf", bufs=1) as sbuf,
            tc.tile_pool(name="wstream", bufs=2) as wstream,
            tc.tile_pool(name="psum", bufs=3, space="PSUM") as psum,
        ):
            g.sbuf, g.psum = sbuf, psum
            D = I
            I = {}
            for _n, _ap in D.items():
                if _n in ('rev3w', 'lt1w', 'xpatch', 'pack16', 'pack32'):
                    continue
                _t = sbuf.tile(list(_ap.shape), _ap.dtype, tag="in_" + _n)
                nc.sync.dma_start(_t[:], _ap)
                I[_n] = _t
            pk16 = sbuf.tile([128, TOT16], F16, tag="pack16")
            nc.sync.dma_start(pk16[:], D['pack16'])
            pk32 = sbuf.tile([128, TOT32], F32, tag="pack32")
            nc.sync.dma_start(pk32[:], D['pack32'])
            for _n, (_o, _p, _w) in OFF16.items():
                I[_n] = pk16[0:_p, _o:_o + _w]
            for _n, (_o, _p, _w) in OFF32.items():
                I[_n] = pk32[0:_p, _o:_o + _w]
            g.ident = I['ident']
            g.ones = I['ones']
            g.scratch = sbuf.tile([128, 800], F32, tag="scratch")

            # ================= ENCODER =================
            B0 = sbuf.tile([32, 34, 67], F16, tag="big1")
            nc.gpsimd.memset(B0[:], 0.0)
            # L0: K=9 im2col; row tiles of 7; patches streamed per tile
            y0 = 0
            while y0 < 32:
                nr = min(7, 32 - y0)
                xp = wstream.tile([9, 512], F16, tag="xp")
                nc.sync.dma_start(xp[0:9, 0:nr * 65],
                                  D['xpatch'][:, y0 * 65:(y0 + nr) * 65])
                ps = psum.tile([32, nr, 65], F32, tag="mm")
                nc.tensor.matmul(ps[:], I['ew0'], xp[0:9, 0:nr * 65],
                                 start=True, stop=False)
                nc.tensor.matmul(ps[:], I['eb0r'], g.ones[0:1, 0:nr * 65],
                                 start=False, stop=True)
                emit_act(g, B0[0:32, 1 + y0:1 + y0 + nr, 1:66], ps, 32, nr * 65,
                         'lrelu')
                y0 += nr
            # rb1 (32->64, s2): c1
            B1 = sbuf.tile([64, 18, 35], F16, tag="B1")
            nc.gpsimd.memset(B1[:], 0.0)
            emit_conv(g, 'e_rb1c1', B0, B1, 32, 64, 32, 65, 2, I['ew11'],
                      I['eb11r'], 'lrelu', rows_per_tile=8)
            # rb1 ds (1x1 s2) + bn
            ds_tiles = []
            for (ty, nr) in [(0, 8), (8, 8)]:
                ps = psum.tile([64, nr, 33], F32, tag="mm")
                rhs = B0[0:32, 1 + 2 * ty: 1 + 2 * ty + 2 * nr: 2, 1:67:2]
                nc.tensor.matmul(ps[:], I['ewd1'], rhs, start=True, stop=True)
                ds_tiles.append((ps, ty, nr))
            dsA_f32 = sbuf.tile([64, 16, 33], F32, tag="bigf32")
            dsA16 = sbuf.tile([64, 16, 33], F16, tag="dsA16")
            emit_bn(g, ds_tiles, 64, 528, I['ebd1'], I['eg1'], I['ebn1'],
                    dsA_f32, dsA16)
            # rb1 c2 + identity add
            B2 = sbuf.tile([64, 18, 35], F16, tag="B2")
            nc.gpsimd.memset(B2[:], 0.0)
            emit_conv(g, 'e_rb1c2', B1, B2, 64, 64, 16, 33, 1, I['ew12'],
                      I['eb12r'], 'lrelu', rows_per_tile=8,
                      extra_ident_rhs=dsA16)
            # rb2 (64->128, s2)
            B3 = sbuf.tile([128, 10, 19], F16, tag="B3")
            nc.gpsimd.memset(B3[:], 0.0)
            emit_conv(g, 'e_rb2c1', B2, B3, 64, 128, 16, 33, 2, I['ew21'],
                      I['eb21r'], 'lrelu')
            ps = psum.tile([128, 8, 17], F32, tag="mm")
            nc.tensor.matmul(ps[:], I['ewd2'], B2[0:64, 1:17:2, 1:35:2],
                             start=True, stop=True)
            dsB_f32 = sbuf.tile([128, 8, 17], F32, tag="dsB_f32")
            dsB16 = sbuf.tile([128, 8, 17], F16, tag="dsB16")
            emit_bn(g, [(ps, 0, 8)], 128, 136, I['ebd2'], I['eg2'], I['ebn2'],
                    dsB_f32, dsB16)
            B4 = sbuf.tile([128, 10, 19], F16, tag="B4")
            nc.gpsimd.memset(B4[:], 0.0)
            emit_conv(g, 'e_rb2c2', B3, B4, 128, 128, 8, 17, 1, I['ew22'],
                      I['eb22r'], 'lrelu', extra_ident_rhs=dsB16)
            # rb3 (128->128, s1, no ds)
            B5 = sbuf.tile([128, 10, 19], F16, tag="B5")
            nc.gpsimd.memset(B5[:], 0.0)
            emit_conv(g, 'e_rb3c1', B4, B5, 128, 128, 8, 17, 1, I['ew31'],
                      I['eb31r'], 'lrelu')
            B6 = sbuf.tile([128, 10, 19], F16, tag="B6")
            nc.gpsimd.memset(B6[:], 0.0)
            emit_conv(g, 'e_rb3c2', B5, B6, 128, 128, 8, 17, 1, I['ew32'],
                      I['eb32r'], 'lrelu', extra_ident_rhs=B4[0:128, 1:9, 1:18])

            # e0 export + reload as k-chunk columns (full 136 chunks)
            nc.sync.dma_start(e0_dram.ap(), B6[0:128, 1:9, 1:18])
            e0c = sbuf.tile([128, 136], F16, tag="e0c")
            e0r = e0_dram.ap().rearrange("(a b) -> b a", b=128)
            nc.sync.dma_start(e0c[:], e0r)

            # ====== LT1 output-sharded (64 outputs per core) + AllGather ======
            psz = psum.tile([64, 1], F32, tag="mm")
            for kb in range(4):
                lt1b = wstream.tile([128, 34 * 64], F16, tag="lt1b")
                nc.sync.dma_start(lt1b[:], D['lt1w'][:, kb * 2176:(kb + 1) * 2176])
                for kk in range(34):
                    k = 34 * kb + kk
                    nc.tensor.matmul(psz[:], lt1b[:, kk * 64:(kk + 1) * 64],
                                     e0c[:, k:k + 1],
                                     start=(k == 0), stop=False)
            nc.tensor.matmul(psz[:], I['ltb1cr'], g.ones[0:1, 0:1],
                             start=False, stop=True)
            z1p = sbuf.tile([64, 1], F32, tag="z1p")
            emit_act(g, z1p[:], psz, 64, 1, 'lrelu')
            nc.sync.dma_start(z1p_dram.ap(), z1p[:])
            nc.gpsimd.collective_compute(
                "AllGather", mybir.AluOpType.bypass, replica_groups=rg,
                ins=[z1p_dram.ap()], outs=[z1r_dram.ap()])
            z1g = sbuf.tile([128, 4], F32, tag="z1g")
            nc.sync.dma_start(z1g[:], z1r_dram.ap().rearrange("(a b) -> b a", b=128))
            z16 = sbuf.tile([128, 4], F16, tag="z16")
            nc.vector.tensor_copy(z16[:], z1g[:])

            # ================= mids =================
            mids = [('mw2', 'mb2', 4, 2), ('mw3', 'mb3', 2, 1),
                    ('mw4', 'mb4', 1, 2), ('mw5', 'mb5', 2, 4)]
            zcur = z16
            for wn, bn, nk, nm in mids:
                wt = I[wn]
                znext = sbuf.tile([128, nm], F16, tag=wn + "_z")
                emit_matvec_op(g, wt, nk, nm, zcur, I[bn + 'r'], 'lrelu', znext, "mid")
                zcur = znext

            # ================= rev3 + AllGather =================
            # rhs-streaming, nt-major blocks; per-tile DMA out to dram
            NT_R3 = [512] * 10 + [256]
            off = 0
            for wnt in NT_R3:
                wck = wstream.tile([128, 4 * 512], F16, tag="rev3wc")
                nc.sync.dma_start(wck[0:128, 0:4 * wnt],
                                  D['rev3w'][:, 4 * off:4 * off + 4 * wnt])
                ps = psum.tile([1, wnt], F32, tag="mm")
                for k in range(4):
                    nc.tensor.matmul(ps[:], zcur[:, k:k + 1],
                                     wck[0:128, k * wnt:(k + 1) * wnt],
                                     start=(k == 0), stop=(k == 3))
                rsb = wstream.tile([1, 512], F16, tag="rsb")
                nc.vector.scalar_tensor_tensor(
                    rsb[0:1, 0:wnt], ps[:], 1.0,
                    I['rev3br'][0:1, off:off + wnt],
                    mybir.AluOpType.mult, mybir.AluOpType.add)
                nc.sync.dma_start(rloc_dram.ap()[off:off + wnt], rsb[0:1, 0:wnt])
                off += wnt
            nc.gpsimd.collective_compute(
                "AllGather", mybir.AluOpType.bypass, replica_groups=rg,
                ins=[rloc_dram.ap()], outs=[rall_dram.ap()])

            # ================= decoders: conv chains + fc1 =================
            zf1both = sbuf.tile([128, 1], F32, tag="zf1both")
            WDM = WD1
            sIn = sbuf.tile([128, 4, WDM + 2], F16, tag="d_sIn")
            A1 = sbuf.tile([64, 4, WDM + 2], F16, tag="d_A1")
            A2 = sbuf.tile([64, 4, WDM + 2], F16, tag="d_A2")
            A3 = sbuf.tile([64, 4, WDM + 2], F16, tag="d_A3")
            B1d = sbuf.tile([64, 6, 2 * WDM + 2], F16, tag="d_B1d")
            C1 = sbuf.tile([32, 6, 2 * WDM + 2], F16, tag="d_C1")
            C2 = sbuf.tile([32, 6, 2 * WDM + 2], F16, tag="d_C2")
            D1 = sbuf.tile([32, 10, 4 * WDM + 2], F16, tag="big1")
            z1sh = sbuf.tile([8, 3, 4 * WDM + 2], F16, tag="d_z1sh")
            for _b in (sIn, A1, A2, A3, B1d, C1, C2, D1, z1sh):
                nc.gpsimd.memset(_b[:], 0.0)
            for di, wd in enumerate([WD0, WD1]):
                p = f'd{di}_'
                w4 = 4 * wd
                npx3 = 8 * w4 // 4  # = 2*w4? no: level3 pixels = 8 * (4*wd) / 4
                # level sizes: L1 (H=2, wd), L2 (H=4, 2wd), L3 (H=8, 4wd)
                w2 = 2 * wd
                # -- weights
                wts = {wn: I[p + wn] for wn in
                       ['w_in', 'rb1w1', 'rb1w2', 'ct1w', 'rb2w1', 'rb2w2',
                        'rb2ds', 'ct2w', 'rb3w1', 'rb3ds']}
                rb3w2 = I[p + 'rb3w2']

                off = 0 if di == 0 else FLAT0
                rsl = rall_dram.ap()[off:off + 128 * 2 * wd].rearrange(
                    "(c h w) -> c h w", c=128, h=2)
                nc.sync.dma_start(sIn[0:128, 1:3, 1:1 + wd], rsl)
                emit_conv(g, p + 'cin', sIn, A1, 128, 64, 2, wd, 1,
                          wts['w_in'], I[p + 'b_inr'], 'lrelu')
                emit_conv(g, p + 'rb1c1', A1, A2, 64, 64, 2, wd, 1,
                          wts['rb1w1'], I[p + 'rb1b1r'], 'lrelu')
                emit_conv(g, p + 'rb1c2', A2, A3, 64, 64, 2, wd, 1,
                          wts['rb1w2'], I[p + 'rb1b2r'], 'lrelu',
                          extra_ident_rhs=A1[0:64, 1:3, 1:1 + wd])
                # ct1: 64->64, L1 (2, wd) -> L2 (4, 2wd)
                TAPS = {0: [(1, 0)], 1: [(2, 0), (0, 1)]}
                for q in (0, 1):
                    for d in (0, 1):
                        taps = [(ky, kx, dy, dx) for (ky, dy) in TAPS[q]
                                for (kx, dx) in TAPS[d]]
                        ps = psum.tile([64, 2, wd], F32, tag="mm")
                        for mi, (ky, kx, dy, dx) in enumerate(taps):
                            t = 3 * ky + kx
                            rhs = A3[0:64, 1 + dy:3 + dy, 1 + dx:1 + dx + wd]
                            nc.tensor.matmul(ps[:], wts['ct1w'][:, t * 64:(t + 1) * 64],
                                             rhs, start=(mi == 0), stop=False)
                        nc.tensor.matmul(ps[:], I[p + 'ct1br'],
                                         g.ones[0:1, 0:2 * wd],
                                         start=False, stop=True)
                        emit_act(g, B1d[0:64, 1 + q:1 + q + 4:2, 1 + d:1 + d + w2:2],
                                 ps, 64, 2 * wd, 'lrelu')
                # rb2: 64->32 with ds+bn, at L2 (4, w2)
                rpt = 512 // w2
                emit_conv(g, p + 'rb2c1', B1d, C1, 64, 32, 4, w2, 1,
                          wts['rb2w1'], I[p + 'rb2b1r'], 'lrelu', rows_per_tile=rpt)
                ds_tiles = []
                y0 = 0
                while y0 < 4:
                    nr = min(rpt, 4 - y0)
                    ps = psum.tile([32, nr, w2], F32, tag="mm")
                    nc.tensor.matmul(ps[:], wts['rb2ds'],
                                     B1d[0:64, 1 + y0:1 + y0 + nr, 1:1 + w2],
                                     start=True, stop=True)
                    ds_tiles.append((ps, y0, nr))
                    y0 += nr
                dsC_f32 = sbuf.tile([32, 4, w2], F32, tag="bigf32")
                dsC16 = sbuf.tile([32, 4, w2], F16, tag="d_dsC16")
                emit_bn(g, ds_tiles, 32, 4 * w2, I[p + 'rb2dsb'], I[p + 'rb2g'],
                        I[p + 'rb2bb'], dsC_f32, dsC16)
                emit_conv(g, p + 'rb2c2', C1, C2, 32, 32, 4, w2, 1,
                          wts['rb2w2'], I[p + 'rb2b2r'], 'lrelu',
                          rows_per_tile=rpt, extra_ident_rhs=dsC16)
                # ct2: 32->32, L2 (4, w2) -> L3 (8, w4)
                for q in (0, 1):
                    for d in (0, 1):
                        taps = [(ky, kx, dy, dx) for (ky, dy) in TAPS[q]
                                for (kx, dx) in TAPS[d]]
                        y0 = 0
                        while y0 < 4:
                            nr = min(rpt, 4 - y0)
                            ps = psum.tile([32, nr, w2], F32, tag="mm")
                            for mi, (ky, kx, dy, dx) in enumerate(taps):
                                t = 3 * ky + kx
                                rhs = C2[0:32, 1 + y0 + dy:1 + y0 + dy + nr,
                                         1 + dx:1 + dx + w2]
                                nc.tensor.matmul(ps[:], wts['ct2w'][:, t * 32:(t + 1) * 32],
                                                 rhs, start=(mi == 0), stop=False)
                            nc.tensor.matmul(ps[:], I[p + 'ct2br'],
                                             g.ones[0:1, 0:nr * w2],
                                             start=False, stop=True)
                            emit_act(g, D1[0:32, 1 + 2 * y0 + q:1 + 2 * y0 + q + 2 * nr:2,
                                           1 + d:1 + d + w4:2],
                                     ps, 32, nr * w2, 'lrelu')
                            y0 += nr
                # ---- rb3 tail (32 -> 1) at L3 (8, w4) ----
                npx = 8 * w4
                npx2 = npx // 2
                z1f = sbuf.tile([1, npx], F16, tag="d_flat1")
                for hf in range(2):
                    P = sbuf.tile([128, 3, npx2], F16, tag="d_patches")
                    for dy in range(3):
                        for dx in range(3):
                            t = 3 * dy + dx
                            srcw = D1[0:32, dy + 4 * hf:dy + 4 * hf + 4,
                                      dx:dx + w4]
                            nc.sync.dma_start(
                                P[(32 * t) % 128:(32 * t) % 128 + 32,
                                  t // 4, 0:npx2], srcw)
                    n0 = 0
                    while n0 < npx2:
                        nn = min(512, npx2 - n0)
                        ps = psum.tile([1, nn], F32, tag="mm")
                        for j, kr in ((0, 128), (1, 128), (2, 32)):
                            nc.tensor.matmul(ps[:], wts['rb3w1'][0:kr, j:j + 1],
                                             P[0:kr, j, n0:n0 + nn],
                                             start=(j == 0), stop=False)
                        nc.tensor.matmul(ps[:], I[p + 'rb3b1r'],
                                         g.ones[0:1, 0:nn],
                                         start=False, stop=True)
                        emit_act(g, z1f[:, hf * npx2 + n0:hf * npx2 + n0 + nn],
                                 ps, 1, nn, 'lrelu')
                        n0 += nn
                for dy in range(3):
                    p0 = max(0, 1 - dy)
                    p1 = min(8, 9 - dy)
                    r0 = p0 + dy - 1
                    r1 = p1 + dy - 1
                    nc.sync.dma_start(
                        z1sh[p0:p1, dy, 1:1 + w4],
                        z1f[0:1, r0 * w4:r1 * w4].rearrange(
                            "a (h w) -> a h w", w=w4))
                # conv2 1->1 on H-partition layout (DVE); rows pre-shifted
                acc = sbuf.tile([8, w4], F32, tag="d_acc")
                nc.gpsimd.memset(acc[:], 0.0)
                for dy in range(3):
                    for dx in range(3):
                        t = 3 * dy + dx
                        nc.vector.scalar_tensor_tensor(
                            acc[:], z1sh[0:8, dy, dx:dx + w4],
                            rb3w2[:, t:t + 1], acc[:],
                            mybir.AluOpType.mult, mybir.AluOpType.add)
                # ds 32->1 + bn
                dsD = sbuf.tile([1, npx], F16, tag="d_flat2")
                dacc = sbuf.tile([1, 8], F32, tag=p + "dacc")
                for r in range(8):
                    ps = psum.tile([1, w4], F32, tag="mm")
                    nc.tensor.matmul(ps[:], wts['rb3ds'],
                                     D1[0:32, 1 + r, 1:1 + w4],
                                     start=True, stop=True)
                    nc.scalar.activation(dsD[:, r * w4:(r + 1) * w4], ps[:],
                                         mybir.ActivationFunctionType.Identity,
                                         bias=I[p + 'rb3dsb'], scale=1.0,
                                         accum_out=dacc[:, r:r + 1])
                dsum = sbuf.tile([1, 1], F32, tag=p + "dsum")
                nc.vector.tensor_reduce(dsum[:], dacc[:], mybir.AxisListType.X,
                                        mybir.AluOpType.add)
                dacc2 = sbuf.tile([1, 4], F32, tag=p + "dacc2")
                qn = npx // 4
                for qq in range(4):
                    nc.scalar.activation(g.scratch[0:1, 0:qn],
                                         dsD[0:1, qq * qn:(qq + 1) * qn],
                                         mybir.ActivationFunctionType.Square,
                                         accum_out=dacc2[:, qq:qq + 1])
                dsq = sbuf.tile([1, 1], F32, tag=p + "dsq")
                nc.vector.tensor_reduce(dsq[:], dacc2[:], mybir.AxisListType.X,
                                        mybir.AluOpType.add)
                inv_n = 1.0 / npx
                dmean = sbuf.tile([1, 1], F32, tag=p + "dmean")
                nc.scalar.mul(dmean[:], dsum[:], inv_n)
                dex2 = sbuf.tile([1, 1], F32, tag=p + "dex2")
                nc.scalar.mul(dex2[:], dsq[:], inv_n)
                dm2 = sbuf.tile([1, 1], F32, tag=p + "dm2")
                nc.vector.tensor_mul(dm2[:], dmean[:], dmean[:])
                dvar = sbuf.tile([1, 1], F32, tag=p + "dvar")
                nc.vector.tensor_sub(dvar[:], dex2[:], dm2[:])
                nc.vector.tensor_scalar_add(dvar[:], dvar[:], EPS)
                dstd = sbuf.tile([1, 1], F32, tag=p + "dstd")
                nc.scalar.activation(dstd[:], dvar[:],
                                     mybir.ActivationFunctionType.Sqrt,
                                     bias=0.0, scale=1.0)
                distd = sbuf.tile([1, 1], F32, tag=p + "distd")
                nc.vector.reciprocal(distd[:], dstd[:])
                dsc = sbuf.tile([1, 1], F32, tag=p + "dsc")
                nc.vector.tensor_mul(dsc[:], I[p + 'rb3g'], distd[:])
                dms = sbuf.tile([1, 1], F32, tag=p + "dms")
                nc.vector.tensor_mul(dms[:], dmean[:], dsc[:])
                dt_ = sbuf.tile([1, 1], F32, tag=p + "dt")
                nc.vector.tensor_sub(dt_[:], I[p + 'rb3bb'], dms[:])
                nc.vector.tensor_scalar(dsD[:], dsD[:], dsc[:], dt_[:],
                                        mybir.AluOpType.mult, mybir.AluOpType.add)
                dsimg = sbuf.tile([8, w4], F16, tag="d_dsimg")
                nc.gpsimd.dma_start(dsimg[:],
                                    dsD[:].rearrange("a (h w) -> a h w", h=8))
                hsum = sbuf.tile([8, w4], F32, tag="d_hsum")
                nc.vector.scalar_tensor_tensor(hsum[:], acc[:],
                                               I[p + 'rb3b2p8'][:],
                                               dsimg[:],
                                               mybir.AluOpType.add,
                                               mybir.AluOpType.add)
                hh16 = sbuf.tile([8, w4], F16, tag="d_hh16")
                htmp = sbuf.tile([8, w4], F32, tag="d_htmp")
                nc.scalar.mul(htmp[:], hsum[:], ALPHA)
                nc.vector.tensor_max(hh16[:], htmp[:], hsum[:])
                nc.sync.dma_start(
                    hh_dram[di].ap()[0:npx].rearrange("(h w) -> h w", h=8), hh16[:])
                nk = (NK_LT1, 25)[di]
                hT = sbuf.tile([128, nk], F16, tag=p + "hT")
                nc.sync.dma_start(hT[:],
                                  hh_dram[di].ap().rearrange("(a b) -> b a", b=128))
                # fc1 shard: 64 outputs
                fw1 = I[p + 'fw1']
                psf = psum.tile([64, 1], F32, tag="mm")
                for k in range(nk):
                    nc.tensor.matmul(psf[:], fw1[:, k * 64:(k + 1) * 64],
                                     hT[:, k:k + 1], start=(k == 0), stop=False)
                nc.tensor.matmul(psf[:], I[p + 'fb1r'], g.ones[0:1, 0:1],
                                 start=False, stop=True)
                emit_act(g, zf1both[64 * di:64 * di + 64, 0:1], psf, 64, 1,
                         'lrelu')

            # fused fc1 AllGather
            nc.sync.dma_start(zf1_dram.ap(), zf1both[:])
            nc.gpsimd.collective_compute(
                "AllGather", mybir.AluOpType.bypass, replica_groups=rg,
                ins=[zf1_dram.ap()], outs=[zfall_dram.ap()])

            # ================= decoders: fc2/fc3 + masking =================
            for di, (wimg, m) in enumerate([(W0, 1), (W1, 2)]):
                p = f'd{di}_'
                nt = (17, 25)[di]
                zfg = sbuf.tile([128, 4], F32, tag=p + "zfg")
                # zfall[128*c + 64*dec + j]; dec di's vector z[i], i = 64*c + j.
                # dst (p, k) holds z[128k + p]: c = 2k + p//64, j = p%64
                #   -> dram idx = 256k + 128*(p//64) + 64*di + p%64
                zview = zfall_dram.ap().rearrange("(k h j) -> h j k", h=4, j=64)
                # zview[h, j, k] = dram[256k + 64h + j]; need h = 2*(p//64) + di
                for half in range(2):
                    nc.sync.dma_start(
                        zfg[64 * half:64 * half + 64, 0:4],
                        zview[2 * half + di, :, :])
                zfg16 = sbuf.tile([128, 4], F16, tag=p + "zfg16")
                nc.vector.tensor_copy(zfg16[:], zfg[:])
                fw2 = I[p + 'fw2']
                zf2 = sbuf.tile([128, 2], F16, tag=p + "zf2")
                emit_matvec_op(g, fw2, 4, 2, zfg16, I[p + 'fb2r'], 'lrelu',
                               zf2, "mid")
                fw3 = I[p + 'fw3']
                npx3 = nt * 128
                NT3 = [512] * (npx3 // 512) + ([npx3 % 512] if npx3 % 512 else [])
                off = 0
                pos = 0
                for wnt in NT3:
                    ps = psum.tile([1, wnt], F32, tag="mm")
                    for k in range(2):
                        nc.tensor.matmul(ps[:], zf2[:, k:k + 1],
                                         fw3[0:128, pos + k * wnt:pos + (k + 1) * wnt],
                                         start=(k == 0), stop=(k == 1))
                    yfl = wstream.tile([1, 512], F32, tag="yfl")
                    nc.vector.scalar_tensor_tensor(
                        yfl[0:1, 0:wnt], ps[:], 1.0,
                        I[p + 'fb3r'][0:1, off:off + wnt],
                        mybir.AluOpType.mult, mybir.AluOpType.add)
                    nc.sync.dma_start(y_dram[di].ap()[off:off + wnt],
                                      yfl[0:1, 0:wnt])
                    pos += 2 * wnt
                    off += wnt
                ysb = sbuf.tile([H, wimg], F32, tag=p + "ysb")
                nc.sync.dma_start(ysb[:], y_dram[di].ap()[0:H * wimg]
                                  .rearrange("(h w) -> h w", h=H))
                # masking
                nz = sbuf.tile([H, m], F32, tag=p + "nz")
                nc.vector.tensor_scalar(nz[:], ysb[0:H, wimg - m:wimg], 0.0, None,
                                        mybir.AluOpType.is_gt)
                nc.sync.dma_start(O[f'm{di}'], nz[:])
                nzsq = sbuf.tile([H, 32], F32, tag=p + "nzsq")
                nc.gpsimd.memset(nzsq[:], 0.0)
                nc.vector.tensor_copy(nzsq[0:H, 0:m], nz[:])
                nzT = sbuf.tile([H, 32], F32, tag=p + "nzT")
                nc.vector.transpose(nzT[:], nzsq[:])
                AT = sbuf.tile([m + 1, 32], F32, tag=p + "AT")
                nc.sync.dma_start(AT[0:1, :], I['border'][0:1, :])
                nc.sync.dma_start(AT[1:1 + m, :], nzT[0:m, :])
                E = sbuf.tile([m + 1, 32 * (m + 1)], F32, tag=p + "E")
                nc.gpsimd.memset(E[:], 0.0)
                for j in range(m):
                    nc.sync.dma_start(E[j:j + 1, 32 * j:32 * (j + 1)],
                                      nzT[j:j + 1, 0:32])
                nc.sync.dma_start(E[m:m + 1, 32 * m:32 * (m + 1)],
                                  I['ones32f'][0:1, :])
                psm = psum.tile([H, 32 * (m + 1)], F32, tag="mm")
                nc.tensor.matmul(psm[:], AT[:], E[:], start=True, stop=True)
                dout = sbuf.tile([H, 32 * (m + 1)], F32, tag=p + "dout")
                nc.vector.scalar_tensor_tensor(dout[:], ysb[0:H, 0:32 * (m + 1)],
                                               1.0 / S_FC, psm[:],
                                               mybir.AluOpType.mult,
                                               mybir.AluOpType.mult)
                nc.sync.dma_start(O[f'd{di}'], dout[:])

    nc.compile()
    return nc


# ----------------------------------------------------------------------------
# host-side input prep
# ----------------------------------------------------------------------------

def prep_inputs(x, enc0_params, lt_params, rev_params, dec_params):
    """Returns list of 8 per-core input dicts."""
    f32 = lambda a: np.asarray(a, np.float32)
    f16 = lambda a: np.asarray(a, np.float32).astype(NP16)

    base = {}
    # L0 im2col patches from x (pure gather + zero pad)
    xi = f32(x)[0, 0]  # (32, 65)
    xpad = np.zeros((34, 67), np.float32)
    xpad[1:33, 1:66] = xi
    patches = np.zeros((9, 2080), np.float32)
    for dy in range(3):
        for dx in range(3):
            patches[3 * dy + dx] = xpad[dy:dy + 32, dx:dx + 65].reshape(-1)
    base['xpatch'] = f16(patches)

    e = enc0_params
    base['ew0'] = f16(f32(e['w0'])[:, 0].reshape(32, 9).T)
    base['eb0'] = col1(e['b0'])
    base['ew11'] = pack_conv(f32(e['rb1']['w1']))
    base['eb11'] = col1(e['rb1']['b1'])
    base['ew12'] = pack_conv(f32(e['rb1']['w2']))
    base['eb12'] = col1(e['rb1']['b2'])
    base['ewd1'] = f16(f32(e['rb1']['ds_w'])[:, :, 0, 0].T)
    base['ebd1'] = col1(e['rb1']['ds_b'])
    base['eg1'] = col1(e['rb1']['bn_g'])
    base['ebn1'] = col1(e['rb1']['bn_b'])
    base['ew21'] = pack_conv(f32(e['rb2']['w1']))
    base['eb21'] = col1(e['rb2']['b1'])
    base['ew22'] = pack_conv(f32(e['rb2']['w2']))
    base['eb22'] = col1(e['rb2']['b2'])
    base['ewd2'] = f16(f32(e['rb2']['ds_w'])[:, :, 0, 0].T)
    base['ebd2'] = col1(e['rb2']['ds_b'])
    base['eg2'] = col1(e['rb2']['bn_g'])
    base['ebn2'] = col1(e['rb2']['bn_b'])
    base['ew31'] = pack_conv(f32(e['rb3']['w1']))
    base['eb31'] = col1(e['rb3']['b1'])
    base['ew32'] = pack_conv(f32(e['rb3']['w2']))
    base['eb32'] = col1(e['rb3']['b2'])
    base['ident'] = np.eye(128, dtype=NP16)
    base['ones'] = np.ones((1, 512), NP16)
    base['ones32f'] = np.ones((1, 32), np.float32)
    row16 = lambda a, s=1.0: (np.asarray(a, np.float32) * np.float32(s)).reshape(1, -1).astype(NP16)
    base['eb0r'] = row16(e['b0'])
    base['eb11r'] = row16(e['rb1']['b1'])
    base['eb12r'] = row16(e['rb1']['b2'])
    base['eb21r'] = row16(e['rb2']['b1'])
    base['eb22r'] = row16(e['rb2']['b2'])
    base['eb31r'] = row16(e['rb3']['b1'])
    base['eb32r'] = row16(e['rb3']['b2'])
    base['mb2r'] = row16(lt_params['b2'])
    base['mb3r'] = row16(lt_params['b3'])
    base['mb4r'] = row16(rev_params['b1'])
    base['mb5r'] = row16(rev_params['b2'])

    for i, (wn, bn, nk, nm) in enumerate([('mw2', 'mb2', 4, 2), ('mw3', 'mb3', 2, 1),
                                          ('mw4', 'mb4', 1, 2), ('mw5', 'mb5', 2, 4)]):
        src = [lt_params, lt_params, rev_params, rev_params][i]
        key = ['w2', 'w3', 'w1', 'w2'][i]
        w = f32(src[key])          # (out, in)
        b = f32(src[key.replace('w', 'b')])
        base[wn] = pack_matvec(w.T, nk, nm)
        base[bn] = b.reshape(nm, 128).T.copy()

    border = np.ones((1, 32), np.float32)
    border[0, [0, 1, 30, 31]] = 0.0
    base['border'] = border

    # decoder shared (replicated) weights
    for di in range(2):
        d = dec_params[di]
        p = f'd{di}_'
        S = np.float32(S_REV3)
        base[p + 'w_in'] = pack_conv(f32(d['w_in']))
        base[p + 'b_in'] = col1(f32(d['b_in']) * S)
        base[p + 'rb1w1'] = pack_conv(f32(d['rb1']['w1']))
        base[p + 'rb1b1'] = col1(f32(d['rb1']['b1']) * S)
        base[p + 'rb1w2'] = pack_conv(f32(d['rb1']['w2']))
        base[p + 'rb1b2'] = col1(f32(d['rb1']['b2']) * S)
        base[p + 'ct1w'] = pack_convt(f32(d['ct1_w']))
        base[p + 'ct1b'] = col1(f32(d['ct1_b']) * S)
        base[p + 'rb2w1'] = pack_conv(f32(d['rb2']['w1']) / S)
        base[p + 'rb2b1'] = col1(d['rb2']['b1'])
        base[p + 'rb2w2'] = pack_conv(f32(d['rb2']['w2']))
        base[p + 'rb2b2'] = col1(d['rb2']['b2'])
        base[p + 'rb2ds'] = f16(f32(d['rb2']['ds_w'])[:, :, 0, 0].T / S)
        base[p + 'rb2dsb'] = col1(d['rb2']['ds_b'])
        base[p + 'rb2g'] = col1(d['rb2']['bn_g'])
        base[p + 'rb2bb'] = col1(d['rb2']['bn_b'])
        base[p + 'ct2w'] = pack_convt(f32(d['ct2_w']))
        base[p + 'ct2b'] = col1(d['ct2_b'])
        base[p + 'b_inr'] = row16(d['b_in'], S)
        base[p + 'rb1b1r'] = row16(d['rb1']['b1'], S)
        base[p + 'rb1b2r'] = row16(d['rb1']['b2'], S)
        base[p + 'ct1br'] = row16(d['ct1_b'], S)
        base[p + 'rb2b1r'] = row16(d['rb2']['b1'])
        base[p + 'rb2b2r'] = row16(d['rb2']['b2'])
        base[p + 'ct2br'] = row16(d['ct2_b'])
        base[p + 'rb3b1r'] = row16(d['rb3']['b1'])
        base[p + 'fb2r'] = row16(d['fc2_b'], S_FC)
        # rb3: conv1 32->1: flat k = cin + 32*t -> chunks (128, 3)
        w1 = f32(d['rb3']['w1'])  # (1, 32, 3, 3)
        flat = np.zeros(384, np.float32)
        for dy in range(3):
            for dx in range(3):
                t = 3 * dy + dx
                flat[32 * t:32 * t + 32] = w1[0, :, dy, dx]
        base[p + 'rb3w1'] = f16(flat.reshape(3, 128).T)
        base[p + 'rb3b1'] = col1(d['rb3']['b1'])
        w2 = f32(d['rb3']['w2'])[0, 0]  # (3,3)
        base[p + 'rb3w2'] = np.tile(w2.reshape(1, 9), (8, 1)).astype(np.float32)
        base[p + 'rb3b2'] = col1(d['rb3']['b2'])
        base[p + 'rb3b2p8'] = np.full((8, 1), np.float32(np.asarray(d['rb3']['b2']).ravel()[0]), np.float32)
        base[p + 'rb3ds'] = f16(f32(d['rb3']['ds_w'])[:, :, 0, 0].T)
        base[p + 'rb3dsb'] = col1(d['rb3']['ds_b'])
        base[p + 'rb3g'] = col1(d['rb3']['bn_g'])
        base[p + 'rb3bb'] = col1(d['rb3']['bn_b'])
        # fc2 / fc3 (replicated)
        w2f = f32(d['fc2_w'])
        base[p + 'fw2'] = pack_matvec(w2f.T, 4, 2)
        base[p + 'fb2'] = (f32(d['fc2_b']) * S_FC).reshape(2, 128).T.copy()
        nt = (17, 25)[di]
        w3 = f32(d['fc3_w'])      # (2080/3136, 256)
        w3p = np.zeros((nt * 128, 256), np.float32)
        w3p[:w3.shape[0]] = w3
        w3pT = np.ascontiguousarray(w3p.T)    # (256, nt*128)
        npx3 = nt * 128
        fw3 = np.zeros((128, 2 * npx3), NP16)
        pos = 0
        off = 0
        for wnt in [512] * (npx3 // 512) + ([npx3 % 512] if npx3 % 512 else []):
            for k in range(2):
                fw3[:, pos:pos + wnt] = w3pT[128 * k:128 * (k + 1), off:off + wnt]
                pos += wnt
            off += wnt
        base[p + 'fw3'] = fw3
        b3p = np.zeros(nt * 128, np.float32)
        b3p[:w3.shape[0]] = f32(d['fc3_b']) * S_FC
        base[p + 'fb3r'] = b3p.reshape(1, -1)

    # assemble packed small-input tensors (same for all cores)
    pk16 = np.zeros((128, TOT16), NP16)
    for name, (off, pp, ww) in OFF16.items():
        a = np.asarray(base[name], NP16)
        assert a.shape == (pp, ww), (name, a.shape, (pp, ww))
        pk16[:pp, off:off + ww] = a
    base['pack16'] = pk16
    pk32 = np.zeros((128, TOT32), np.float32)
    for name, (off, pp, ww) in OFF32.items():
        a = np.asarray(base[name], np.float32)
        assert a.shape == (pp, ww), (name, a.shape, (pp, ww))
        pk32[:pp, off:off + ww] = a
    base['pack32'] = pk32

    # per-core shards
    W1eff = f32(lt_params['w1'])[:, :FLAT0]    # (512, 17408)
    W1T = W1eff.T                              # (17408, 512)
    W3r = f32(rev_params['w3']) * np.float32(S_REV3)   # (43008, 512)
    b3r = f32(rev_params['b3']) * np.float32(S_REV3)
    in_maps = []
    for c in range(N_CORES):
        m = dict(base)
        # lt1 output-shard: 64 outputs per core; block k = W1T[128k:+128, 64c:+64]
        lt1w = np.zeros((128, 136 * 64), NP16)
        for k in range(136):
            lt1w[:, k * 64:(k + 1) * 64] = W1T[128 * k:128 * (k + 1),
                                               64 * c:64 * (c + 1)]
        m['lt1w'] = lt1w
        m['ltb1c'] = col1(f32(lt_params['b1'])[64 * c:64 * (c + 1)])
        m['ltb1cr'] = f32(lt_params['b1'])[64 * c:64 * (c + 1)].reshape(1, -1).astype(NP16)
        W3c = W3r[5376 * c:5376 * (c + 1)]     # (5376, 512)
        W3cT = np.ascontiguousarray(W3c.T)     # (512, 5376)
        r3 = np.zeros((128, 4 * 42 * 128), NP16)
        off = 0
        pos = 0
        for wnt in [512] * 10 + [256]:
            for k in range(4):
                r3[:, pos:pos + wnt] = W3cT[128 * k:128 * (k + 1), off:off + wnt]
                pos += wnt
            off += wnt
        m['rev3w'] = r3
        m['rev3br'] = b3r[5376 * c:5376 * (c + 1)].reshape(1, -1)
        for di in range(2):
            d = dec_params[di]
            p = f'd{di}_'
            Hh, Wh = 8, (4 * WD0, 4 * WD1)[di]
            fw, fb = build_convout_fold(d['fc1_w'], d['fc1_b'], f32(d['w_out']),
                                        f32(d['b_out']), Hh, Wh)
            fw = fw * np.float32(S_FC)
            fb = fb * np.float32(S_FC)
            rows = fw[64 * c:64 * (c + 1)]     # (64, npx)
            nk = (NK_LT1, 25)[di]
            fwp = np.zeros((128, nk * 64), NP16)
            rT = rows.T                        # (npx, 64)
            for k in range(nk):
                fwp[:, k * 64:(k + 1) * 64] = rT[k * 128:(k + 1) * 128]
            m[p + 'fw1'] = fwp
            m[p + 'fb1'] = col1(fb[64 * c:64 * (c + 1)])
            m[p + 'fb1r'] = fb[64 * c:64 * (c + 1)].reshape(1, -1).astype(NP16)
        in_maps.append(m)
    return in_maps


_CACHE = {}


def kernel(x, enc0_params, lt_params, rev_params, dec_params):
    if 'nc' not in _CACHE:
        _CACHE['nc'] = build_program()
    nc = _CACHE['nc']
    in_maps = prep_inputs(x, enc0_params, lt_params, rev_params, dec_params)
    res = run_bass_kernel_spmd(nc, in_maps, list(range(N_CORES)))
    r0 = res.results[0]
    d0 = np.asarray(r0['d0'], np.float32)
    d1 = np.asarray(r0['d1'], np.float32)
    m0 = np.asarray(r0['m0'], np.float32)
    m1 = np.asarray(r0['m1'], np.float32)
    return d0, d1, m0, m1


# revision 37
# speedup vs baseline: 1.2033x; 1.0279x over previous
"""Trainium2 Bass kernel for nn_AutoEncoder_31533649887292.

8-core SPMD plan (uniform program, per-core data):
  - encoder replicated on all cores (serial conv chain, tap-accumulated matmuls)
  - lt1 (43008->512, but cols 17408..43008 multiply zeros -> dropped):
    K-sharded 8-way, partials AllReduce'd (512 floats)
  - lt2/lt3/rev1/rev2 replicated (output-on-partition matvec layout)
  - rev3 (512->43008) output-sharded 8-way + AllGather (fp16)
  - decoders run sequentially, replicated; per-decoder fc1 output-sharded
    8-way with ONE fused AllGather for both decoders
  - conv_out (1->1 conv) folded into fc1 weights host-side
  - numerics: fp16 matmul operands, fp32 PSUM/stats; compensated scales
    S_REV3=64 (undone inside dec rb2 weights) and S_FC=256 (undone at output)
"""
import numpy as np
import ml_dtypes

import concourse.bacc as bacc
import concourse.mybir as mybir
import concourse.tile as tile
from concourse.bass_utils import run_bass_kernel_spmd

F16 = mybir.dt.float16
F32 = mybir.dt.float32
NP16 = np.float16

N_CORES = 8
EPS = 1e-5
ALPHA = 0.01
S_REV3 = 64.0
S_FC = 256.0

H = 32
W0, W1 = 65, 98           # output widths
WD0, WD1 = 68, 100        # decoder entry widths (H=2)
FLAT0 = 17408             # e0 flatten / s0 size
NK_LT1 = 17               # 2176/128 k-chunks per core


# ----------------------------------------------------------------------------
# host-side weight packing helpers
# ----------------------------------------------------------------------------

def pack_conv(w):
    """w (Cout, Cin, 3, 3) -> lhsT pack (Cin, 9*Cout), tap t=3dy+dx."""
    Cout, Cin = w.shape[0], w.shape[1]
    out = np.zeros((Cin, 9 * Cout), NP16)
    for dy in range(3):
        for dx in range(3):
            t = 3 * dy + dx
            out[:, t * Cout:(t + 1) * Cout] = w[:, :, dy, dx].T
    return out


def pack_convt(w):
    """w (Cin, Cout, 3, 3) -> (Cin, 9*Cout), tap t=3ky+kx, already lhsT."""
    Cin, Cout = w.shape[0], w.shape[1]
    out = np.zeros((Cin, 9 * Cout), NP16)
    for ky in range(3):
        for kx in range(3):
            t = 3 * ky + kx
            out[:, t * Cout:(t + 1) * Cout] = w[:, :, ky, kx]
    return out


def pack_matvec(wT, nk, nm):
    """wT (K, N) (K=128*nk, N=128*nm) -> (128, nk*nm*128) block pack:
    block (k, m) at cols (k*nm+m)*128."""
    K, N = wT.shape
    out = np.zeros((128, nk * nm * 128), NP16)
    for k in range(nk):
        for m in range(nm):
            blk = wT[k * 128:(k + 1) * 128, m * 128:(m + 1) * 128]
            out[:blk.shape[0], (k * nm + m) * 128:(k * nm + m) * 128 + blk.shape[1]] = blk
    return out


def col1(v, dtype=np.float32):
    return np.ascontiguousarray(np.asarray(v, dtype).reshape(-1, 1))


def build_convout_fold(fc1_w, fc1_b, w_out, b_out, Hh, Wh):
    n = Hh * Wh
    C = np.zeros((n, n), np.float32)
    w = np.asarray(w_out)[0, 0]
    idx = np.arange(n).reshape(Hh, Wh)
    ys, xs = np.meshgrid(np.arange(Hh), np.arange(Wh), indexing='ij')
    for dy in range(3):
        for dx in range(3):
            yi, xi = ys + dy - 1, xs + dx - 1
            valid = (yi >= 0) & (yi < Hh) & (xi >= 0) & (xi < Wh)
            C[idx[ys[valid], xs[valid]], idx[yi[valid], xi[valid]]] += w[dy, dx]
    fc1_w = np.asarray(fc1_w, np.float32)
    new_w = fc1_w @ C
    new_b = np.asarray(fc1_b, np.float32) + fc1_w @ (np.float32(b_out[0]) * np.ones(n, np.float32))
    return new_w, new_b


# Small per-core-identical inputs consolidated into two packed tensors
# (one DMA each). Layout shared by builder and host via these specs.
PACK16 = [
    ('ew0', 9, 32), ('ew11', 32, 576), ('ew12', 64, 576), ('ewd1', 32, 64),
    ('ew21', 64, 1152), ('ew22', 128, 1152), ('ewd2', 64, 128),
    ('ew31', 128, 1152), ('ew32', 128, 1152),
    ('ident', 128, 128), ('ones', 1, 512),
    ('eb0r', 1, 32), ('eb11r', 1, 64), ('eb12r', 1, 64), ('eb21r', 1, 128),
    ('eb22r', 1, 128), ('eb31r', 1, 128), ('eb32r', 1, 128),
    ('mb2r', 1, 256), ('mb3r', 1, 128), ('mb4r', 1, 256), ('mb5r', 1, 512),
    ('mw2', 128, 1024), ('mw3', 128, 256), ('mw4', 128, 256), ('mw5', 128, 1024),
] + [(f'd{i}_' + n, p, w) for i in range(2) for n, p, w in [
    ('w_in', 128, 576), ('rb1w1', 64, 576), ('rb1w2', 64, 576),
    ('ct1w', 64, 576), ('rb2w1', 64, 288), ('rb2w2', 32, 288),
    ('rb2ds', 64, 32), ('ct2w', 32, 288), ('rb3w1', 128, 3), ('rb3ds', 32, 1),
    ('fw2', 128, 1024),
    ('b_inr', 1, 64), ('rb1b1r', 1, 64), ('rb1b2r', 1, 64), ('ct1br', 1, 64),
    ('rb2b1r', 1, 32), ('rb2b2r', 1, 32), ('ct2br', 1, 32), ('rb3b1r', 1, 1),
    ('fb2r', 1, 256)]]
PACK32 = [
    ('ones32f', 1, 32), ('border', 1, 32),
    ('ebd1', 64, 1), ('eg1', 64, 1), ('ebn1', 64, 1),
    ('ebd2', 128, 1), ('eg2', 128, 1), ('ebn2', 128, 1),
] + [(f'd{i}_' + n, p, w) for i in range(2) for n, p, w in [
    ('rb2dsb', 32, 1), ('rb2g', 32, 1), ('rb2bb', 32, 1),
    ('rb3dsb', 1, 1), ('rb3g', 1, 1), ('rb3bb', 1, 1),
    ('rb3w2', 8, 9), ('rb3b2p8', 8, 1),
    ('fb3r', 1, (2176, 3200)[i])]]


def _pack_layout(spec):
    offs = {}
    off = 0
    for name, pp, ww in spec:
        offs[name] = (off, pp, ww)
        off += ww
    return offs, off


OFF16, TOT16 = _pack_layout(PACK16)
OFF32, TOT32 = _pack_layout(PACK32)


# ----------------------------------------------------------------------------
# device program
# ----------------------------------------------------------------------------

class Ctx:
    pass


def emit_conv(g, name, src, dst, Cin, Cout, Hin, Win, stride, w_ap, b_ap,
              act, rows_per_tile=None, extra_ident_rhs=None):
    """Tap-accumulated 3x3 conv.
    src: padded fp16 tile (Cin, Hin+2, Win+2); dst padded fp16 tile or None.
    b_ap: f16 ROW bias (1, Cout), folded into psum via ones-matmul.
    act: 'lrelu' | 'none'. extra_ident_rhs: AP (Cout, Hout, Wout) added via
    identity matmul (residual). Returns list of (psum_ap, y0, nrows) if dst
    is None (caller evicts)."""
    nc = g.nc
    Hout = (Hin + stride - 1) // stride
    Wout = (Win + stride - 1) // stride
    if rows_per_tile is None:
        rows_per_tile = max(1, 512 // Wout)
    tiles = []
    y0 = 0
    while y0 < Hout:
        nr = min(rows_per_tile, Hout - y0)
        ps = g.psum.tile([Cout, nr, Wout], F32, tag="mm")
        mi = 0
        for dy in range(3):
            for dx in range(3):
                t = 3 * dy + dx
                rhs = src[0:Cin,
                          dy + stride * y0: dy + stride * (y0 + nr - 1) + 1: stride,
                          dx: dx + stride * (Wout - 1) + 1: stride]
                nc.tensor.matmul(ps[:], w_ap[:, t * Cout:(t + 1) * Cout], rhs,
                                 start=(mi == 0), stop=False)
                mi += 1
        if extra_ident_rhs is not None:
            nc.tensor.matmul(ps[:], g.ident[0:Cout, 0:Cout],
                             extra_ident_rhs[0:Cout, y0:y0 + nr, 0:Wout],
                             start=False, stop=False)
        # bias broadcast into psum: lhsT = bias row (1, Cout), rhs = ones (1, N)
        nc.tensor.matmul(ps[:], b_ap, g.ones[0:1, 0:nr * Wout],
                         start=False, stop=True)
        if dst is not None:
            emit_act(g, dst[0:Cout, 1 + y0:1 + y0 + nr, 1:1 + Wout], ps,
                     Cout, nr * Wout, act)
        tiles.append((ps, y0, nr))
        y0 += nr
    return tiles


def emit_act(g, dst_ap, ps, C, n, act):
    """dst = lrelu(ps) (or copy). lrelu = max(0.01*ps, ps): ACT mul + DVE max."""
    nc = g.nc
    if act == 'lrelu':
        g.evct = getattr(g, 'evct', 0) + 1
        tmp = g.sbuf.tile([128, 512], F32, tag=f"evtmp{g.evct % 2}")
        nc.vector.tensor_scalar_mul(tmp[0:C, 0:n], ps[:], ALPHA)
        nc.vector.tensor_max(dst_ap, tmp[0:C, 0:n], ps[:])
    else:
        nc.scalar.copy(dst_ap, ps[:])


def zero_border(g, buf, C, Hp, Wp):
    """zero only the 1-px border of a padded (C, Hp, Wp) buffer."""
    nc = g.nc
    nc.gpsimd.memset(buf[0:C, 0:1, :], 0.0)
    nc.gpsimd.memset(buf[0:C, Hp - 1:Hp, :], 0.0)
    nc.gpsimd.memset(buf[0:C, 1:Hp - 1, 0:1], 0.0)
    nc.gpsimd.memset(buf[0:C, 1:Hp - 1, Wp - 1:Wp], 0.0)


def emit_bn(g, ds_tiles, C, npx, b_ap, g_ap, bb_ap, dsf32, ds16_dst):
    """BN with batch stats. ds_tiles: psum tiles from ds conv (list of
    (ps, y0, nr) covering (C, H, W)); evict to dsf32 (C, npx-ish 3D or 2D)
    with accum sums; then stats + apply -> ds16_dst (fp16)."""
    nc = g.nc
    nt = len(ds_tiles)
    acc = g.sbuf.tile([C, nt], F32, tag="bn_acc")
    for i, (ps, y0, nr) in enumerate(ds_tiles):
        nc.scalar.activation(dsf32[0:C, y0:y0 + nr, :], ps[:],
                             mybir.ActivationFunctionType.Identity,
                             bias=b_ap, scale=1.0,
                             accum_out=acc[:, i:i + 1])
    ssum = g.sbuf.tile([C, 1], F32, tag="bn_s")
    if nt > 1:
        nc.vector.tensor_reduce(ssum[:], acc[:], mybir.AxisListType.X,
                                mybir.AluOpType.add)
    else:
        nc.vector.tensor_copy(ssum[:], acc[:])
    sq = g.sbuf.tile([C, 1], F32, tag="bn_sq")
    scr = g.scratch  # (128, 2080) f32 scratch
    nc.scalar.activation(scr[0:C, 0:npx], dsf32[0:C].opt(),
                         mybir.ActivationFunctionType.Square,
                         accum_out=sq[:])
    inv_n = 1.0 / npx
    mean = g.sbuf.tile([C, 1], F32, tag="bn_m")
    nc.scalar.mul(mean[:], ssum[:], inv_n)
    ex2 = g.sbuf.tile([C, 1], F32, tag="bn_e")
    nc.scalar.mul(ex2[:], sq[:], inv_n)
    m2 = g.sbuf.tile([C, 1], F32, tag="bn_m2")
    nc.vector.tensor_mul(m2[:], mean[:], mean[:])
    var = g.sbuf.tile([C, 1], F32, tag="bn_v")
    nc.vector.tensor_sub(var[:], ex2[:], m2[:])
    nc.vector.tensor_scalar_add(var[:], var[:], EPS)
    std = g.sbuf.tile([C, 1], F32, tag="bn_std")
    nc.scalar.activation(std[:], var[:], mybir.ActivationFunctionType.Sqrt,
                         bias=0.0, scale=1.0)
    istd = g.sbuf.tile([C, 1], F32, tag="bn_istd")
    nc.vector.reciprocal(istd[:], std[:])
    s = g.sbuf.tile([C, 1], F32, tag="bn_sc")
    nc.vector.tensor_mul(s[:], g_ap, istd[:])
    ms = g.sbuf.tile([C, 1], F32, tag="bn_ms")
    nc.vector.tensor_mul(ms[:], mean[:], s[:])
    t = g.sbuf.tile([C, 1], F32, tag="bn_t")
    nc.vector.tensor_sub(t[:], bb_ap, ms[:])
    nc.vector.tensor_scalar(ds16_dst[:], dsf32[0:C].opt(), s[:], t[:],
                            mybir.AluOpType.mult, mybir.AluOpType.add)


def emit_matvec_op(g, w_ap, nk, nm, rhs_cols, biasrow_ap, act, out16, psum_tag):
    """out-on-partitions matvec: w_ap (128, nk*nm*128) blocks; rhs_cols
    (128, nk) fp16; psum (128, nm); biasrow (1, 128*nm) f16 folded via
    ones-matmul; act lrelu or none; out16 (128, nm) fp16 (or f32)."""
    nc = g.nc
    ps = g.psum.tile([128, nm], F32, tag="mm")
    for m in range(nm):
        for k in range(nk):
            nc.tensor.matmul(ps[:, m:m + 1],
                             w_ap[:, (k * nm + m) * 128:(k * nm + m) * 128 + 128],
                             rhs_cols[:, k:k + 1],
                             start=(k == 0), stop=False)
        nc.tensor.matmul(ps[:, m:m + 1], biasrow_ap[0:1, m * 128:(m + 1) * 128],
                         g.ones[0:1, 0:1], start=False, stop=True)
    emit_act(g, out16[:], ps, 128, nm, act)


def build_program():
    nc = bacc.Bacc("TRN2", target_bir_lowering=False, debug=False,
                   num_devices=N_CORES)
    g = Ctx()
    g.nc = nc

    def inp(name, shape, dt):
        return nc.dram_tensor(name, list(shape), dt, kind="ExternalInput").ap()

    # --- declare I/O ---
    I = {}
    I['xpatch'] = inp('xpatch', (9, 2080), F16)
    I['pack16'] = inp('pack16', (128, TOT16), F16)
    I['pack32'] = inp('pack32', (128, TOT32), F32)
    I['lt1w'] = inp('lt1w', (128, 136 * 64), F16)
    I['ltb1cr'] = inp('ltb1cr', (1, 64), F16)
    I['rev3w'] = inp('rev3w', (128, 4 * 42 * 128), F16)
    I['rev3br'] = inp('rev3br', (1, 5376), F32)
    for i, (wd, nk, nt) in enumerate([(WD0, NK_LT1, 17), (WD1, 25, 25)]):
        p = f'd{i}_'
        I[p + 'fb1r'] = inp(p + 'fb1r', (1, 64), F16)
        I[p + 'fw1'] = inp(p + 'fw1', (128, nk * 64), F16)
        I[p + 'fw3'] = inp(p + 'fw3', (128, 2 * nt * 128), F16)

    O = {}
    O['d0'] = nc.dram_tensor('d0', [H, 64], F32, kind="ExternalOutput").ap()
    O['d1'] = nc.dram_tensor('d1', [H, 96], F32, kind="ExternalOutput").ap()
    O['m0'] = nc.dram_tensor('m0', [H, 1], F32, kind="ExternalOutput").ap()
    O['m1'] = nc.dram_tensor('m1', [H, 2], F32, kind="ExternalOutput").ap()

    # internal DRAM
    e0_dram = nc.dram_tensor('e0_dram', [FLAT0], F16)
    z1p_dram = nc.dram_tensor('z1p_dram', [64], F32)
    z1r_dram = nc.dram_tensor('z1r_dram', [512], F32, addr_space="Shared")
    rloc_dram = nc.dram_tensor('rloc_dram', [5376], F16)
    rall_dram = nc.dram_tensor('rall_dram', [43008], F16, addr_space="Shared")
    hh_dram = [nc.dram_tensor(f'hh{i}_dram', [128 * (NK_LT1, 25)[i]], F16)
               for i in range(2)]
    zf1_dram = nc.dram_tensor('zf1_dram', [128], F32)
    zfall_dram = nc.dram_tensor('zfall_dram', [1024], F32, addr_space="Shared")
    y_dram = [nc.dram_tensor(f'y{i}_dram', [128 * (17, 25)[i]], F32)
              for i in range(2)]

    rg = [list(range(N_CORES))]

    with tile.TileContext(nc) as tc:
        with (
            tc.tile_pool(name="sbuf", bufs=1) as sbuf,
            tc.tile_pool(name="wstream", bufs=2) as wstream,
            tc.tile_pool(name="psum", bufs=3, space="PSUM") as psum,
        ):
            g.sbuf, g.psum = sbuf, psum
            D = I
            I = {}
            for _n, _ap in D.items():
                if _n in ('rev3w', 'lt1w', 'xpatch', 'pack16', 'pack32'):
                    continue
                _t = sbuf.tile(list(_ap.shape), _ap.dtype, tag="in_" + _n)
                nc.sync.dma_start(_t[:], _ap)
                I[_n] = _t
            pk16 = sbuf.tile([128, TOT16], F16, tag="pack16")
            nc.sync.dma_start(pk16[:], D['pack16'])
            pk32 = sbuf.tile([128, TOT32], F32, tag="pack32")
            nc.sync.dma_start(pk32[:], D['pack32'])
            for _n, (_o, _p, _w) in OFF16.items():
                I[_n] = pk16[0:_p, _o:_o + _w]
            for _n, (_o, _p, _w) in OFF32.items():
                I[_n] = pk32[0:_p, _o:_o + _w]
            g.ident = I['ident']
            g.ones = I['ones']
            g.scratch = sbuf.tile([128, 800], F32, tag="scratch")

            # ================= ENCODER =================
            B0 = sbuf.tile([32, 34, 67], F16, tag="big1")
            nc.gpsimd.memset(B0[:], 0.0)
            # L0: K=9 im2col; row tiles of 7; patches streamed per tile
            y0 = 0
            while y0 < 32:
                nr = min(7, 32 - y0)
                xp = wstream.tile([9, 512], F16, tag="xp")
                nc.sync.dma_start(xp[0:9, 0:nr * 65],
                                  D['xpatch'][:, y0 * 65:(y0 + nr) * 65])
                ps = psum.tile([32, nr, 65], F32, tag="mm")
                nc.tensor.matmul(ps[:], I['ew0'], xp[0:9, 0:nr * 65],
                                 start=True, stop=False)
                nc.tensor.matmul(ps[:], I['eb0r'], g.ones[0:1, 0:nr * 65],
                                 start=False, stop=True)
                emit_act(g, B0[0:32, 1 + y0:1 + y0 + nr, 1:66], ps, 32, nr * 65,
                         'lrelu')
                y0 += nr
            # rb1 (32->64, s2): c1
            B1 = sbuf.tile([64, 18, 35], F16, tag="B1")
            nc.gpsimd.memset(B1[:], 0.0)
            emit_conv(g, 'e_rb1c1', B0, B1, 32, 64, 32, 65, 2, I['ew11'],
                      I['eb11r'], 'lrelu', rows_per_tile=8)
            # rb1 ds (1x1 s2) + bn
            ds_tiles = []
            for (ty, nr) in [(0, 8), (8, 8)]:
                ps = psum.tile([64, nr, 33], F32, tag="mm")
                rhs = B0[0:32, 1 + 2 * ty: 1 + 2 * ty + 2 * nr: 2, 1:67:2]
                nc.tensor.matmul(ps[:], I['ewd1'], rhs, start=True, stop=True)
                ds_tiles.append((ps, ty, nr))
            dsA_f32 = sbuf.tile([64, 16, 33], F32, tag="bigf32")
            dsA16 = sbuf.tile([64, 16, 33], F16, tag="dsA16")
            emit_bn(g, ds_tiles, 64, 528, I['ebd1'], I['eg1'], I['ebn1'],
                    dsA_f32, dsA16)
            # rb1 c2 + identity add
            B2 = sbuf.tile([64, 18, 35], F16, tag="B2")
            nc.gpsimd.memset(B2[:], 0.0)
            emit_conv(g, 'e_rb1c2', B1, B2, 64, 64, 16, 33, 1, I['ew12'],
                      I['eb12r'], 'lrelu', rows_per_tile=8,
                      extra_ident_rhs=dsA16)
            # rb2 (64->128, s2)
            B3 = sbuf.tile([128, 10, 19], F16, tag="B3")
            nc.gpsimd.memset(B3[:], 0.0)
            emit_conv(g, 'e_rb2c1', B2, B3, 64, 128, 16, 33, 2, I['ew21'],
                      I['eb21r'], 'lrelu')
            ps = psum.tile([128, 8, 17], F32, tag="mm")
            nc.tensor.matmul(ps[:], I['ewd2'], B2[0:64, 1:17:2, 1:35:2],
                             start=True, stop=True)
            dsB_f32 = sbuf.tile([128, 8, 17], F32, tag="dsB_f32")
            dsB16 = sbuf.tile([128, 8, 17], F16, tag="dsB16")
            emit_bn(g, [(ps, 0, 8)], 128, 136, I['ebd2'], I['eg2'], I['ebn2'],
                    dsB_f32, dsB16)
            B4 = sbuf.tile([128, 10, 19], F16, tag="B4")
            nc.gpsimd.memset(B4[:], 0.0)
            emit_conv(g, 'e_rb2c2', B3, B4, 128, 128, 8, 17, 1, I['ew22'],
                      I['eb22r'], 'lrelu', extra_ident_rhs=dsB16)
            # rb3 (128->128, s1, no ds)
            B5 = sbuf.tile([128, 10, 19], F16, tag="B5")
            nc.gpsimd.memset(B5[:], 0.0)
            emit_conv(g, 'e_rb3c1', B4, B5, 128, 128, 8, 17, 1, I['ew31'],
                      I['eb31r'], 'lrelu')
            B6 = sbuf.tile([128, 10, 19], F16, tag="B6")
            nc.gpsimd.memset(B6[:], 0.0)
            emit_conv(g, 'e_rb3c2', B5, B6, 128, 128, 8, 17, 1, I['ew32'],
                      I['eb32r'], 'lrelu', extra_ident_rhs=B4[0:128, 1:9, 1:18])

            # e0 export + reload as k-chunk columns (full 136 chunks)
            nc.sync.dma_start(e0_dram.ap(), B6[0:128, 1:9, 1:18])
            e0c = sbuf.tile([128, 136], F16, tag="e0c")
            e0r = e0_dram.ap().rearrange("(a b) -> b a", b=128)
            nc.sync.dma_start(e0c[:], e0r)

            # ====== LT1 output-sharded (64 outputs per core) + AllGather ======
            psz = psum.tile([64, 1], F32, tag="mm")
            for kb in range(4):
                lt1b = wstream.tile([128, 34 * 64], F16, tag="lt1b")
                nc.sync.dma_start(lt1b[:], D['lt1w'][:, kb * 2176:(kb + 1) * 2176])
                for kk in range(34):
                    k = 34 * kb + kk
                    nc.tensor.matmul(psz[:], lt1b[:, kk * 64:(kk + 1) * 64],
                                     e0c[:, k:k + 1],
                                     start=(k == 0), stop=False)
            nc.tensor.matmul(psz[:], I['ltb1cr'], g.ones[0:1, 0:1],
                             start=False, stop=True)
            z1p = sbuf.tile([64, 1], F32, tag="z1p")
            emit_act(g, z1p[:], psz, 64, 1, 'lrelu')
            nc.sync.dma_start(z1p_dram.ap(), z1p[:])
            nc.gpsimd.collective_compute(
                "AllGather", mybir.AluOpType.bypass, replica_groups=rg,
                ins=[z1p_dram.ap()], outs=[z1r_dram.ap()])
            z1g = sbuf.tile([128, 4], F32, tag="z1g")
            nc.sync.dma_start(z1g[:], z1r_dram.ap().rearrange("(a b) -> b a", b=128))
            z16 = sbuf.tile([128, 4], F16, tag="z16")
            nc.vector.tensor_copy(z16[:], z1g[:])

            # ================= mids =================
            mids = [('mw2', 'mb2', 4, 2), ('mw3', 'mb3', 2, 1),
                    ('mw4', 'mb4', 1, 2), ('mw5', 'mb5', 2, 4)]
            zcur = z16
            for wn, bn, nk, nm in mids:
                wt = I[wn]
                znext = sbuf.tile([128, nm], F16, tag=wn + "_z")
                emit_matvec_op(g, wt, nk, nm, zcur, I[bn + 'r'], 'lrelu', znext, "mid")
                zcur = znext

            # ================= rev3 + AllGather =================
            # rhs-streaming, nt-major blocks; per-tile DMA out to dram
            NT_R3 = [512] * 10 + [256]
            off = 0
            for wnt in NT_R3:
                wck = wstream.tile([128, 4 * 512], F16, tag="rev3wc")
                nc.sync.dma_start(wck[0:128, 0:4 * wnt],
                                  D['rev3w'][:, 4 * off:4 * off + 4 * wnt])
                ps = psum.tile([1, wnt], F32, tag="mm")
                for k in range(4):
                    nc.tensor.matmul(ps[:], zcur[:, k:k + 1],
                                     wck[0:128, k * wnt:(k + 1) * wnt],
                                     start=(k == 0), stop=(k == 3))
                rsb = wstream.tile([1, 512], F16, tag="rsb")
                nc.vector.scalar_tensor_tensor(
                    rsb[0:1, 0:wnt], ps[:], 1.0,
                    I['rev3br'][0:1, off:off + wnt],
                    mybir.AluOpType.mult, mybir.AluOpType.add)
                nc.sync.dma_start(rloc_dram.ap()[off:off + wnt], rsb[0:1, 0:wnt])
                off += wnt
            nc.gpsimd.collective_compute(
                "AllGather", mybir.AluOpType.bypass, replica_groups=rg,
                ins=[rloc_dram.ap()], outs=[rall_dram.ap()])

            # ================= decoders: conv chains + fc1 =================
            zf1both = sbuf.tile([128, 1], F32, tag="zf1both")
            WDM = WD1
            sIn = sbuf.tile([128, 4, WDM + 2], F16, tag="d_sIn")
            A1 = sbuf.tile([64, 4, WDM + 2], F16, tag="d_A1")
            A2 = sbuf.tile([64, 4, WDM + 2], F16, tag="d_A2")
            A3 = sbuf.tile([64, 4, WDM + 2], F16, tag="d_A3")
            B1d = sbuf.tile([64, 6, 2 * WDM + 2], F16, tag="d_B1d")
            C1 = sbuf.tile([32, 6, 2 * WDM + 2], F16, tag="d_C1")
            C2 = sbuf.tile([32, 6, 2 * WDM + 2], F16, tag="d_C2")
            D1 = sbuf.tile([32, 10, 4 * WDM + 2], F16, tag="big1")
            z1sh = sbuf.tile([8, 3, 4 * WDM + 2], F16, tag="d_z1sh")
            for _b in (sIn, A1, A2, A3, B1d, C1, C2, D1, z1sh):
                nc.gpsimd.memset(_b[:], 0.0)
            for di, wd in enumerate([WD0, WD1]):
                p = f'd{di}_'
                w4 = 4 * wd
                npx3 = 8 * w4 // 4  # = 2*w4? no: level3 pixels = 8 * (4*wd) / 4
                # level sizes: L1 (H=2, wd), L2 (H=4, 2wd), L3 (H=8, 4wd)
                w2 = 2 * wd
                # -- weights
                wts = {wn: I[p + wn] for wn in
                       ['w_in', 'rb1w1', 'rb1w2', 'ct1w', 'rb2w1', 'rb2w2',
                        'rb2ds', 'ct2w', 'rb3w1', 'rb3ds']}
                rb3w2 = I[p + 'rb3w2']

                off = 0 if di == 0 else FLAT0
                rsl = rall_dram.ap()[off:off + 128 * 2 * wd].rearrange(
                    "(c h w) -> c h w", c=128, h=2)
                nc.sync.dma_start(sIn[0:128, 1:3, 1:1 + wd], rsl)
                emit_conv(g, p + 'cin', sIn, A1, 128, 64, 2, wd, 1,
                          wts['w_in'], I[p + 'b_inr'], 'lrelu')
                emit_conv(g, p + 'rb1c1', A1, A2, 64, 64, 2, wd, 1,
                          wts['rb1w1'], I[p + 'rb1b1r'], 'lrelu')
                emit_conv(g, p + 'rb1c2', A2, A3, 64, 64, 2, wd, 1,
                          wts['rb1w2'], I[p + 'rb1b2r'], 'lrelu',
                          extra_ident_rhs=A1[0:64, 1:3, 1:1 + wd])
                # ct1: 64->64, L1 (2, wd) -> L2 (4, 2wd)
                TAPS = {0: [(1, 0)], 1: [(2, 0), (0, 1)]}
                for q in (0, 1):
                    for d in (0, 1):
                        taps = [(ky, kx, dy, dx) for (ky, dy) in TAPS[q]
                                for (kx, dx) in TAPS[d]]
                        ps = psum.tile([64, 2, wd], F32, tag="mm")
                        for mi, (ky, kx, dy, dx) in enumerate(taps):
                            t = 3 * ky + kx
                            rhs = A3[0:64, 1 + dy:3 + dy, 1 + dx:1 + dx + wd]
                            nc.tensor.matmul(ps[:], wts['ct1w'][:, t * 64:(t + 1) * 64],
                                             rhs, start=(mi == 0), stop=False)
                        nc.tensor.matmul(ps[:], I[p + 'ct1br'],
                                         g.ones[0:1, 0:2 * wd],
                                         start=False, stop=True)
                        emit_act(g, B1d[0:64, 1 + q:1 + q + 4:2, 1 + d:1 + d + w2:2],
                                 ps, 64, 2 * wd, 'lrelu')
                # rb2: 64->32 with ds+bn, at L2 (4, w2)
                rpt = 512 // w2
                emit_conv(g, p + 'rb2c1', B1d, C1, 64, 32, 4, w2, 1,
                          wts['rb2w1'], I[p + 'rb2b1r'], 'lrelu', rows_per_tile=rpt)
                ds_tiles = []
                y0 = 0
                while y0 < 4:
                    nr = min(rpt, 4 - y0)
                    ps = psum.tile([32, nr, w2], F32, tag="mm")
                    nc.tensor.matmul(ps[:], wts['rb2ds'],
                                     B1d[0:64, 1 + y0:1 + y0 + nr, 1:1 + w2],
                                     start=True, stop=True)
                    ds_tiles.append((ps, y0, nr))
                    y0 += nr
                dsC_f32 = sbuf.tile([32, 4, w2], F32, tag="bigf32")
                dsC16 = sbuf.tile([32, 4, w2], F16, tag="d_dsC16")
                emit_bn(g, ds_tiles, 32, 4 * w2, I[p + 'rb2dsb'], I[p + 'rb2g'],
                        I[p + 'rb2bb'], dsC_f32, dsC16)
                emit_conv(g, p + 'rb2c2', C1, C2, 32, 32, 4, w2, 1,
                          wts['rb2w2'], I[p + 'rb2b2r'], 'lrelu',
                          rows_per_tile=rpt, extra_ident_rhs=dsC16)
                # ct2: 32->32, L2 (4, w2) -> L3 (8, w4)
                for q in (0, 1):
                    for d in (0, 1):
                        taps = [(ky, kx, dy, dx) for (ky, dy) in TAPS[q]
                                for (kx, dx) in TAPS[d]]
                        y0 = 0
                        while y0 < 4:
                            nr = min(rpt, 4 - y0)
                            ps = psum.tile([32, nr, w2], F32, tag="mm")
                            for mi, (ky, kx, dy, dx) in enumerate(taps):
                                t = 3 * ky + kx
                                rhs = C2[0:32, 1 + y0 + dy:1 + y0 + dy + nr,
                                         1 + dx:1 + dx + w2]
                                nc.tensor.matmul(ps[:], wts['ct2w'][:, t * 32:(t + 1) * 32],
                                                 rhs, start=(mi == 0), stop=False)
                            nc.tensor.matmul(ps[:], I[p + 'ct2br'],
                                             g.ones[0:1, 0:nr * w2],
                                             start=False, stop=True)
                            emit_act(g, D1[0:32, 1 + 2 * y0 + q:1 + 2 * y0 + q + 2 * nr:2,
                                           1 + d:1 + d + w4:2],
                                     ps, 32, nr * w2, 'lrelu')
                            y0 += nr
                # ---- rb3 tail (32 -> 1) at L3 (8, w4) ----
                npx = 8 * w4
                npx2 = npx // 2
                z1f = sbuf.tile([1, npx], F16, tag="d_flat1")
                for hf in range(2):
                    P = sbuf.tile([128, 3, npx2], F16, tag="d_patches")
                    for dy in range(3):
                        for dx in range(3):
                            t = 3 * dy + dx
                            srcw = D1[0:32, dy + 4 * hf:dy + 4 * hf + 4,
                                      dx:dx + w4]
                            nc.sync.dma_start(
                                P[(32 * t) % 128:(32 * t) % 128 + 32,
                                  t // 4, 0:npx2], srcw)
                    n0 = 0
                    while n0 < npx2:
                        nn = min(512, npx2 - n0)
                        ps = psum.tile([1, nn], F32, tag="mm")
                        for j, kr in ((0, 128), (1, 128), (2, 32)):
                            nc.tensor.matmul(ps[:], wts['rb3w1'][0:kr, j:j + 1],
                                             P[0:kr, j, n0:n0 + nn],
                                             start=(j == 0), stop=False)
                        nc.tensor.matmul(ps[:], I[p + 'rb3b1r'],
                                         g.ones[0:1, 0:nn],
                                         start=False, stop=True)
                        emit_act(g, z1f[:, hf * npx2 + n0:hf * npx2 + n0 + nn],
                                 ps, 1, nn, 'lrelu')
                        n0 += nn
                for dy in range(3):
                    p0 = max(0, 1 - dy)
                    p1 = min(8, 9 - dy)
                    r0 = p0 + dy - 1
                    r1 = p1 + dy - 1
                    nc.sync.dma_start(
                        z1sh[p0:p1, dy, 1:1 + w4],
                        z1f[0:1, r0 * w4:r1 * w4].rearrange(
                            "a (h w) -> a h w", w=w4))
                # conv2 1->1 on H-partition layout (DVE); rows pre-shifted
                acc = sbuf.tile([8, w4], F32, tag="d_acc")
                nc.gpsimd.memset(acc[:], 0.0)
                for dy in range(3):
                    for dx in range(3):
                        t = 3 * dy + dx
                        nc.vector.scalar_tensor_tensor(
                            acc[:], z1sh[0:8, dy, dx:dx + w4],
                            rb3w2[:, t:t + 1], acc[:],
                            mybir.AluOpType.mult, mybir.AluOpType.add)
                # ds 32->1 + bn
                dsD = sbuf.tile([1, npx], F16, tag="d_flat2")
                dacc = sbuf.tile([1, 8], F32, tag=p + "dacc")
                for r in range(8):
                    ps = psum.tile([1, w4], F32, tag="mm")
                    nc.tensor.matmul(ps[:], wts['rb3ds'],
                                     D1[0:32, 1 + r, 1:1 + w4],
                                     start=True, stop=True)
                    nc.scalar.activation(dsD[:, r * w4:(r + 1) * w4], ps[:],
                                         mybir.ActivationFunctionType.Identity,
                                         bias=I[p + 'rb3dsb'], scale=1.0,
                                         accum_out=dacc[:, r:r + 1])
                dsum = sbuf.tile([1, 1], F32, tag=p + "dsum")
                nc.vector.tensor_reduce(dsum[:], dacc[:], mybir.AxisListType.X,
                                        mybir.AluOpType.add)
                dacc2 = sbuf.tile([1, 4], F32, tag=p + "dacc2")
                qn = npx // 4
                for qq in range(4):
                    nc.scalar.activation(g.scratch[0:1, 0:qn],
                                         dsD[0:1, qq * qn:(qq + 1) * qn],
                                         mybir.ActivationFunctionType.Square,
                                         accum_out=dacc2[:, qq:qq + 1])
                dsq = sbuf.tile([1, 1], F32, tag=p + "dsq")
                nc.vector.tensor_reduce(dsq[:], dacc2[:], mybir.AxisListType.X,
                                        mybir.AluOpType.add)
                inv_n = 1.0 / npx
                dmean = sbuf.tile([1, 1], F32, tag=p + "dmean")
                nc.scalar.mul(dmean[:], dsum[:], inv_n)
                dex2 = sbuf.tile([1, 1], F32, tag=p + "dex2")
                nc.scalar.mul(dex2[:], dsq[:], inv_n)
                dm2 = sbuf.tile([1, 1], F32, tag=p + "dm2")
                nc.vector.tensor_mul(dm2[:], dmean[:], dmean[:])
                dvar = sbuf.tile([1, 1], F32, tag=p + "dvar")
                nc.vector.tensor_sub(dvar[:], dex2[:], dm2[:])
                nc.vector.tensor_scalar_add(dvar[:], dvar[:], EPS)
                dstd = sbuf.tile([1, 1], F32, tag=p + "dstd")
                nc.scalar.activation(dstd[:], dvar[:],
                                     mybir.ActivationFunctionType.Sqrt,
                                     bias=0.0, scale=1.0)
                distd = sbuf.tile([1, 1], F32, tag=p + "distd")
                nc.vector.reciprocal(distd[:], dstd[:])
                dsc = sbuf.tile([1, 1], F32, tag=p + "dsc")
                nc.vector.tensor_mul(dsc[:], I[p + 'rb3g'], distd[:])
                dms = sbuf.tile([1, 1], F32, tag=p + "dms")
                nc.vector.tensor_mul(dms[:], dmean[:], dsc[:])
                dt_ = sbuf.tile([1, 1], F32, tag=p + "dt")
                nc.vector.tensor_sub(dt_[:], I[p + 'rb3bb'], dms[:])
                nc.vector.tensor_scalar(dsD[:], dsD[:], dsc[:], dt_[:],
                                        mybir.AluOpType.mult, mybir.AluOpType.add)
                dsimg = sbuf.tile([8, w4], F16, tag="d_dsimg")
                nc.gpsimd.dma_start(dsimg[:],
                                    dsD[:].rearrange("a (h w) -> a h w", h=8))
                hsum = sbuf.tile([8, w4], F32, tag="d_hsum")
                nc.vector.scalar_tensor_tensor(hsum[:], acc[:],
                                               I[p + 'rb3b2p8'][:],
                                               dsimg[:],
                                               mybir.AluOpType.add,
                                               mybir.AluOpType.add)
                hh16 = sbuf.tile([8, w4], F16, tag="d_hh16")
                htmp = sbuf.tile([8, w4], F32, tag="d_htmp")
                nc.scalar.mul(htmp[:], hsum[:], ALPHA)
                nc.vector.tensor_max(hh16[:], htmp[:], hsum[:])
                nc.sync.dma_start(
                    hh_dram[di].ap()[0:npx].rearrange("(h w) -> h w", h=8), hh16[:])
                nk = (NK_LT1, 25)[di]
                hT = sbuf.tile([128, nk], F16, tag=p + "hT")
                nc.sync.dma_start(hT[:],
                                  hh_dram[di].ap().rearrange("(a b) -> b a", b=128))
                # fc1 shard: 64 outputs
                fw1 = I[p + 'fw1']
                psf = psum.tile([64, 1], F32, tag="mm")
                for k in range(nk):
                    nc.tensor.matmul(psf[:], fw1[:, k * 64:(k + 1) * 64],
                                     hT[:, k:k + 1], start=(k == 0), stop=False)
                nc.tensor.matmul(psf[:], I[p + 'fb1r'], g.ones[0:1, 0:1],
                                 start=False, stop=True)
                emit_act(g, zf1both[64 * di:64 * di + 64, 0:1], psf, 64, 1,
                         'lrelu')

            # fused fc1 AllGather
            nc.sync.dma_start(zf1_dram.ap(), zf1both[:])
            nc.gpsimd.collective_compute(
                "AllGather", mybir.AluOpType.bypass, replica_groups=rg,
                ins=[zf1_dram.ap()], outs=[zfall_dram.ap()])

            # ================= decoders: fc2/fc3 + masking =================
            for di, (wimg, m) in enumerate([(W0, 1), (W1, 2)]):
                p = f'd{di}_'
                nt = (17, 25)[di]
                zfg = sbuf.tile([128, 4], F32, tag=p + "zfg")
                # zfall[128*c + 64*dec + j]; dec di's vector z[i], i = 64*c + j.
                # dst (p, k) holds z[128k + p]: c = 2k + p//64, j = p%64
                #   -> dram idx = 256k + 128*(p//64) + 64*di + p%64
                zview = zfall_dram.ap().rearrange("(k h j) -> h j k", h=4, j=64)
                # zview[h, j, k] = dram[256k + 64h + j]; need h = 2*(p//64) + di
                for half in range(2):
                    nc.sync.dma_start(
                        zfg[64 * half:64 * half + 64, 0:4],
                        zview[2 * half + di, :, :])
                zfg16 = sbuf.tile([128, 4], F16, tag=p + "zfg16")
                nc.vector.tensor_copy(zfg16[:], zfg[:])
                fw2 = I[p + 'fw2']
                zf2 = sbuf.tile([128, 2], F16, tag=p + "zf2")
                emit_matvec_op(g, fw2, 4, 2, zfg16, I[p + 'fb2r'], 'lrelu',
                               zf2, "mid")
                fw3 = I[p + 'fw3']
                npx3 = nt * 128
                NT3 = [512] * (npx3 // 512) + ([npx3 % 512] if npx3 % 512 else [])
                off = 0
                pos = 0
                for wnt in NT3:
                    ps = psum.tile([1, wnt], F32, tag="mm")
                    for k in range(2):
                        nc.tensor.matmul(ps[:], zf2[:, k:k + 1],
                                         fw3[0:128, pos + k * wnt:pos + (k + 1) * wnt],
                                         start=(k == 0), stop=(k == 1))
                    yfl = wstream.tile([1, 512], F32, tag="yfl")
                    nc.vector.scalar_tensor_tensor(
                        yfl[0:1, 0:wnt], ps[:], 1.0,
                        I[p + 'fb3r'][0:1, off:off + wnt],
                        mybir.AluOpType.mult, mybir.AluOpType.add)
                    nc.sync.dma_start(y_dram[di].ap()[off:off + wnt],
                                      yfl[0:1, 0:wnt])
                    pos += 2 * wnt
                    off += wnt
                ysb = sbuf.tile([H, wimg], F32, tag=p + "ysb")
                nc.sync.dma_start(ysb[:], y_dram[di].ap()[0:H * wimg]
                                  .rearrange("(h w) -> h w", h=H))
                # masking
                nz = sbuf.tile([H, m], F32, tag=p + "nz")
                nc.vector.tensor_scalar(nz[:], ysb[0:H, wimg - m:wimg], 0.0, None,
                                        mybir.AluOpType.is_gt)
                nc.sync.dma_start(O[f'm{di}'], nz[:])
                nzsq = sbuf.tile([H, 32], F32, tag=p + "nzsq")
                nc.gpsimd.memset(nzsq[:], 0.0)
                nc.vector.tensor_copy(nzsq[0:H, 0:m], nz[:])
                nzT = sbuf.tile([H, 32], F32, tag=p + "nzT")
                nc.vector.transpose(nzT[:], nzsq[:])
                AT = sbuf.tile([m + 1, 32], F32, tag=p + "AT")
                nc.sync.dma_start(AT[0:1, :], I['border'][0:1, :])
                nc.sync.dma_start(AT[1:1 + m, :], nzT[0:m, :])
                E = sbuf.tile([m + 1, 32 * (m + 1)], F32, tag=p + "E")
                nc.gpsimd.memset(E[:], 0.0)
                for j in range(m):
                    nc.sync.dma_start(E[j:j + 1, 32 * j:32 * (j + 1)],
                                      nzT[j:j + 1, 0:32])
                nc.sync.dma_start(E[m:m + 1, 32 * m:32 * (m + 1)],
                                  I['ones32f'][0:1, :])
                psm = psum.tile([H, 32 * (m + 1)], F32, tag="mm")
                nc.tensor.matmul(psm[:], AT[:], E[:], start=True, stop=True)
                dout = sbuf.tile([H, 32 * (m + 1)], F32, tag=p + "dout")
                nc.vector.scalar_tensor_tensor(dout[:], ysb[0:H, 0:32 * (m + 1)],
                                               1.0 / S_FC, psm[:],
                                               mybir.AluOpType.mult,
                                               mybir.AluOpType.mult)
                nc.sync.dma_start(O[f'd{di}'], dout[:])

    nc.compile()
    return nc


# ----------------------------------------------------------------------------
# host-side input prep
# ----------------------------------------------------------------------------

def prep_inputs(x, enc0_params, lt_params, rev_params, dec_params):
    """Returns list of 8 per-core input dicts."""
    f32 = lambda a: np.asarray(a, np.float32)
    f16 = lambda a: np.asarray(a, np.float32).astype(NP16)

    base = {}
    # L0 im2col patches from x (pure gather + zero pad)
    xi = f32(x)[0, 0]  # (32, 65)
    xpad = np.zeros((34, 67), np.float32)
    xpad[1:33, 1:66] = xi
    patches = np.zeros((9, 2080), np.float32)
    for dy in range(3):
        for dx in range(3):
            patches[3 * dy + dx] = xpad[dy:dy + 32, dx:dx + 65].reshape(-1)
    base['xpatch'] = f16(patches)

    e = enc0_params
    base['ew0'] = f16(f32(e['w0'])[:, 0].reshape(32, 9).T)
    base['eb0'] = col1(e['b0'])
    base['ew11'] = pack_conv(f32(e['rb1']['w1']))
    base['eb11'] = col1(e['rb1']['b1'])
    base['ew12'] = pack_conv(f32(e['rb1']['w2']))
    base['eb12'] = col1(e['rb1']['b2'])
    base['ewd1'] = f16(f32(e['rb1']['ds_w'])[:, :, 0, 0].T)
    base['ebd1'] = col1(e['rb1']['ds_b'])
    base['eg1'] = col1(e['rb1']['bn_g'])
    base['ebn1'] = col1(e['rb1']['bn_b'])
    base['ew21'] = pack_conv(f32(e['rb2']['w1']))
    base['eb21'] = col1(e['rb2']['b1'])
    base['ew22'] = pack_conv(f32(e['rb2']['w2']))
    base['eb22'] = col1(e['rb2']['b2'])
    base['ewd2'] = f16(f32(e['rb2']['ds_w'])[:, :, 0, 0].T)
    base['ebd2'] = col1(e['rb2']['ds_b'])
    base['eg2'] = col1(e['rb2']['bn_g'])
    base['ebn2'] = col1(e['rb2']['bn_b'])
    base['ew31'] = pack_conv(f32(e['rb3']['w1']))
    base['eb31'] = col1(e['rb3']['b1'])
    base['ew32'] = pack_conv(f32(e['rb3']['w2']))
    base['eb32'] = col1(e['rb3']['b2'])
    base['ident'] = np.eye(128, dtype=NP16)
    base['ones'] = np.ones((1, 512), NP16)
    base['ones32f'] = np.ones((1, 32), np.float32)
    row16 = lambda a, s=1.0: (np.asarray(a, np.float32) * np.float32(s)).reshape(1, -1).astype(NP16)
    base['eb0r'] = row16(e['b0'])
    base['eb11r'] = row16(e['rb1']['b1'])
    base['eb12r'] = row16(e['rb1']['b2'])
    base['eb21r'] = row16(e['rb2']['b1'])
    base['eb22r'] = row16(e['rb2']['b2'])
    base['eb31r'] = row16(e['rb3']['b1'])
    base['eb32r'] = row16(e['rb3']['b2'])
    base['mb2r'] = row16(lt_params['b2'])
    base['mb3r'] = row16(lt_params['b3'])
    base['mb4r'] = row16(rev_params['b1'])
    base['mb5r'] = row16(rev_params['b2'])

    for i, (wn, bn, nk, nm) in enumerate([('mw2', 'mb2', 4, 2), ('mw3', 'mb3', 2, 1),
                                          ('mw4', 'mb4', 1, 2), ('mw5', 'mb5', 2, 4)]):
        src = [lt_params, lt_params, rev_params, rev_params][i]
        key = ['w2', 'w3', 'w1', 'w2'][i]
        w = f32(src[key])          # (out, in)
        b = f32(src[key.replace('w', 'b')])
        base[wn] = pack_matvec(w.T, nk, nm)
        base[bn] = b.reshape(nm, 128).T.copy()

    border = np.ones((1, 32), np.float32)
    border[0, [0, 1, 30, 31]] = 0.0
    base['border'] = border

    # decoder shared (replicated) weights
    for di in range(2):
        d = dec_params[di]
        p = f'd{di}_'
        S = np.float32(S_REV3)
        base[p + 'w_in'] = pack_conv(f32(d['w_in']))
        base[p + 'b_in'] = col1(f32(d['b_in']) * S)
        base[p + 'rb1w1'] = pack_conv(f32(d['rb1']['w1']))
        base[p + 'rb1b1'] = col1(f32(d['rb1']['b1']) * S)
        base[p + 'rb1w2'] = pack_conv(f32(d['rb1']['w2']))
        base[p + 'rb1b2'] = col1(f32(d['rb1']['b2']) * S)
        base[p + 'ct1w'] = pack_convt(f32(d['ct1_w']))
        base[p + 'ct1b'] = col1(f32(d['ct1_b']) * S)
        base[p + 'rb2w1'] = pack_conv(f32(d['rb2']['w1']) / S)
        base[p + 'rb2b1'] = col1(d['rb2']['b1'])
        base[p + 'rb2w2'] = pack_conv(f32(d['rb2']['w2']))
        base[p + 'rb2b2'] = col1(d['rb2']['b2'])
        base[p + 'rb2ds'] = f16(f32(d['rb2']['ds_w'])[:, :, 0, 0].T / S)
        base[p + 'rb2dsb'] = col1(d['rb2']['ds_b'])
        base[p + 'rb2g'] = col1(d['rb2']['bn_g'])
        base[p + 'rb2bb'] = col1(d['rb2']['bn_b'])
        base[p + 'ct2w'] = pack_convt(f32(d['ct2_w']))
        base[p + 'ct2b'] = col1(d['ct2_b'])
        base[p + 'b_inr'] = row16(d['b_in'], S)
        base[p + 'rb1b1r'] = row16(d['rb1']['b1'], S)
        base[p + 'rb1b2r'] = row16(d['rb1']['b2'], S)
        base[p + 'ct1br'] = row16(d['ct1_b'], S)
        base[p + 'rb2b1r'] = row16(d['rb2']['b1'])
        base[p + 'rb2b2r'] = row16(d['rb2']['b2'])
        base[p + 'ct2br'] = row16(d['ct2_b'])
        base[p + 'rb3b1r'] = row16(d['rb3']['b1'])
        base[p + 'fb2r'] = row16(d['fc2_b'], S_FC)
        # rb3: conv1 32->1: flat k = cin + 32*t -> chunks (128, 3)
        w1 = f32(d['rb3']['w1'])  # (1, 32, 3, 3)
        flat = np.zeros(384, np.float32)
        for dy in range(3):
            for dx in range(3):
                t = 3 * dy + dx
                flat[32 * t:32 * t + 32] = w1[0, :, dy, dx]
        base[p + 'rb3w1'] = f16(flat.reshape(3, 128).T)
        base[p + 'rb3b1'] = col1(d['rb3']['b1'])
        w2 = f32(d['rb3']['w2'])[0, 0]  # (3,3)
        base[p + 'rb3w2'] = np.tile(w2.reshape(1, 9), (8, 1)).astype(np.float32)
        base[p + 'rb3b2'] = col1(d['rb3']['b2'])
        base[p + 'rb3b2p8'] = np.full((8, 1), np.float32(np.asarray(d['rb3']['b2']).ravel()[0]), np.float32)
        base[p + 'rb3ds'] = f16(f32(d['rb3']['ds_w'])[:, :, 0, 0].T)
        base[p + 'rb3dsb'] = col1(d['rb3']['ds_b'])
        base[p + 'rb3g'] = col1(d['rb3']['bn_g'])
        base[p + 'rb3bb'] = col1(d['rb3']['bn_b'])
        # fc2 / fc3 (replicated)
        w2f = f32(d['fc2_w'])
        base[p + 'fw2'] = pack_matvec(w2f.T, 4, 2)
        base[p + 'fb2'] = (f32(d['fc2_b']) * S_FC).reshape(2, 128).T.copy()
        nt = (17, 25)[di]
        w3 = f32(d['fc3_w'])      # (2080/3136, 256)
        w3p = np.zeros((nt * 128, 256), np.float32)
        w3p[:w3.shape[0]] = w3
        w3pT = np.ascontiguousarray(w3p.T)    # (256, nt*128)
        npx3 = nt * 128
        fw3 = np.zeros((128, 2 * npx3), NP16)
        pos = 0
        off = 0
        for wnt in [512] * (npx3 // 512) + ([npx3 % 512] if npx3 % 512 else []):
            for k in range(2):
                fw3[:, pos:pos + wnt] = w3pT[128 * k:128 * (k + 1), off:off + wnt]
                pos += wnt
            off += wnt
        base[p + 'fw3'] = fw3
        b3p = np.zeros(nt * 128, np.float32)
        b3p[:w3.shape[0]] = f32(d['fc3_b']) * S_FC
        base[p + 'fb3r'] = b3p.reshape(1, -1)

    # assemble packed small-input tensors (same for all cores)
    pk16 = np.zeros((128, TOT16), NP16)
    for name, (off, pp, ww) in OFF16.items():
        a = np.asarray(base[name], NP16)
        assert a.shape == (pp, ww), (name, a.shape, (pp, ww))
        pk16[:pp, off:off + ww] = a
    base['pack16'] = pk16
    pk32 = np.zeros((128, TOT32), np.float32)
    for name, (off, pp, ww) in OFF32.items():
        a = np.asarray(base[name], np.float32)
        assert a.shape == (pp, ww), (name, a.shape, (pp, ww))
        pk32[:pp, off:off + ww] = a
    base['pack32'] = pk32

    # per-core shards
    W1eff = f32(lt_params['w1'])[:, :FLAT0]    # (512, 17408)
    W1T = W1eff.T                              # (17408, 512)
    W3r = f32(rev_params['w3']) * np.float32(S_REV3)   # (43008, 512)
    b3r = f32(rev_params['b3']) * np.float32(S_REV3)
    in_maps = []
    for c in range(N_CORES):
        m = dict(base)
        # lt1 output-shard: 64 outputs per core; block k = W1T[128k:+128, 64c:+64]
        lt1w = np.zeros((128, 136 * 64), NP16)
        for k in range(136):
            lt1w[:, k * 64:(k + 1) * 64] = W1T[128 * k:128 * (k + 1),
                                               64 * c:64 * (c + 1)]
        m['lt1w'] = lt1w
        m['ltb1c'] = col1(f32(lt_params['b1'])[64 * c:64 * (c + 1)])
        m['ltb1cr'] = f32(lt_params['b1'])[64 * c:64 * (c + 1)].reshape(1, -1).astype(NP16)
        W3c = W3r[5376 * c:5376 * (c + 1)]     # (5376, 512)
        W3cT = np.ascontiguousarray(W3c.T)     # (512, 5376)
        r3 = np.zeros((128, 4 * 42 * 128), NP16)
        off = 0
        pos = 0
        for wnt in [512] * 10 + [256]:
            for k in range(4):
                r3[:, pos:pos + wnt] = W3cT[128 * k:128 * (k + 1), off:off + wnt]
                pos += wnt
            off += wnt
        m['rev3w'] = r3
        m['rev3br'] = b3r[5376 * c:5376 * (c + 1)].reshape(1, -1)
        for di in range(2):
            d = dec_params[di]
            p = f'd{di}_'
            Hh, Wh = 8, (4 * WD0, 4 * WD1)[di]
            fw, fb = build_convout_fold(d['fc1_w'], d['fc1_b'], f32(d['w_out']),
                                        f32(d['b_out']), Hh, Wh)
            fw = fw * np.float32(S_FC)
            fb = fb * np.float32(S_FC)
            rows = fw[64 * c:64 * (c + 1)]     # (64, npx)
            nk = (NK_LT1, 25)[di]
            fwp = np.zeros((128, nk * 64), NP16)
            rT = rows.T                        # (npx, 64)
            for k in range(nk):
                fwp[:, k * 64:(k + 1) * 64] = rT[k * 128:(k + 1) * 128]
            m[p + 'fw1'] = fwp
            m[p + 'fb1'] = col1(fb[64 * c:64 * (c + 1)])
            m[p + 'fb1r'] = fb[64 * c:64 * (c + 1)].reshape(1, -1).astype(NP16)
        in_maps.append(m)
    return in_maps


_CACHE = {}


def kernel(x, enc0_params, lt_params, rev_params, dec_params):
    if 'nc' not in _CACHE:
        _CACHE['nc'] = build_program()
    nc = _CACHE['nc']
    in_maps = prep_inputs(x, enc0_params, lt_params, rev_params, dec_params)
    res = run_bass_kernel_spmd(nc, in_maps, list(range(N_CORES)))
    r0 = res.results[0]
    d0 = np.asarray(r0['d0'], np.float32)
    d1 = np.asarray(r0['d1'], np.float32)
    m0 = np.asarray(r0['m0'], np.float32)
    m1 = np.asarray(r0['m1'], np.float32)
    return d0, d1, m0, m1
